# revision 2
# baseline (speedup 1.0000x reference)
"""Trainium2 Bass kernel for nn_DLFG_79817672229311 (segment_reduce).

Computes, data-parallel over the batch axis on 8 NeuronCores:
  history = [extInfo, ratings, 1]                    # [BS, 20033] per core
  x1 = lrelu(history @ [w1;b1].T); BN folded into w2 on host
  x2..x5 = lrelu(x @ wl.T + bl)
  gen = tanh(x5 @ w6.T + b6)                         # [BS, 65]
  s, cnt = per-row sum / count of nonzero ratings
  addv = s/cnt + gen[:, 64]
  out = gen[:, :64] @ movie_factors.T + addv[:, None] + movie_bias

Design (per core; layer 1 is at the fp8 DoubleRow compute wall ~135us, so
everything else hides under or packs tightly around it):
- Activations ride transposed ([feature, batch]): batch (512) is the matmul
  free dim, features the partition dim, so no on-device transposes are needed.
- History is staged to SBUF once in fp8 (ratings 0..5 are exact in e4m3) with
  a host-appended ones-row; layer 1 runs fp8 DoubleRow against 2^15-pre-scaled
  fp8 [w1;b1] slabs streamed from HBM, so the bias rides the contraction and
  the epilogue is a bias-free paired Lrelu.  K is host-padded to an even tile
  count so every step is a DoubleRow pair.
- BOTH cnt and s accumulate on the Vector engine under the layer-1 shadow as
  paired fused chains (cnt: min+add, s: plain add) into fp16 accumulators
  (integer partial sums <= 790 are exact in fp16; fp16 halves DVE traffic).
  This removes the former PE ones-matmul s-chain (~16us of Tensor time).
  Partition reduction is 4 tiny fp16 matmuls; the ones-row contribution is
  subtracted in the combine.
- Mid layers interleave their k-accumulation across 3 PSUM banks; some lrelu
  epilogues run on the DVE.
- addv bounces through DRAM into per-partition [128, NBT] and enters the
  reconstruction staging op as its per-partition bias.
- Reconstruction exploits PE row tiling (64x128 mode): the contraction is
  only the 64 factors (movie_bias is added on HOST during dequant), so the
  factor block is duplicated into SBUF partitions 64..127 (both in mft and in
  genext2) and each chunk-pair issues two CONCURRENT matmuls -- tile T0
  (SBUF rows 0-63) on the even chunk, tile T8 (rows 64-127) on the odd chunk,
  landing in adjacent PSUM banks.  This halves reconstruction Tensor time.
- Recon staging: fused scale+bias on alternating Vector/Scalar engines ->
  int8 DRAM in 2048-col blocks, with the output drains alternating between
  the Sync and GpSimd DMA queues so descriptor issue isn't single-queue
  limited.  Host dequantizes by the fixed scale 4/127 and adds movie_bias.
"""

import math
import sys

sys.path.insert(0, "/opt/trn_rl_repo")

import numpy as np
import ml_dtypes

BF16 = ml_dtypes.bfloat16
FP8 = ml_dtypes.float8_e4m3

NCORES = 8
BN_EPS = 0.05
SLOPE = 0.01

FULL_CFG = dict(
    BS=512,  # per-core batch
    UINFO=32,
    M=20000,
    F=64,
    DIMS=(1024, 512, 256, 512, 1024, 65),  # fan-outs of the 6 linear layers
    HTC=4,  # history K-tiles per DMA chunk (must be even for DoubleRow pairs)
    W1_SCALE=2.0**15,  # fp8 pre-scale: w1 ~ U(+-0.007) sits in e4m3 subnormals
    MLP_SCALES=(4096.0, 4096.0, 2048.0, 4096.0),  # 2^k per layer, |w|*s < 240
    OUT_DT="i8",  # "i8" (host dequant) or "bf16"
    OUT_SCALE=127.0 / 4.0,  # int8 quantization scale (|out| <= ~3.2)
    SC_DT="f16",  # s/cnt DVE accumulator dtype ("f16" or "f32")
)


def _derived(cfg):
    d = dict(cfg)
    d["KH"] = cfg["UINFO"] + cfg["M"] + 1  # +1 ones-row carrying b1
    t1 = math.ceil(d["KH"] / 128)
    d["T1"] = t1 + (t1 % 2)  # pad to even so all steps are DoubleRow pairs
    d["NBT"] = cfg["BS"] // 128  # batch tiles per core
    d["CHUNKS"] = [(o, min(512, cfg["M"] - o)) for o in range(0, cfg["M"], 512)]
    return d


def build_nc(cfg):
    """Build + compile the (single-core SPMD) Bass program."""
    import concourse.bass as bass
    import concourse.tile as tile
    from concourse import bacc, mybir

    d = _derived(cfg)
    BS, UINFO, M, F = cfg["BS"], cfg["UINFO"], cfg["M"], cfg["F"]
    DIMS = cfg["DIMS"]
    T1, NBT, CHUNKS, HTC = d["T1"], d["NBT"], d["CHUNKS"], cfg["HTC"]
    FO1 = DIMS[0]
    FO1T = FO1 // 128
    w1_unscale = 1.0 / cfg["W1_SCALE"]
    f32 = mybir.dt.float32
    bf16 = mybir.dt.bfloat16
    f16 = mybir.dt.float16
    f8 = mybir.dt.float8e4
    i8 = mybir.dt.int8
    AF = mybir.ActivationFunctionType
    ALU = mybir.AluOpType

    OUT_I8 = cfg["OUT_DT"] == "i8"
    odt = i8 if OUT_I8 else bf16
    OSC = cfg["OUT_SCALE"] if OUT_I8 else 1.0
    scdt = f16 if cfg["SC_DT"] == "f16" else f32

    nc = bacc.Bacc("TRN2", target_bir_lowering=False, debug=False)

    # ---- DRAM I/O ----
    ht_d = nc.dram_tensor("ht", [128, T1, BS], f8, kind="ExternalInput")
    w1t_d = nc.dram_tensor("w1t", [128, T1, FO1], f8, kind="ExternalInput")
    w_d = {}
    for li in range(2, 7):
        fi, fo = DIMS[li - 2], DIMS[li - 1]
        wdt = f8 if li < 6 else bf16
        w_d[li] = nc.dram_tensor(f"w{li}t", [128, fi // 128, fo], wdt, kind="ExternalInput")
    bp_d = {}
    for li in range(2, 6):
        fot = math.ceil(DIMS[li - 1] / 128)
        bp_d[li] = nc.dram_tensor(f"b{li}p", [128, fot], f32, kind="ExternalInput")
    b6_d = nc.dram_tensor("b6p", [128, 1], f32, kind="ExternalInput")
    mft_d = nc.dram_tensor("mft", [128, M], bf16, kind="ExternalInput")
    out_d = nc.dram_tensor("out", [BS, M], odt, kind="ExternalOutput")
    av_d = nc.dram_tensor("av_scr", [BS], f32)  # addv row->partition bounce

    with tile.TileContext(nc) as tc, bass.ExitStack() as ctx:
        const = ctx.enter_context(tc.tile_pool(name="const", bufs=1))
        htp = ctx.enter_context(tc.tile_pool(name="htp", bufs=1))
        w1p = ctx.enter_context(tc.tile_pool(name="w1p", bufs=8))
        actp = ctx.enter_context(tc.tile_pool(name="actp", bufs=1))
        scr = ctx.enter_context(tc.tile_pool(name="scr", bufs=2))
        ost = ctx.enter_context(tc.tile_pool(name="ost", bufs=6))
        psp = ctx.enter_context(tc.tile_pool(name="psp", bufs=4, space="PSUM"))

        # ---- constants in SBUF (dispatched on the Scalar DMA queue) ----
        bp_sb = {}
        for li in range(2, 6):
            fot = math.ceil(DIMS[li - 1] / 128)
            bp_sb[li] = const.tile([128, fot], f32, name=f"b{li}p", tag=f"b{li}p")
            nc.scalar.dma_start(out=bp_sb[li][:], in_=bp_d[li][:])
        b6_sb = const.tile([128, 1], f32, name="b6p", tag="b6p")
        nc.scalar.dma_start(out=b6_sb[:], in_=b6_d[:])
        onesf = const.tile([128, 1], scdt, name="onesf", tag="onesf")
        nc.vector.memset(onesf[:], 1.0)
        # per-partition mask for history tile 0 (extInfo rows excluded)
        rmask = const.tile([128, 1], f32, name="rmask", tag="rmask")
        nc.vector.memset(rmask[:], 1.0)
        nc.vector.memset(rmask[0:UINFO, :], 0.0)

        # ---- layer 1: one pass over history segments ----
        segs = []
        t0 = 0
        for tn in [2, 2]:
            segs.append((t0, tn))
            t0 += tn
        while t0 < T1:
            tn = min(HTC, T1 - t0)
            segs.append((t0, tn))
            t0 += tn
        NSEG = len(segs)

        x1t = actp.tile([128, FO1T, BS], f8, name="x1t", tag="x1t")
        c_acc = const.tile([128, 2, BS], scdt, name="c_acc", tag="c_acc")
        s_acc = const.tile([128, 2, BS], scdt, name="s_acc", tag="s_acc")
        # 4 paired PSUM tiles (2 banks each) -> bias-free paired epilogues
        ps1 = [psp.tile([128, 2, BS], f32, name="ps1", tag="ps") for _ in range(FO1T // 2)]

        nstep = T1 // 2
        step_i = 0
        sc_first = True
        for si_, (ts_, tn) in enumerate(segs):
            htt = htp.tile([128, tn, BS], f8, name="ht", tag="ht", bufs=NSEG)
            nc.sync.dma_start(out=htt[:], in_=ht_d[:, ts_ : ts_ + tn, :])

            lo = 0
            while lo < tn:
                t = ts_ + lo
                w1s = w1p.tile([128, 2, FO1], f8, name="w1s", tag="w1s")
                if step_i == 0:
                    h = FO1 // 2
                    nc.sync.dma_start(out=w1s[:, 0:2, 0:h], in_=w1t_d[:, t : t + 2, 0:h])
                    nc.sync.dma_start(out=w1s[:, 0:2, h:FO1], in_=w1t_d[:, t : t + 2, h:FO1])
                else:
                    nc.sync.dma_start(out=w1s[:, 0:2, :], in_=w1t_d[:, t : t + 2, :])
                for fo in range(FO1T):
                    fsl = slice(fo * 128, (fo + 1) * 128)
                    pdst = ps1[fo // 2][:, fo % 2, :]
                    nc.tensor.matmul(
                        pdst,
                        lhsT=w1s[:, 0:2, fsl],
                        rhs=htt[:, lo : lo + 2, :],
                        start=(step_i == 0),
                        stop=(step_i == nstep - 1),
                        perf_mode=mybir.MatmulPerfMode.DoubleRow,
                    )
                step_i += 1
                lo += 2

            # cnt AND s accumulation on the DVE under the layer-1 shadow:
            # cnt is a fused min+add chain, s a plain add chain, both on
            # paired tiles into fp16 accumulators (exact for these integer
            # sums).  Segment 0 holds extInfo rows; init via masked ops.
            if sc_first:
                assert tn == 2
                nc.vector.tensor_scalar(
                    c_acc[:, 0, :], htt[:, 0, :], 1.0, rmask[:], op0=ALU.min, op1=ALU.mult
                )
                nc.vector.tensor_scalar(
                    c_acc[:, 1, :], htt[:, 1, :], 1.0, None, op0=ALU.min
                )
                nc.vector.tensor_scalar(
                    s_acc[:, 0, :], htt[:, 0, :], 1.0, rmask[:], op0=ALU.mult, op1=ALU.mult
                )
                nc.vector.tensor_copy(s_acc[:, 1, :], htt[:, 1, :])
                sc_first = False
            else:
                o = 0
                while o < tn:
                    nc.vector.scalar_tensor_tensor(
                        c_acc[:], htt[:, o : o + 2, :], 1.0, c_acc[:],
                        op0=ALU.min, op1=ALU.add,
                    )
                    nc.vector.tensor_add(s_acc[:], htt[:, o : o + 2, :], s_acc[:])
                    o += 2

        # layer-1 epilogue (bias-free: bias rode the matmul via the ones-row)
        for j in range(FO1T // 2):
            nc.scalar.activation(
                x1t[:, 2 * j : 2 * j + 2, :], ps1[j][:], AF.Lrelu,
                scale=w1_unscale, alpha=SLOPE,
            )

        # ---- remaining weights + movie factors: emitted late on the Sync
        # queue so the layer-1 ht/w1 stream gets the DMA bandwidth first.
        w_sb = {}
        for li in range(2, 7):
            fi, fo = DIMS[li - 2], DIMS[li - 1]
            wdt = f8 if li < 6 else bf16
            w_sb[li] = const.tile([128, fi // 128, fo], wdt, name=f"w{li}t", tag=f"w{li}t")
            nc.sync.dma_start(out=w_sb[li][:], in_=w_d[li][:])
        mft = const.tile([128, M], bf16, name="mft", tag="mft")
        nc.sync.dma_start(out=mft[:], in_=mft_d[:])

        # ---- layers 2..5 (lrelu) ----
        xin = x1t
        for li in range(2, 6):
            fi, fo = DIMS[li - 2], DIMS[li - 1]
            fit, fot = fi // 128, fo // 128
            xdt = f8 if li < 5 else bf16
            unsc = 1.0 / cfg["MLP_SCALES"][li - 2]
            xout = actp.tile(
                [128, fot, BS], xdt, name=f"x{li}t",
                tag=("x1t" if li == 5 else "x2t" if li == 4 else f"x{li}t"),
            )
            # interleave the k-accumulation across up to 3 fo-tile banks so
            # consecutive matmuls don't serialize on one bank's drain
            for g0 in range(0, fot, 3):
                gn = min(3, fot - g0)
                pss = [psp.tile([128, BS], f32, name="ps", tag="ps") for _ in range(gn)]
                ki = 0
                while ki < fit:
                    n2 = 2 if ki + 2 <= fit else 1
                    for j in range(gn):
                        ft = g0 + j
                        if n2 == 2:
                            nc.tensor.matmul(
                                pss[j][:],
                                lhsT=w_sb[li][:, ki : ki + 2, ft * 128 : (ft + 1) * 128],
                                rhs=xin[:, ki : ki + 2, :],
                                start=(ki == 0),
                                stop=(ki + 2 == fit),
                                perf_mode=mybir.MatmulPerfMode.DoubleRow,
                            )
                        else:
                            nc.tensor.matmul(
                                pss[j][:],
                                lhsT=w_sb[li][:, ki, ft * 128 : (ft + 1) * 128],
                                rhs=xin[:, ki, :],
                                start=(ki == 0),
                                stop=True,
                            )
                    ki += n2
                for j in range(gn):
                    ft = g0 + j
                    if li >= 4 and fot >= 4 and j == 2:
                        # offload some lrelu epilogues to the DVE (2-op form:
                        # z = ps*unsc + b; x = max(z*slope, z))
                        tmp = scr.tile([128, BS], f32, name="tmp", tag="tmp")
                        nc.vector.tensor_scalar(
                            tmp[:], pss[j][:], unsc, bp_sb[li][:, ft : ft + 1],
                            op0=ALU.mult, op1=ALU.add,
                        )
                        nc.vector.scalar_tensor_tensor(
                            xout[:, ft, :], tmp[:], SLOPE, tmp[:],
                            op0=ALU.mult, op1=ALU.max,
                        )
                    else:
                        nc.scalar.activation(
                            xout[:, ft, :], pss[j][:], AF.Lrelu,
                            bias=bp_sb[li][:, ft : ft + 1], scale=unsc, alpha=SLOPE,
                        )
            xin = xout

        # ---- s/cnt partition reduces + meanV combine, emitted before layer
        # 6 so meanV (pre-scaled by the output quant scale) is ready before
        # the reconstruction staging needs addv.
        c_red = psp.tile([1, 2, BS], f32, name="c_red", tag="ps")
        s_red = psp.tile([1, 2, BS], f32, name="s_red", tag="ps")
        nc.tensor.matmul(c_red[:, 0, :], lhsT=onesf[:], rhs=c_acc[:, 0, :], start=True, stop=True)
        nc.tensor.matmul(c_red[:, 1, :], lhsT=onesf[:], rhs=c_acc[:, 1, :], start=True, stop=True)
        nc.tensor.matmul(s_red[:, 0, :], lhsT=onesf[:], rhs=s_acc[:, 0, :], start=True, stop=True)
        nc.tensor.matmul(s_red[:, 1, :], lhsT=onesf[:], rhs=s_acc[:, 1, :], start=True, stop=True)
        # the host-appended ones-row landed in chain half 0 of both chains:
        # subtract its +1 per batch from cnt and from s.
        c0_sb = const.tile([1, BS], f32, name="c0_sb", tag="c0_sb")
        nc.vector.tensor_scalar_sub(c0_sb[:], c_red[0:1, 0, :], 1.0)
        c_sb = const.tile([1, BS], f32, name="c_sb", tag="c_sb")
        nc.vector.tensor_add(c_sb[:], c0_sb[:], c_red[0:1, 1, :])
        rc_sb = const.tile([1, BS], f32, name="rc_sb", tag="rc_sb")
        nc.vector.reciprocal(rc_sb[:], c_sb[:])
        # s pre-scaled by OSC while combining the two chain halves
        s_sb = const.tile([1, BS], f32, name="s_sb", tag="s_sb")
        nc.vector.tensor_scalar(
            s_sb[:], s_red[0:1, 0, :], -1.0, float(OSC), op0=ALU.add, op1=ALU.mult
        )
        nc.vector.scalar_tensor_tensor(
            s_sb[:], s_red[0:1, 1, :], float(OSC), s_sb[:], op0=ALU.mult, op1=ALU.add
        )
        mv_sb = const.tile([1, BS], f32, name="mv_sb", tag="mv_sb")
        nc.vector.tensor_mul(mv_sb[:], rc_sb[:], s_sb[:])

        # ---- layer 6 (tanh) -> genf [65, BS] f32 ----
        fi, fo = DIMS[4], DIMS[5]
        fit = fi // 128
        assert fo == F + 1
        ps6 = psp.tile([fo, BS], f32, name="ps6", tag="ps")
        for ki in range(fit):
            nc.tensor.matmul(
                ps6[:],
                lhsT=w_sb[6][:, ki, 0:fo],
                rhs=xin[:, ki, :],
                start=(ki == 0),
                stop=(ki == fit - 1),
            )
        genf = actp.tile([fo, BS], f32, name="genf", tag="genf")
        nc.scalar.activation(genf[:], ps6[:], AF.Tanh, bias=b6_sb[0:fo, 0:1], scale=1.0)

        # ---- genext2: factor rows in bf16, duplicated into partitions
        # 64..127 (via SBUF->SBUF DMA) so reconstruction can row-tile the PE.
        genext2 = actp.tile([128, BS], bf16, name="genext2", tag="genext")
        nc.vector.tensor_copy(genext2[0:F, :], genf[0:F, :])
        nc.sync.dma_start(out=genext2[F : 2 * F, :], in_=genext2[0:F, :])
        gl_sb = const.tile([1, BS], f32, name="gl_sb", tag="gl_sb")
        nc.sync.dma_start(out=gl_sb[:], in_=genf[F : F + 1, :])

        # addv = meanV*OSC + gen_last*OSC, bounced through DRAM into
        # per-partition layout [128, NBT] for the staging ops.
        av_sb = const.tile([1, BS], f32, name="av_sb", tag="av_sb")
        nc.vector.scalar_tensor_tensor(
            av_sb[:], gl_sb[:], float(OSC), mv_sb[:], op0=ALU.mult, op1=ALU.add
        )
        nc.sync.dma_start(out=av_d[:], in_=av_sb[0:1, :])
        addv_t = const.tile([128, NBT], f32, name="addv_t", tag="addv_t")
        nc.sync.dma_start(out=addv_t[:], in_=av_d.ap().rearrange("(t p) -> p t", p=128))

        # ---- reconstruction: out[bt*128+p, m] over movie chunk-pairs.
        # PE in 64x128 row-tiled mode: tile T0 (SBUF partitions 0-63) runs
        # the even chunk, tile T8 (64-127, the duplicated factor rows) the
        # odd chunk CONCURRENTLY, into adjacent PSUM banks.
        PAIRS = [CHUNKS[i : i + 2] for i in range(0, len(CHUNKS), 2)]
        for bt in range(NBT):
            lhsT_lo = genext2[0:F, bt * 128 : (bt + 1) * 128]
            lhsT_hi = genext2[F : 2 * F, bt * 128 : (bt + 1) * 128]
            st = None
            for pi, pair in enumerate(PAIRS):
                eng = 0 if (pi % 9) in (0, 2, 4, 6) else 1  # 11:9 Scalar:Vector
                pr = psp.tile([128, 2, 512], f32, name="pr", tag="ps")
                for j, (co, cw) in enumerate(pair):
                    nc.tensor.matmul(
                        pr[:, j, 0:cw],
                        lhsT=(lhsT_lo if j == 0 else lhsT_hi),
                        rhs=(mft[0:F, co : co + cw] if j == 0 else mft[F : 2 * F, co : co + cw]),
                        start=True, stop=True,
                    )
                pw = sum(cw for _, cw in pair)
                if pi % 2 == 0:
                    st = ost.tile([128, 2048], odt, name="st", tag="st")
                    so, po = 0, pair[0][0]
                # stage the full [2,512] pair; only the valid prefix is DMA'd
                nst = 1024 if pw == 1024 else 512 + pair[1][1]
                pr2d = pr[:].opt()  # [128, 2, 512] -> contiguous [128, 1024]
                if eng == 0:
                    nc.vector.tensor_scalar(
                        st[:, so : so + 1024], pr2d, OSC, addv_t[:, bt : bt + 1],
                        op0=ALU.mult, op1=ALU.add,
                    )
                else:
                    nc.scalar.activation(
                        st[:, so : so + 1024], pr2d, AF.Identity,
                        bias=addv_t[:, bt : bt + 1], scale=OSC,
                    )
                so += nst
                if pi % 2 == 1 or pi == len(PAIRS) - 1:
                    q = nc.sync if (pi // 2) % 2 == 0 else nc.gpsimd
                    q.dma_start(
                        out=out_d[bt * 128 : (bt + 1) * 128, po : po + so],
                        in_=st[:, 0:so],
                    )

    nc.compile()
    return nc


def prep_in_maps(cfg, inputs):
    """Shard + lay out the full inputs into per-core DRAM input maps."""
    d = _derived(cfg)
    BS, UINFO, M, F, DIMS, T1 = cfg["BS"], cfg["UINFO"], cfg["M"], cfg["F"], cfg["DIMS"], d["T1"]
    extInfo = np.asarray(inputs["extInfo"], np.float32)
    ratings = np.asarray(inputs["ratings"], np.float32)

    # BN (eval) fold into layer 2: y = g'(lrelu1) + b' with g' = bn_g/sqrt(1+eps)
    g = np.asarray(inputs["bn_g"], np.float32) / np.float32(np.sqrt(1.0 + BN_EPS))
    bnb = np.asarray(inputs["bn_b"], np.float32)
    w2 = np.asarray(inputs["w2"], np.float32)
    w2f = w2 * g[None, :]
    b2f = np.asarray(inputs["b2"], np.float32) + w2 @ bnb

    shared = {}
    # w1t: [KH,FO1] -> padded [T1*128, FO1] -> [128, T1, FO1]; the row at
    # index UINFO+M carries b1 (matching the ones-row in the history).
    w1 = np.asarray(inputs["w1"], np.float32)
    b1 = np.asarray(inputs["b1"], np.float32)
    FO1 = DIMS[0]
    w1tp = np.zeros((T1 * 128, FO1), FP8)
    w1tp[0 : w1.shape[1]] = (w1.T * np.float32(cfg["W1_SCALE"])).astype(FP8)
    w1tp[UINFO + M] = (b1 * np.float32(cfg["W1_SCALE"])).astype(FP8)
    shared["w1t"] = np.ascontiguousarray(w1tp.reshape(T1, 128, FO1).transpose(1, 0, 2))

    def pack_w(wT, fo, dt=BF16, scale=1.0):
        fi = wT.shape[0]
        w = (wT.astype(np.float32) * np.float32(scale)).astype(dt)
        return np.ascontiguousarray(w.reshape(fi // 128, 128, fo).transpose(1, 0, 2))

    scs = cfg["MLP_SCALES"]
    shared["w2t"] = pack_w(w2f.T, DIMS[1], FP8, scs[0])
    for li, wname in ((3, "w3"), (4, "w4"), (5, "w5"), (6, "w6")):
        w = np.asarray(inputs[wname], np.float32)
        fo = DIMS[li - 1]
        if li < 6:
            shared[f"w{li}t"] = pack_w(w.T, fo, FP8, scs[li - 2])
        else:
            shared[f"w{li}t"] = pack_w(w.T, fo)

    def pack_b(b, fo):
        fot = math.ceil(fo / 128)
        bp = np.zeros(fot * 128, np.float32)
        bp[:fo] = b
        return np.ascontiguousarray(bp.reshape(fot, 128).T)

    bsrc = {2: b2f}
    for li in (3, 4, 5):
        bsrc[li] = np.asarray(inputs[f"b{li}"], np.float32)
    for li in range(2, 6):
        shared[f"b{li}p"] = pack_b(bsrc[li], DIMS[li - 1])
    shared["b6p"] = pack_b(np.asarray(inputs["b6"], np.float32), DIMS[5])

    # bf16 mft: factor rows duplicated into partitions 64..127 for the
    # row-tiled reconstruction (movie_bias is added on host at dequant).
    mft = np.zeros((128, M), BF16)
    mft[0:F] = np.asarray(inputs["movie_factors"], np.float32).T.astype(BF16)
    mft[F : 2 * F] = mft[0:F]
    shared["mft"] = mft

    in_maps = []
    for c in range(NCORES):
        sl = slice(c * BS, (c + 1) * BS)
        htc = np.zeros((T1 * 128, BS), FP8)
        htc[0:UINFO] = extInfo[sl].T.astype(FP8)
        htc[UINFO : UINFO + M] = ratings[sl].T.astype(FP8)
        htc[UINFO + M] = np.float32(1.0)  # ones-row: picks up b1 from w1t
        m = dict(shared)
        m["ht"] = np.ascontiguousarray(htc.reshape(T1, 128, BS).transpose(1, 0, 2))
        in_maps.append(m)
    return in_maps


_NC_CACHE = {}


def run_on_hw(cfg, inputs, trace=False):
    from concourse.bass_utils import run_bass_kernel_spmd

    key = tuple(sorted((k, v) for k, v in cfg.items() if k != "DIMS")) + (cfg["DIMS"],)
    if key not in _NC_CACHE:
        _NC_CACHE[key] = build_nc(cfg)
    nc = _NC_CACHE[key]
    in_maps = prep_in_maps(cfg, inputs)
    br = run_bass_kernel_spmd(nc, in_maps, list(range(NCORES)), trace=trace)
    BS, M = cfg["BS"], cfg["M"]
    out = np.empty((NCORES * BS, M), np.float32)
    dq = np.float32(1.0 / cfg["OUT_SCALE"]) if cfg["OUT_DT"] == "i8" else np.float32(1.0)
    mb = np.asarray(inputs["movie_bias"], np.float32)[None, :]
    for c in range(NCORES):
        out[c * BS : (c + 1) * BS] = (
            np.asarray(br.results[c]["out"], dtype=np.float32) * dq + mb
        )
    return out, br


def kernel(**inputs) -> np.ndarray:
    try:
        out, _ = run_on_hw(FULL_CFG, inputs, trace=False)
    except Exception:
        # one retry for transient device/runtime hiccups
        out, _ = run_on_hw(FULL_CFG, inputs, trace=False)
    return out


# revision 7
# speedup vs baseline: 1.0296x; 1.0296x over previous
"""Trainium2 Bass kernel for nn_DLFG_79817672229311 (segment_reduce).

Computes, data-parallel over the batch axis on 8 NeuronCores:
  history = [extInfo, ratings, 1]                    # [BS, 20033] per core
  x1 = lrelu(history @ [w1;b1].T); BN folded into w2 on host
  x2..x5 = lrelu(x @ wl.T + bl)
  gen = tanh(x5 @ w6.T + b6)                         # [BS, 65]
  s, cnt = per-row sum / count of nonzero ratings
  addv = s/cnt + gen[:, 64]
  out = gen[:, :64] @ movie_factors.T + addv[:, None] + movie_bias

Design (per core; layer 1 is at the fp8 DoubleRow compute wall ~135us, so
everything else hides under or packs tightly around it):
- Activations ride transposed ([feature, batch]): batch (512) is the matmul
  free dim, features the partition dim, so no on-device transposes are needed.
- History is staged to SBUF once in fp8 (ratings 0..5 are exact in e4m3) with
  a host-appended ones-row; layer 1 runs fp8 DoubleRow against 2^15-pre-scaled
  fp8 [w1;b1] slabs streamed from HBM, so the bias rides the contraction and
  the epilogue is a bias-free paired Lrelu.  K is host-padded to an even tile
  count so every step is a DoubleRow pair.
- BOTH cnt and s accumulate on the Vector engine under the layer-1 shadow as
  paired fused chains (cnt: min+add, s: plain add) into fp16 accumulators
  (integer partial sums <= 790 are exact in fp16; fp16 halves DVE traffic).
  This removes the former PE ones-matmul s-chain (~16us of Tensor time).
  Partition reduction is 4 tiny fp16 matmuls; the ones-row contribution is
  subtracted in the combine.
- Mid layers interleave their k-accumulation across 3 PSUM banks; some lrelu
  epilogues run on the DVE.
- addv bounces through DRAM into per-partition [128, NBT] and enters the
  reconstruction staging op as its per-partition bias.
- Reconstruction exploits PE row tiling (64x128 mode): the contraction is
  only the 64 factors (movie_bias is added on HOST during dequant), so the
  factor block is duplicated into SBUF partitions 64..127 (both in mft and in
  genext2) and each chunk-pair issues two CONCURRENT matmuls -- tile T0
  (SBUF rows 0-63) on the even chunk, tile T8 (rows 64-127) on the odd chunk,
  landing in adjacent PSUM banks.  This halves reconstruction Tensor time.
- Recon staging: fused scale+bias on alternating Vector/Scalar engines ->
  int8 DRAM in 2048-col blocks, with the output drains alternating between
  the Sync and GpSimd DMA queues so descriptor issue isn't single-queue
  limited.  Host dequantizes by the fixed scale 4/127 and adds movie_bias.
"""

import math
import sys

sys.path.insert(0, "/opt/trn_rl_repo")

import numpy as np
import ml_dtypes

BF16 = ml_dtypes.bfloat16
FP8 = ml_dtypes.float8_e4m3

NCORES = 8
BN_EPS = 0.05
SLOPE = 0.01

FULL_CFG = dict(
    BS=512,  # per-core batch
    UINFO=32,
    M=20000,
    F=64,
    DIMS=(1024, 512, 256, 512, 1024, 65),  # fan-outs of the 6 linear layers
    HTC=4,  # history K-tiles per DMA chunk (must be even for DoubleRow pairs)
    W1_SCALE=2.0**15,  # fp8 pre-scale: w1 ~ U(+-0.007) sits in e4m3 subnormals
    MLP_SCALES=(4096.0, 4096.0, 2048.0, 4096.0),  # 2^k per layer, |w|*s < 240
    OUT_DT="i8",  # "i8" (host dequant) or "bf16"
    OUT_SCALE=127.0 / 4.0,  # int8 quantization scale (|out| <= ~3.2)
    SC_DT="bf16",  # s/cnt DVE accumulator dtype ("bf16", "f16" or "f32")
)


def _derived(cfg):
    d = dict(cfg)
    d["KH"] = cfg["UINFO"] + cfg["M"] + 1  # +1 ones-row carrying b1
    t1 = math.ceil(d["KH"] / 128)
    d["T1"] = t1 + (t1 % 2)  # pad to even so all steps are DoubleRow pairs
    d["NBT"] = cfg["BS"] // 128  # batch tiles per core
    d["CHUNKS"] = [(o, min(512, cfg["M"] - o)) for o in range(0, cfg["M"], 512)]
    return d


def build_nc(cfg):
    """Build + compile the (single-core SPMD) Bass program."""
    import concourse.bass as bass
    import concourse.tile as tile
    from concourse import bacc, mybir

    d = _derived(cfg)
    BS, UINFO, M, F = cfg["BS"], cfg["UINFO"], cfg["M"], cfg["F"]
    DIMS = cfg["DIMS"]
    T1, NBT, CHUNKS, HTC = d["T1"], d["NBT"], d["CHUNKS"], cfg["HTC"]
    FO1 = DIMS[0]
    FO1T = FO1 // 128
    w1_unscale = 1.0 / cfg["W1_SCALE"]
    f32 = mybir.dt.float32
    bf16 = mybir.dt.bfloat16
    f16 = mybir.dt.float16
    f8 = mybir.dt.float8e4
    i8 = mybir.dt.int8
    AF = mybir.ActivationFunctionType
    ALU = mybir.AluOpType

    OUT_I8 = cfg["OUT_DT"] == "i8"
    odt = i8 if OUT_I8 else bf16
    OSC = cfg["OUT_SCALE"] if OUT_I8 else 1.0
    scdt = {"bf16": bf16, "f16": f16, "f32": f32}[cfg["SC_DT"]]

    nc = bacc.Bacc("TRN2", target_bir_lowering=False, debug=False)

    # ---- DRAM I/O ----
    ht_d = nc.dram_tensor("ht", [128, T1, BS], f8, kind="ExternalInput")
    w1t_d = nc.dram_tensor("w1t", [128, T1, FO1], f8, kind="ExternalInput")
    w_d = {}
    for li in range(2, 7):
        fi, fo = DIMS[li - 2], DIMS[li - 1]
        wdt = f8 if li < 6 else bf16
        w_d[li] = nc.dram_tensor(f"w{li}t", [128, fi // 128, fo], wdt, kind="ExternalInput")
    bp_d = {}
    for li in range(2, 6):
        fot = math.ceil(DIMS[li - 1] / 128)
        bp_d[li] = nc.dram_tensor(f"b{li}p", [128, fot], f32, kind="ExternalInput")
    b6_d = nc.dram_tensor("b6p", [128, 1], f32, kind="ExternalInput")
    mft_d = nc.dram_tensor("mft", [128, M], bf16, kind="ExternalInput")
    out_d = nc.dram_tensor("out", [BS, M], odt, kind="ExternalOutput")
    av_d = nc.dram_tensor("av_scr", [BS], f32)  # addv row->partition bounce

    with tile.TileContext(nc) as tc, bass.ExitStack() as ctx:
        const = ctx.enter_context(tc.tile_pool(name="const", bufs=1))
        htp = ctx.enter_context(tc.tile_pool(name="htp", bufs=1))
        w1p = ctx.enter_context(tc.tile_pool(name="w1p", bufs=8))
        actp = ctx.enter_context(tc.tile_pool(name="actp", bufs=1))
        ost = ctx.enter_context(tc.tile_pool(name="ost", bufs=6))
        psp = ctx.enter_context(tc.tile_pool(name="psp", bufs=4, space="PSUM"))

        # ---- constants in SBUF (dispatched on the Scalar DMA queue) ----
        bp_sb = {}
        for li in range(2, 6):
            fot = math.ceil(DIMS[li - 1] / 128)
            bp_sb[li] = const.tile([128, fot], f32, name=f"b{li}p", tag=f"b{li}p")
            nc.scalar.dma_start(out=bp_sb[li][:], in_=bp_d[li][:])
        b6_sb = const.tile([128, 1], f32, name="b6p", tag="b6p")
        nc.scalar.dma_start(out=b6_sb[:], in_=b6_d[:])
        onesf = const.tile([128, 1], scdt, name="onesf", tag="onesf")
        nc.vector.memset(onesf[:], 1.0)
        # per-partition mask for history tile 0 (extInfo rows excluded)
        rmask = const.tile([128, 1], f32, name="rmask", tag="rmask")
        nc.vector.memset(rmask[:], 1.0)
        nc.vector.memset(rmask[0:UINFO, :], 0.0)

        # ---- layer 1: one pass over history segments ----
        segs = []
        t0 = 0
        for tn in [2, 2]:
            segs.append((t0, tn))
            t0 += tn
        while t0 < T1:
            tn = min(HTC, T1 - t0)
            segs.append((t0, tn))
            t0 += tn
        NSEG = len(segs)

        x1t = actp.tile([128, FO1T, BS], f8, name="x1t", tag="x1t")
        c_acc = const.tile([128, 2, BS], scdt, name="c_acc", tag="c_acc")
        s_acc = const.tile([128, 2, BS], scdt, name="s_acc", tag="s_acc")
        # 4 paired PSUM tiles (2 banks each) -> bias-free paired epilogues
        ps1 = [psp.tile([128, 2, BS], f32, name="ps1", tag="ps") for _ in range(FO1T // 2)]

        nstep = T1 // 2
        step_i = 0
        sc_first = True
        for si_, (ts_, tn) in enumerate(segs):
            htt = htp.tile([128, tn, BS], f8, name="ht", tag="ht", bufs=NSEG)
            nc.sync.dma_start(out=htt[:], in_=ht_d[:, ts_ : ts_ + tn, :])

            lo = 0
            while lo < tn:
                t = ts_ + lo
                w1s = w1p.tile([128, 2, FO1], f8, name="w1s", tag="w1s")
                if step_i == 0:
                    h = FO1 // 2
                    nc.sync.dma_start(out=w1s[:, 0:2, 0:h], in_=w1t_d[:, t : t + 2, 0:h])
                    nc.sync.dma_start(out=w1s[:, 0:2, h:FO1], in_=w1t_d[:, t : t + 2, h:FO1])
                else:
                    nc.sync.dma_start(out=w1s[:, 0:2, :], in_=w1t_d[:, t : t + 2, :])
                for fo in range(FO1T):
                    fsl = slice(fo * 128, (fo + 1) * 128)
                    pdst = ps1[fo // 2][:, fo % 2, :]
                    nc.tensor.matmul(
                        pdst,
                        lhsT=w1s[:, 0:2, fsl],
                        rhs=htt[:, lo : lo + 2, :],
                        start=(step_i == 0),
                        stop=(step_i == nstep - 1),
                        perf_mode=mybir.MatmulPerfMode.DoubleRow,
                    )
                step_i += 1
                lo += 2

            # cnt AND s accumulation on the DVE under the layer-1 shadow:
            # cnt is a fused min+add chain, s a plain add chain, both on
            # paired tiles into fp16 accumulators (exact for these integer
            # sums).  Segment 0 holds extInfo rows; init via masked ops.
            if sc_first:
                assert tn == 2
                nc.vector.tensor_scalar(
                    c_acc[:, 0, :], htt[:, 0, :], 1.0, rmask[:], op0=ALU.min, op1=ALU.mult
                )
                nc.vector.tensor_scalar(
                    c_acc[:, 1, :], htt[:, 1, :], 1.0, None, op0=ALU.min
                )
                nc.vector.tensor_scalar(
                    s_acc[:, 0, :], htt[:, 0, :], 1.0, rmask[:], op0=ALU.mult, op1=ALU.mult
                )
                nc.vector.tensor_copy(s_acc[:, 1, :], htt[:, 1, :])
                sc_first = False
            else:
                o = 0
                while o < tn:
                    nc.vector.scalar_tensor_tensor(
                        c_acc[:], htt[:, o : o + 2, :], 1.0, c_acc[:],
                        op0=ALU.min, op1=ALU.add,
                    )
                    nc.vector.tensor_add(s_acc[:], htt[:, o : o + 2, :], s_acc[:])
                    o += 2

        # layer-1 epilogue (bias-free: bias rode the matmul via the ones-row)
        for j in range(FO1T // 2):
            nc.scalar.activation(
                x1t[:, 2 * j : 2 * j + 2, :], ps1[j][:], AF.Lrelu,
                scale=w1_unscale, alpha=SLOPE,
            )

        # ---- remaining weights + movie factors: emitted late on the Sync
        # queue so the layer-1 ht/w1 stream gets the DMA bandwidth first.
        w_sb = {}
        for li in range(2, 7):
            fi, fo = DIMS[li - 2], DIMS[li - 1]
            wdt = f8 if li < 6 else bf16
            w_sb[li] = const.tile([128, fi // 128, fo], wdt, name=f"w{li}t", tag=f"w{li}t")
            nc.sync.dma_start(out=w_sb[li][:], in_=w_d[li][:])
        mft = const.tile([128, M], bf16, name="mft", tag="mft")
        nc.sync.dma_start(out=mft[:], in_=mft_d[:])

        # ---- layers 2..5 (lrelu) ----
        xin = x1t
        for li in range(2, 6):
            fi, fo = DIMS[li - 2], DIMS[li - 1]
            fit, fot = fi // 128, fo // 128
            xdt = f8 if li < 5 else bf16
            unsc = 1.0 / cfg["MLP_SCALES"][li - 2]
            xout = actp.tile(
                [128, fot, BS], xdt, name=f"x{li}t",
                tag=("x1t" if li == 5 else "x2t" if li == 4 else f"x{li}t"),
            )
            # interleave the k-accumulation across up to 4 fo-tile banks so
            # consecutive matmuls don't serialize on one bank's drain
            for g0 in range(0, fot, 4):
                gn = min(4, fot - g0)
                pss = [psp.tile([128, BS], f32, name="ps", tag="ps") for _ in range(gn)]
                ki = 0
                while ki < fit:
                    n2 = 2 if ki + 2 <= fit else 1
                    for j in range(gn):
                        ft = g0 + j
                        if n2 == 2:
                            nc.tensor.matmul(
                                pss[j][:],
                                lhsT=w_sb[li][:, ki : ki + 2, ft * 128 : (ft + 1) * 128],
                                rhs=xin[:, ki : ki + 2, :],
                                start=(ki == 0),
                                stop=(ki + 2 == fit),
                                perf_mode=mybir.MatmulPerfMode.DoubleRow,
                            )
                        else:
                            nc.tensor.matmul(
                                pss[j][:],
                                lhsT=w_sb[li][:, ki, ft * 128 : (ft + 1) * 128],
                                rhs=xin[:, ki, :],
                                start=(ki == 0),
                                stop=True,
                            )
                    ki += n2
                # epilogues all on the Scalar engine: the DVE queue carries
                # the s/cnt chains during layer 1 and would stall these
                # (strict FIFO), which in turn stalls the next layer's PE.
                for j in range(gn):
                    ft = g0 + j
                    nc.scalar.activation(
                        xout[:, ft, :], pss[j][:], AF.Lrelu,
                        bias=bp_sb[li][:, ft : ft + 1], scale=unsc, alpha=SLOPE,
                    )
            xin = xout

        # ---- s/cnt partition reduces + meanV combine, emitted before layer
        # 6 so meanV (pre-scaled by the output quant scale) is ready before
        # the reconstruction staging needs addv.
        c_red = psp.tile([1, 2, BS], f32, name="c_red", tag="ps")
        s_red = psp.tile([1, 2, BS], f32, name="s_red", tag="ps")
        nc.tensor.matmul(c_red[:, 0, :], lhsT=onesf[:], rhs=c_acc[:, 0, :], start=True, stop=True)
        nc.tensor.matmul(c_red[:, 1, :], lhsT=onesf[:], rhs=c_acc[:, 1, :], start=True, stop=True)
        nc.tensor.matmul(s_red[:, 0, :], lhsT=onesf[:], rhs=s_acc[:, 0, :], start=True, stop=True)
        nc.tensor.matmul(s_red[:, 1, :], lhsT=onesf[:], rhs=s_acc[:, 1, :], start=True, stop=True)
        # the host-appended ones-row landed in chain half 0 of both chains:
        # subtract its +1 per batch from cnt and from s.
        c0_sb = const.tile([1, BS], f32, name="c0_sb", tag="c0_sb")
        nc.vector.tensor_scalar_sub(c0_sb[:], c_red[0:1, 0, :], 1.0)
        c_sb = const.tile([1, BS], f32, name="c_sb", tag="c_sb")
        nc.vector.tensor_add(c_sb[:], c0_sb[:], c_red[0:1, 1, :])
        rc_sb = const.tile([1, BS], f32, name="rc_sb", tag="rc_sb")
        nc.vector.reciprocal(rc_sb[:], c_sb[:])
        # s pre-scaled by OSC while combining the two chain halves
        s_sb = const.tile([1, BS], f32, name="s_sb", tag="s_sb")
        nc.vector.tensor_scalar(
            s_sb[:], s_red[0:1, 0, :], -1.0, float(OSC), op0=ALU.add, op1=ALU.mult
        )
        nc.vector.scalar_tensor_tensor(
            s_sb[:], s_red[0:1, 1, :], float(OSC), s_sb[:], op0=ALU.mult, op1=ALU.add
        )
        mv_sb = const.tile([1, BS], f32, name="mv_sb", tag="mv_sb")
        nc.vector.tensor_mul(mv_sb[:], rc_sb[:], s_sb[:])

        # ---- layer 6 (tanh) -> genf [65, BS] f32 ----
        fi, fo = DIMS[4], DIMS[5]
        fit = fi // 128
        assert fo == F + 1
        ps6 = psp.tile([fo, BS], f32, name="ps6", tag="ps")
        for ki in range(fit):
            nc.tensor.matmul(
                ps6[:],
                lhsT=w_sb[6][:, ki, 0:fo],
                rhs=xin[:, ki, :],
                start=(ki == 0),
                stop=(ki == fit - 1),
            )
        genf = actp.tile([fo, BS], f32, name="genf", tag="genf")
        nc.scalar.activation(genf[:], ps6[:], AF.Tanh, bias=b6_sb[0:fo, 0:1], scale=1.0)

        # ---- genext2: factor rows in bf16, duplicated into partitions
        # 64..127 (via SBUF->SBUF DMA) so reconstruction can row-tile the PE.
        genext2 = actp.tile([128, BS], bf16, name="genext2", tag="genext")
        nc.vector.tensor_copy(genext2[0:F, :], genf[0:F, :])
        nc.sync.dma_start(out=genext2[F : 2 * F, :], in_=genext2[0:F, :])
        gl_sb = const.tile([1, BS], f32, name="gl_sb", tag="gl_sb")
        nc.sync.dma_start(out=gl_sb[:], in_=genf[F : F + 1, :])

        # addv = meanV*OSC + gen_last*OSC, bounced through DRAM into
        # per-partition layout [128, NBT] for the staging ops.
        av_sb = const.tile([1, BS], f32, name="av_sb", tag="av_sb")
        nc.vector.scalar_tensor_tensor(
            av_sb[:], gl_sb[:], float(OSC), mv_sb[:], op0=ALU.mult, op1=ALU.add
        )
        nc.sync.dma_start(out=av_d[:], in_=av_sb[0:1, :])
        addv_t = const.tile([128, NBT], f32, name="addv_t", tag="addv_t")
        nc.sync.dma_start(out=addv_t[:], in_=av_d.ap().rearrange("(t p) -> p t", p=128))

        # ---- reconstruction: out[bt*128+p, m] over movie chunk-pairs.
        # PE in 64x128 row-tiled mode: tile T0 (SBUF partitions 0-63) runs
        # the even chunk, tile T8 (64-127, the duplicated factor rows) the
        # odd chunk CONCURRENTLY, into adjacent PSUM banks.
        PAIRS = [CHUNKS[i : i + 2] for i in range(0, len(CHUNKS), 2)]
        for bt in range(NBT):
            lhsT_lo = genext2[0:F, bt * 128 : (bt + 1) * 128]
            lhsT_hi = genext2[F : 2 * F, bt * 128 : (bt + 1) * 128]
            st = None
            for pi, pair in enumerate(PAIRS):
                eng = 0 if (pi % 9) in (0, 2, 4, 6) else 1  # 11:9 Scalar:Vector
                pr = psp.tile([128, 2, 512], f32, name="pr", tag="ps")
                for j, (co, cw) in enumerate(pair):
                    nc.tensor.matmul(
                        pr[:, j, 0:cw],
                        lhsT=(lhsT_lo if j == 0 else lhsT_hi),
                        rhs=(mft[0:F, co : co + cw] if j == 0 else mft[F : 2 * F, co : co + cw]),
                        start=True, stop=True,
                    )
                pw = sum(cw for _, cw in pair)
                if pi % 2 == 0:
                    st = ost.tile([128, 2048], odt, name="st", tag="st")
                    so, po = 0, pair[0][0]
                # stage the full [2,512] pair; only the valid prefix is DMA'd
                nst = 1024 if pw == 1024 else 512 + pair[1][1]
                pr2d = pr[:].opt()  # [128, 2, 512] -> contiguous [128, 1024]
                if eng == 0:
                    nc.vector.tensor_scalar(
                        st[:, so : so + 1024], pr2d, OSC, addv_t[:, bt : bt + 1],
                        op0=ALU.mult, op1=ALU.add,
                    )
                else:
                    nc.scalar.activation(
                        st[:, so : so + 1024], pr2d, AF.Identity,
                        bias=addv_t[:, bt : bt + 1], scale=OSC,
                    )
                so += nst
                if pi % 2 == 1 or pi == len(PAIRS) - 1:
                    q = nc.sync if (pi // 2) % 2 == 0 else nc.gpsimd
                    q.dma_start(
                        out=out_d[bt * 128 : (bt + 1) * 128, po : po + so],
                        in_=st[:, 0:so],
                    )

    nc.compile()
    return nc


def prep_in_maps(cfg, inputs):
    """Shard + lay out the full inputs into per-core DRAM input maps."""
    d = _derived(cfg)
    BS, UINFO, M, F, DIMS, T1 = cfg["BS"], cfg["UINFO"], cfg["M"], cfg["F"], cfg["DIMS"], d["T1"]
    extInfo = np.asarray(inputs["extInfo"], np.float32)
    ratings = np.asarray(inputs["ratings"], np.float32)

    # BN (eval) fold into layer 2: y = g'(lrelu1) + b' with g' = bn_g/sqrt(1+eps)
    g = np.asarray(inputs["bn_g"], np.float32) / np.float32(np.sqrt(1.0 + BN_EPS))
    bnb = np.asarray(inputs["bn_b"], np.float32)
    w2 = np.asarray(inputs["w2"], np.float32)
    w2f = w2 * g[None, :]
    b2f = np.asarray(inputs["b2"], np.float32) + w2 @ bnb

    shared = {}
    # w1t: [KH,FO1] -> padded [T1*128, FO1] -> [128, T1, FO1]; the row at
    # index UINFO+M carries b1 (matching the ones-row in the history).
    w1 = np.asarray(inputs["w1"], np.float32)
    b1 = np.asarray(inputs["b1"], np.float32)
    FO1 = DIMS[0]
    w1tp = np.zeros((T1 * 128, FO1), FP8)
    w1tp[0 : w1.shape[1]] = (w1.T * np.float32(cfg["W1_SCALE"])).astype(FP8)
    w1tp[UINFO + M] = (b1 * np.float32(cfg["W1_SCALE"])).astype(FP8)
    shared["w1t"] = np.ascontiguousarray(w1tp.reshape(T1, 128, FO1).transpose(1, 0, 2))

    def pack_w(wT, fo, dt=BF16, scale=1.0):
        fi = wT.shape[0]
        w = (wT.astype(np.float32) * np.float32(scale)).astype(dt)
        return np.ascontiguousarray(w.reshape(fi // 128, 128, fo).transpose(1, 0, 2))

    scs = cfg["MLP_SCALES"]
    shared["w2t"] = pack_w(w2f.T, DIMS[1], FP8, scs[0])
    for li, wname in ((3, "w3"), (4, "w4"), (5, "w5"), (6, "w6")):
        w = np.asarray(inputs[wname], np.float32)
        fo = DIMS[li - 1]
        if li < 6:
            shared[f"w{li}t"] = pack_w(w.T, fo, FP8, scs[li - 2])
        else:
            shared[f"w{li}t"] = pack_w(w.T, fo)

    def pack_b(b, fo):
        fot = math.ceil(fo / 128)
        bp = np.zeros(fot * 128, np.float32)
        bp[:fo] = b
        return np.ascontiguousarray(bp.reshape(fot, 128).T)

    bsrc = {2: b2f}
    for li in (3, 4, 5):
        bsrc[li] = np.asarray(inputs[f"b{li}"], np.float32)
    for li in range(2, 6):
        shared[f"b{li}p"] = pack_b(bsrc[li], DIMS[li - 1])
    shared["b6p"] = pack_b(np.asarray(inputs["b6"], np.float32), DIMS[5])

    # bf16 mft: factor rows duplicated into partitions 64..127 for the
    # row-tiled reconstruction (movie_bias is added on host at dequant).
    mft = np.zeros((128, M), BF16)
    mft[0:F] = np.asarray(inputs["movie_factors"], np.float32).T.astype(BF16)
    mft[F : 2 * F] = mft[0:F]
    shared["mft"] = mft

    in_maps = []
    for c in range(NCORES):
        sl = slice(c * BS, (c + 1) * BS)
        htc = np.zeros((T1 * 128, BS), FP8)
        htc[0:UINFO] = extInfo[sl].T.astype(FP8)
        htc[UINFO : UINFO + M] = ratings[sl].T.astype(FP8)
        htc[UINFO + M] = np.float32(1.0)  # ones-row: picks up b1 from w1t
        m = dict(shared)
        m["ht"] = np.ascontiguousarray(htc.reshape(T1, 128, BS).transpose(1, 0, 2))
        in_maps.append(m)
    return in_maps


_NC_CACHE = {}


def run_on_hw(cfg, inputs, trace=False):
    from concourse.bass_utils import run_bass_kernel_spmd

    key = tuple(sorted((k, v) for k, v in cfg.items() if k != "DIMS")) + (cfg["DIMS"],)
    if key not in _NC_CACHE:
        _NC_CACHE[key] = build_nc(cfg)
    nc = _NC_CACHE[key]
    in_maps = prep_in_maps(cfg, inputs)
    br = run_bass_kernel_spmd(nc, in_maps, list(range(NCORES)), trace=trace)
    BS, M = cfg["BS"], cfg["M"]
    out = np.empty((NCORES * BS, M), np.float32)
    dq = np.float32(1.0 / cfg["OUT_SCALE"]) if cfg["OUT_DT"] == "i8" else np.float32(1.0)
    mb = np.asarray(inputs["movie_bias"], np.float32)[None, :]
    for c in range(NCORES):
        out[c * BS : (c + 1) * BS] = (
            np.asarray(br.results[c]["out"], dtype=np.float32) * dq + mb
        )
    return out, br


def kernel(**inputs) -> np.ndarray:
    try:
        out, _ = run_on_hw(FULL_CFG, inputs, trace=False)
    except Exception:
        # one retry for transient device/runtime hiccups
        out, _ = run_on_hw(FULL_CFG, inputs, trace=False)
    return out


# revision 13
# speedup vs baseline: 1.1013x; 1.0696x over previous
"""Trainium2 Bass kernel for nn_DLFG_79817672229311 (segment_reduce).

Computes, data-parallel over the batch axis on 8 NeuronCores:
  history = [extInfo, ratings, 1]                    # [BS, 20033] per core
  x1 = lrelu(history @ [w1;b1].T); BN folded into w2 on host
  x2..x5 = lrelu(x @ wl.T + bl)
  gen = tanh(x5 @ w6.T + b6)                         # [BS, 65]
  s, cnt = per-row sum / count of nonzero ratings
  addv = s/cnt + gen[:, 64]
  out = gen[:, :64] @ movie_factors.T + addv[:, None] + movie_bias

Design (per core; layer 1 is at the fp8 DoubleRow compute wall ~135us, so
everything else hides under or packs tightly around it):
- Activations ride transposed ([feature, batch]): batch (512) is the matmul
  free dim, features the partition dim, so no on-device transposes are needed.
- History is staged to SBUF once in fp8 (ratings 0..5 are exact in e4m3) with
  a host-appended ones-row; layer 1 runs fp8 DoubleRow against 2^15-pre-scaled
  fp8 [w1;b1] slabs streamed from HBM, so the bias rides the contraction and
  the epilogue is a bias-free paired Lrelu.  K is host-padded to an even tile
  count so every step is a DoubleRow pair.
- BOTH cnt and s accumulate on the Vector engine under the layer-1 shadow as
  paired fused chains (cnt: min+add, s: plain add) into fp16 accumulators
  (integer partial sums <= 790 are exact in fp16; fp16 halves DVE traffic).
  This removes the former PE ones-matmul s-chain (~16us of Tensor time).
  Partition reduction is 4 tiny fp16 matmuls; the ones-row contribution is
  subtracted in the combine.
- Mid layers interleave their k-accumulation across 3 PSUM banks; some lrelu
  epilogues run on the DVE.
- addv bounces through DRAM into per-partition [128, NBT] and enters the
  reconstruction staging op as its per-partition bias.
- Reconstruction exploits PE row tiling (64x128 mode): the contraction is
  only the 64 factors (movie_bias is added on HOST during dequant), so the
  factor block is duplicated into SBUF partitions 64..127 (both in mft and in
  genext2) and each chunk-pair issues two CONCURRENT matmuls -- tile T0
  (SBUF rows 0-63) on the even chunk, tile T8 (rows 64-127) on the odd chunk,
  landing in adjacent PSUM banks.  This halves reconstruction Tensor time.
- Recon staging: fused scale+bias on alternating Vector/Scalar engines ->
  int8 DRAM in 2048-col blocks, with the output drains alternating between
  the Sync and GpSimd DMA queues so descriptor issue isn't single-queue
  limited.  Host dequantizes by the fixed scale 4/127 and adds movie_bias.
"""

import math
import sys

sys.path.insert(0, "/opt/trn_rl_repo")

import numpy as np
import ml_dtypes

BF16 = ml_dtypes.bfloat16
FP8 = ml_dtypes.float8_e4m3

NCORES = 8
BN_EPS = 0.05
SLOPE = 0.01

FULL_CFG = dict(
    BS=512,  # per-core batch
    UINFO=32,
    M=20000,
    F=64,
    DIMS=(1024, 512, 256, 512, 1024, 65),  # fan-outs of the 6 linear layers
    HTC=4,  # history K-tiles per DMA chunk (must be even for DoubleRow pairs)
    W1_SCALE=2.0**15,  # fp8 pre-scale: w1 ~ U(+-0.007) sits in e4m3 subnormals
    MLP_SCALES=(4096.0, 4096.0, 2048.0, 4096.0),  # 2^k per layer, |w|*s < 240
    OUT_DT="i8",  # "i8" (host dequant) or "bf16"
    OUT_SCALE=127.0 / 4.0,  # int8 quantization scale (|out| <= ~3.2)
    SC_DT="bf16",  # s/cnt DVE accumulator dtype ("bf16", "f16" or "f32")
    S_DVE_PAIRS=46,  # leading k-pair-steps whose s-accum rides the DVE;
    # the rest run as a PE ones-matmul chain after the mid layers (the DVE
    # can't hold both full chains under the layer-1 shadow, and GpSimd
    # chains poison SBUF bandwidth for everyone)
)


def _derived(cfg):
    d = dict(cfg)
    d["KH"] = cfg["UINFO"] + cfg["M"] + 1  # +1 ones-row carrying b1
    t1 = math.ceil(d["KH"] / 128)
    d["T1"] = t1 + (t1 % 2)  # pad to even so all steps are DoubleRow pairs
    d["NBT"] = cfg["BS"] // 128  # batch tiles per core
    d["CHUNKS"] = [(o, min(512, cfg["M"] - o)) for o in range(0, cfg["M"], 512)]
    return d


def build_nc(cfg):
    """Build + compile the (single-core SPMD) Bass program."""
    import concourse.bass as bass
    import concourse.tile as tile
    from concourse import bacc, mybir

    d = _derived(cfg)
    BS, UINFO, M, F = cfg["BS"], cfg["UINFO"], cfg["M"], cfg["F"]
    DIMS = cfg["DIMS"]
    T1, NBT, CHUNKS, HTC = d["T1"], d["NBT"], d["CHUNKS"], cfg["HTC"]
    FO1 = DIMS[0]
    FO1T = FO1 // 128
    w1_unscale = 1.0 / cfg["W1_SCALE"]
    f32 = mybir.dt.float32
    bf16 = mybir.dt.bfloat16
    f16 = mybir.dt.float16
    f8 = mybir.dt.float8e4
    i8 = mybir.dt.int8
    AF = mybir.ActivationFunctionType
    ALU = mybir.AluOpType

    OUT_I8 = cfg["OUT_DT"] == "i8"
    odt = i8 if OUT_I8 else bf16
    OSC = cfg["OUT_SCALE"] if OUT_I8 else 1.0
    scdt = {"bf16": bf16, "f16": f16, "f32": f32}[cfg["SC_DT"]]

    nc = bacc.Bacc("TRN2", target_bir_lowering=False, debug=False)

    # ---- DRAM I/O ----
    ht_d = nc.dram_tensor("ht", [128, T1, BS], f8, kind="ExternalInput")
    w1t_d = nc.dram_tensor("w1t", [128, T1, FO1], f8, kind="ExternalInput")
    w_d = {}
    for li in range(2, 7):
        fi, fo = DIMS[li - 2], DIMS[li - 1]
        wdt = f8 if li < 6 else bf16
        w_d[li] = nc.dram_tensor(f"w{li}t", [128, fi // 128, fo], wdt, kind="ExternalInput")
    bp_d = {}
    for li in range(2, 6):
        fot = math.ceil(DIMS[li - 1] / 128)
        bp_d[li] = nc.dram_tensor(f"b{li}p", [128, fot], f32, kind="ExternalInput")
    b6_d = nc.dram_tensor("b6p", [128, 1], f32, kind="ExternalInput")
    mft_d = nc.dram_tensor("mft", [128, M], bf16, kind="ExternalInput")
    out_d = nc.dram_tensor("out", [BS, M], odt, kind="ExternalOutput")
    av_d = nc.dram_tensor("av_scr", [BS], f32)  # addv row->partition bounce

    with tile.TileContext(nc) as tc, bass.ExitStack() as ctx:
        const = ctx.enter_context(tc.tile_pool(name="const", bufs=1))
        htp = ctx.enter_context(tc.tile_pool(name="htp", bufs=1))
        w1p = ctx.enter_context(tc.tile_pool(name="w1p", bufs=8))
        actp = ctx.enter_context(tc.tile_pool(name="actp", bufs=1))
        ost = ctx.enter_context(tc.tile_pool(name="ost", bufs=6))
        psp = ctx.enter_context(tc.tile_pool(name="psp", bufs=4, space="PSUM"))

        # ---- constants in SBUF (dispatched on the Scalar DMA queue) ----
        bp_sb = {}
        for li in range(2, 6):
            fot = math.ceil(DIMS[li - 1] / 128)
            bp_sb[li] = const.tile([128, fot], f32, name=f"b{li}p", tag=f"b{li}p")
            nc.scalar.dma_start(out=bp_sb[li][:], in_=bp_d[li][:])
        b6_sb = const.tile([128, 1], f32, name="b6p", tag="b6p")
        nc.scalar.dma_start(out=b6_sb[:], in_=b6_d[:])
        onesf = const.tile([128, 1], scdt, name="onesf", tag="onesf")
        nc.vector.memset(onesf[:], 1.0)
        # fp8 ones for the PE s-tail chain ([128,2,16]: DoubleRow weight
        # pair-step must be a multiple of 16 per the ISA)
        ones16 = const.tile([128, 2, 16], f8, name="ones16", tag="ones16")
        nc.vector.memset(ones16[:], 1.0)
        # per-partition mask for history tile 0 (extInfo rows excluded)
        rmask = const.tile([128, 1], f32, name="rmask", tag="rmask")
        nc.vector.memset(rmask[:], 1.0)
        nc.vector.memset(rmask[0:UINFO, :], 0.0)

        # ---- layer 1: one pass over history segments ----
        segs = []
        t0 = 0
        for tn in [2, 2]:
            segs.append((t0, tn))
            t0 += tn
        while t0 < T1:
            tn = min(HTC, T1 - t0)
            segs.append((t0, tn))
            t0 += tn
        NSEG = len(segs)

        x1t = actp.tile([128, FO1T, BS], f8, name="x1t", tag="x1t")
        c_acc = const.tile([128, 2, BS], scdt, name="c_acc", tag="c_acc")
        s_acc = const.tile([128, 2, BS], scdt, name="s_acc", tag="s_acc")
        # 4 paired PSUM tiles (2 banks each) -> bias-free paired epilogues
        ps1 = [psp.tile([128, 2, BS], f32, name="ps1", tag="ps") for _ in range(FO1T // 2)]

        nstep = T1 // 2
        SD = cfg["S_DVE_PAIRS"]
        s_tail = []  # (htt, lo) pair-steps whose s runs on the PE ones-chain
        step_i = 0
        sc_first = True
        for si_, (ts_, tn) in enumerate(segs):
            htt = htp.tile([128, tn, BS], f8, name="ht", tag="ht", bufs=NSEG)
            nc.sync.dma_start(out=htt[:], in_=ht_d[:, ts_ : ts_ + tn, :])

            lo = 0
            while lo < tn:
                t = ts_ + lo
                w1s = w1p.tile([128, 2, FO1], f8, name="w1s", tag="w1s")
                if step_i == 0:
                    h = FO1 // 2
                    nc.sync.dma_start(out=w1s[:, 0:2, 0:h], in_=w1t_d[:, t : t + 2, 0:h])
                    nc.sync.dma_start(out=w1s[:, 0:2, h:FO1], in_=w1t_d[:, t : t + 2, h:FO1])
                else:
                    nc.sync.dma_start(out=w1s[:, 0:2, :], in_=w1t_d[:, t : t + 2, :])
                for fo in range(FO1T):
                    fsl = slice(fo * 128, (fo + 1) * 128)
                    pdst = ps1[fo // 2][:, fo % 2, :]
                    nc.tensor.matmul(
                        pdst,
                        lhsT=w1s[:, 0:2, fsl],
                        rhs=htt[:, lo : lo + 2, :],
                        start=(step_i == 0),
                        stop=(step_i == nstep - 1),
                        perf_mode=mybir.MatmulPerfMode.DoubleRow,
                    )
                step_i += 1
                lo += 2

            # cnt chain (fused min+add) and the leading part of the s chain
            # (plain add) on the DVE under the layer-1 shadow, into 16-bit
            # accumulators (integer partials <= ~790, exact enough).  The
            # DVE can't hold BOTH full chains before the mid layers end, so
            # s pair-steps >= SD are deferred to a short PE ones-chain.
            # Segment 0 holds extInfo rows; init via masked ops.
            if sc_first:
                assert tn == 2
                nc.vector.tensor_scalar(
                    c_acc[:, 0, :], htt[:, 0, :], 1.0, rmask[:], op0=ALU.min, op1=ALU.mult
                )
                nc.vector.tensor_scalar(
                    c_acc[:, 1, :], htt[:, 1, :], 1.0, None, op0=ALU.min
                )
                nc.vector.tensor_scalar(
                    s_acc[:, 0, :], htt[:, 0, :], 1.0, rmask[:], op0=ALU.mult, op1=ALU.mult
                )
                nc.vector.tensor_copy(s_acc[:, 1, :], htt[:, 1, :])
                sc_first = False
            else:
                o = 0
                while o < tn:
                    nc.vector.scalar_tensor_tensor(
                        c_acc[:], htt[:, o : o + 2, :], 1.0, c_acc[:],
                        op0=ALU.min, op1=ALU.add,
                    )
                    if (ts_ + o) // 2 < SD:
                        nc.vector.tensor_add(s_acc[:], htt[:, o : o + 2, :], s_acc[:])
                    else:
                        s_tail.append((htt, o))
                    o += 2

        # layer-1 epilogue (bias-free: bias rode the matmul via the ones-row)
        for j in range(FO1T // 2):
            nc.scalar.activation(
                x1t[:, 2 * j : 2 * j + 2, :], ps1[j][:], AF.Lrelu,
                scale=w1_unscale, alpha=SLOPE,
            )

        # ---- remaining weights + movie factors: emitted late on the Sync
        # queue so the layer-1 ht/w1 stream gets the DMA bandwidth first.
        w_sb = {}
        for li in range(2, 7):
            fi, fo = DIMS[li - 2], DIMS[li - 1]
            wdt = f8 if li < 6 else bf16
            w_sb[li] = const.tile([128, fi // 128, fo], wdt, name=f"w{li}t", tag=f"w{li}t")
            nc.sync.dma_start(out=w_sb[li][:], in_=w_d[li][:])
        mft = const.tile([128, M], bf16, name="mft", tag="mft")
        nc.sync.dma_start(out=mft[:], in_=mft_d[:])

        # ---- layers 2..5 (lrelu) ----
        xin = x1t
        for li in range(2, 6):
            fi, fo = DIMS[li - 2], DIMS[li - 1]
            fit, fot = fi // 128, fo // 128
            xdt = f8 if li < 5 else bf16
            unsc = 1.0 / cfg["MLP_SCALES"][li - 2]
            xout = actp.tile(
                [128, fot, BS], xdt, name=f"x{li}t",
                tag=("x1t" if li == 5 else "x2t" if li == 4 else f"x{li}t"),
            )
            # interleave the k-accumulation across up to 4 fo-tile banks so
            # consecutive matmuls don't serialize on one bank's drain
            for g0 in range(0, fot, 4):
                gn = min(4, fot - g0)
                pss = [psp.tile([128, BS], f32, name="ps", tag="ps") for _ in range(gn)]
                ki = 0
                while ki < fit:
                    n2 = 2 if ki + 2 <= fit else 1
                    for j in range(gn):
                        ft = g0 + j
                        if n2 == 2:
                            nc.tensor.matmul(
                                pss[j][:],
                                lhsT=w_sb[li][:, ki : ki + 2, ft * 128 : (ft + 1) * 128],
                                rhs=xin[:, ki : ki + 2, :],
                                start=(ki == 0),
                                stop=(ki + 2 == fit),
                                perf_mode=mybir.MatmulPerfMode.DoubleRow,
                            )
                        else:
                            nc.tensor.matmul(
                                pss[j][:],
                                lhsT=w_sb[li][:, ki, ft * 128 : (ft + 1) * 128],
                                rhs=xin[:, ki, :],
                                start=(ki == 0),
                                stop=True,
                            )
                    ki += n2
                # epilogues all on the Scalar engine: the DVE queue carries
                # the s/cnt chains during layer 1 and would stall these
                # (strict FIFO), which in turn stalls the next layer's PE.
                for j in range(gn):
                    ft = g0 + j
                    nc.scalar.activation(
                        xout[:, ft, :], pss[j][:], AF.Lrelu,
                        bias=bp_sb[li][:, ft : ft + 1], scale=unsc, alpha=SLOPE,
                    )
            xin = xout

        # ---- PE s-tail: DoubleRow ones-chain over the deferred pair-steps
        # (history is long resident), double-buffered across one PSUM
        # slot's two banks.
        NT = len(s_tail)
        scx = psp.tile([16, 2, BS], f32, name="scx", tag="ps")
        for si, (htt, lo) in enumerate(s_tail):
            nc.tensor.matmul(
                scx[0:16, si % 2, :], lhsT=ones16[:], rhs=htt[:, lo : lo + 2, :],
                start=(si < 2), stop=(si >= NT - 2),
                perf_mode=mybir.MatmulPerfMode.DoubleRow,
            )

        # ---- s/cnt partition reduces + meanV combine, emitted before layer
        # 6 so meanV (pre-scaled by the output quant scale) is ready before
        # the reconstruction staging needs addv.
        c_red = psp.tile([1, 2, BS], f32, name="c_red", tag="ps")
        s_red = psp.tile([1, 2, BS], f32, name="s_red", tag="ps")
        nc.tensor.matmul(c_red[:, 0, :], lhsT=onesf[:], rhs=c_acc[:, 0, :], start=True, stop=True)
        nc.tensor.matmul(c_red[:, 1, :], lhsT=onesf[:], rhs=c_acc[:, 1, :], start=True, stop=True)
        nc.tensor.matmul(s_red[:, 0, :], lhsT=onesf[:], rhs=s_acc[:, 0, :], start=True, stop=True)
        nc.tensor.matmul(s_red[:, 1, :], lhsT=onesf[:], rhs=s_acc[:, 1, :], start=True, stop=True)
        # the host-appended ones-row rode the PE s-tail (it is in the last
        # k-pair) and counted +1 per batch in cnt chain half 0: subtract 1
        # from each.
        c0_sb = const.tile([1, BS], f32, name="c0_sb", tag="c0_sb")
        nc.vector.tensor_scalar_sub(c0_sb[:], c_red[0:1, 0, :], 1.0)
        c_sb = const.tile([1, BS], f32, name="c_sb", tag="c_sb")
        nc.vector.tensor_add(c_sb[:], c0_sb[:], c_red[0:1, 1, :])
        rc_sb = const.tile([1, BS], f32, name="rc_sb", tag="rc_sb")
        nc.vector.reciprocal(rc_sb[:], c_sb[:])
        # s pre-scaled by OSC while combining the DVE halves + PE-tail banks
        s_sb = const.tile([1, BS], f32, name="s_sb", tag="s_sb")
        nc.vector.tensor_scalar(
            s_sb[:], s_red[0:1, 0, :], -1.0, float(OSC), op0=ALU.add, op1=ALU.mult
        )
        nc.vector.scalar_tensor_tensor(
            s_sb[:], s_red[0:1, 1, :], float(OSC), s_sb[:], op0=ALU.mult, op1=ALU.add
        )
        nc.vector.scalar_tensor_tensor(
            s_sb[:], scx[0:1, 0, :], float(OSC), s_sb[:], op0=ALU.mult, op1=ALU.add
        )
        nc.vector.scalar_tensor_tensor(
            s_sb[:], scx[0:1, 1, :], float(OSC), s_sb[:], op0=ALU.mult, op1=ALU.add
        )
        mv_sb = const.tile([1, BS], f32, name="mv_sb", tag="mv_sb")
        nc.vector.tensor_mul(mv_sb[:], rc_sb[:], s_sb[:])

        # ---- layer 6 (tanh) -> genf [65, BS] f32 ----
        fi, fo = DIMS[4], DIMS[5]
        fit = fi // 128
        assert fo == F + 1
        ps6 = psp.tile([fo, BS], f32, name="ps6", tag="ps")
        for ki in range(fit):
            nc.tensor.matmul(
                ps6[:],
                lhsT=w_sb[6][:, ki, 0:fo],
                rhs=xin[:, ki, :],
                start=(ki == 0),
                stop=(ki == fit - 1),
            )
        genf = actp.tile([fo, BS], f32, name="genf", tag="genf")
        nc.scalar.activation(genf[:], ps6[:], AF.Tanh, bias=b6_sb[0:fo, 0:1], scale=1.0)

        # ---- genext2: factor rows in bf16, duplicated into partitions
        # 64..127 (via SBUF->SBUF DMA) so reconstruction can row-tile the PE.
        genext2 = actp.tile([128, BS], bf16, name="genext2", tag="genext")
        nc.vector.tensor_copy(genext2[0:F, :], genf[0:F, :])
        nc.sync.dma_start(out=genext2[F : 2 * F, :], in_=genext2[0:F, :])
        gl_sb = const.tile([1, BS], f32, name="gl_sb", tag="gl_sb")
        nc.sync.dma_start(out=gl_sb[:], in_=genf[F : F + 1, :])

        # addv = meanV*OSC + gen_last*OSC, bounced through DRAM into
        # per-partition layout [128, NBT] for the staging ops.
        av_sb = const.tile([1, BS], f32, name="av_sb", tag="av_sb")
        nc.vector.scalar_tensor_tensor(
            av_sb[:], gl_sb[:], float(OSC), mv_sb[:], op0=ALU.mult, op1=ALU.add
        )
        nc.sync.dma_start(out=av_d[:], in_=av_sb[0:1, :])
        addv_t = const.tile([128, NBT], f32, name="addv_t", tag="addv_t")
        nc.sync.dma_start(out=addv_t[:], in_=av_d.ap().rearrange("(t p) -> p t", p=128))

        # ---- reconstruction: out[bt*128+p, m] over movie chunk-pairs.
        # PE in 64x128 row-tiled mode: tile T0 (SBUF partitions 0-63) runs
        # the even chunk, tile T8 (64-127, the duplicated factor rows) the
        # odd chunk CONCURRENTLY, into adjacent PSUM banks.
        PAIRS = [CHUNKS[i : i + 2] for i in range(0, len(CHUNKS), 2)]
        for bt in range(NBT):
            lhsT_lo = genext2[0:F, bt * 128 : (bt + 1) * 128]
            lhsT_hi = genext2[F : 2 * F, bt * 128 : (bt + 1) * 128]
            st = None
            for pi, pair in enumerate(PAIRS):
                eng = 0 if (pi % 9) in (0, 2, 4, 6) else 1  # 11:9 Scalar:Vector
                pr = psp.tile([128, 2, 512], f32, name="pr", tag="ps")
                for j, (co, cw) in enumerate(pair):
                    nc.tensor.matmul(
                        pr[:, j, 0:cw],
                        lhsT=(lhsT_lo if j == 0 else lhsT_hi),
                        rhs=(mft[0:F, co : co + cw] if j == 0 else mft[F : 2 * F, co : co + cw]),
                        start=True, stop=True,
                    )
                pw = sum(cw for _, cw in pair)
                if pi % 2 == 0:
                    st = ost.tile([128, 2048], odt, name="st", tag="st")
                    so, po = 0, pair[0][0]
                # stage the full [2,512] pair; only the valid prefix is DMA'd
                nst = 1024 if pw == 1024 else 512 + pair[1][1]
                pr2d = pr[:].opt()  # [128, 2, 512] -> contiguous [128, 1024]
                if eng == 0:
                    nc.vector.tensor_scalar(
                        st[:, so : so + 1024], pr2d, OSC, addv_t[:, bt : bt + 1],
                        op0=ALU.mult, op1=ALU.add,
                    )
                else:
                    nc.scalar.activation(
                        st[:, so : so + 1024], pr2d, AF.Identity,
                        bias=addv_t[:, bt : bt + 1], scale=OSC,
                    )
                so += nst
                if pi % 2 == 1 or pi == len(PAIRS) - 1:
                    q = nc.sync if (pi // 2) % 2 == 0 else nc.gpsimd
                    q.dma_start(
                        out=out_d[bt * 128 : (bt + 1) * 128, po : po + so],
                        in_=st[:, 0:so],
                    )

    nc.compile()
    return nc


def prep_in_maps(cfg, inputs):
    """Shard + lay out the full inputs into per-core DRAM input maps."""
    d = _derived(cfg)
    BS, UINFO, M, F, DIMS, T1 = cfg["BS"], cfg["UINFO"], cfg["M"], cfg["F"], cfg["DIMS"], d["T1"]
    extInfo = np.asarray(inputs["extInfo"], np.float32)
    ratings = np.asarray(inputs["ratings"], np.float32)

    # BN (eval) fold into layer 2: y = g'(lrelu1) + b' with g' = bn_g/sqrt(1+eps)
    g = np.asarray(inputs["bn_g"], np.float32) / np.float32(np.sqrt(1.0 + BN_EPS))
    bnb = np.asarray(inputs["bn_b"], np.float32)
    w2 = np.asarray(inputs["w2"], np.float32)
    w2f = w2 * g[None, :]
    b2f = np.asarray(inputs["b2"], np.float32) + w2 @ bnb

    shared = {}
    # w1t: [KH,FO1] -> padded [T1*128, FO1] -> [128, T1, FO1]; the row at
    # index UINFO+M carries b1 (matching the ones-row in the history).
    w1 = np.asarray(inputs["w1"], np.float32)
    b1 = np.asarray(inputs["b1"], np.float32)
    FO1 = DIMS[0]
    w1tp = np.zeros((T1 * 128, FO1), FP8)
    w1tp[0 : w1.shape[1]] = (w1.T * np.float32(cfg["W1_SCALE"])).astype(FP8)
    w1tp[UINFO + M] = (b1 * np.float32(cfg["W1_SCALE"])).astype(FP8)
    shared["w1t"] = np.ascontiguousarray(w1tp.reshape(T1, 128, FO1).transpose(1, 0, 2))

    def pack_w(wT, fo, dt=BF16, scale=1.0):
        fi = wT.shape[0]
        w = (wT.astype(np.float32) * np.float32(scale)).astype(dt)
        return np.ascontiguousarray(w.reshape(fi // 128, 128, fo).transpose(1, 0, 2))

    scs = cfg["MLP_SCALES"]
    shared["w2t"] = pack_w(w2f.T, DIMS[1], FP8, scs[0])
    for li, wname in ((3, "w3"), (4, "w4"), (5, "w5"), (6, "w6")):
        w = np.asarray(inputs[wname], np.float32)
        fo = DIMS[li - 1]
        if li < 6:
            shared[f"w{li}t"] = pack_w(w.T, fo, FP8, scs[li - 2])
        else:
            shared[f"w{li}t"] = pack_w(w.T, fo)

    def pack_b(b, fo):
        fot = math.ceil(fo / 128)
        bp = np.zeros(fot * 128, np.float32)
        bp[:fo] = b
        return np.ascontiguousarray(bp.reshape(fot, 128).T)

    bsrc = {2: b2f}
    for li in (3, 4, 5):
        bsrc[li] = np.asarray(inputs[f"b{li}"], np.float32)
    for li in range(2, 6):
        shared[f"b{li}p"] = pack_b(bsrc[li], DIMS[li - 1])
    shared["b6p"] = pack_b(np.asarray(inputs["b6"], np.float32), DIMS[5])

    # bf16 mft: factor rows duplicated into partitions 64..127 for the
    # row-tiled reconstruction (movie_bias is added on host at dequant).
    mft = np.zeros((128, M), BF16)
    mft[0:F] = np.asarray(inputs["movie_factors"], np.float32).T.astype(BF16)
    mft[F : 2 * F] = mft[0:F]
    shared["mft"] = mft

    in_maps = []
    for c in range(NCORES):
        sl = slice(c * BS, (c + 1) * BS)
        htc = np.zeros((T1 * 128, BS), FP8)
        htc[0:UINFO] = extInfo[sl].T.astype(FP8)
        htc[UINFO : UINFO + M] = ratings[sl].T.astype(FP8)
        htc[UINFO + M] = np.float32(1.0)  # ones-row: picks up b1 from w1t
        m = dict(shared)
        m["ht"] = np.ascontiguousarray(htc.reshape(T1, 128, BS).transpose(1, 0, 2))
        in_maps.append(m)
    return in_maps


_NC_CACHE = {}


def run_on_hw(cfg, inputs, trace=False):
    from concourse.bass_utils import run_bass_kernel_spmd

    key = tuple(sorted((k, v) for k, v in cfg.items() if k != "DIMS")) + (cfg["DIMS"],)
    if key not in _NC_CACHE:
        _NC_CACHE[key] = build_nc(cfg)
    nc = _NC_CACHE[key]
    in_maps = prep_in_maps(cfg, inputs)
    br = run_bass_kernel_spmd(nc, in_maps, list(range(NCORES)), trace=trace)
    BS, M = cfg["BS"], cfg["M"]
    out = np.empty((NCORES * BS, M), np.float32)
    dq = np.float32(1.0 / cfg["OUT_SCALE"]) if cfg["OUT_DT"] == "i8" else np.float32(1.0)
    mb = np.asarray(inputs["movie_bias"], np.float32)[None, :]
    for c in range(NCORES):
        out[c * BS : (c + 1) * BS] = (
            np.asarray(br.results[c]["out"], dtype=np.float32) * dq + mb
        )
    return out, br


def kernel(**inputs) -> np.ndarray:
    try:
        out, _ = run_on_hw(FULL_CFG, inputs, trace=False)
    except Exception:
        # one retry for transient device/runtime hiccups
        out, _ = run_on_hw(FULL_CFG, inputs, trace=False)
    return out


# revision 17
# speedup vs baseline: 1.1146x; 1.0121x over previous
"""Trainium2 Bass kernel for nn_DLFG_79817672229311 (segment_reduce).

Computes, data-parallel over the batch axis on 8 NeuronCores:
  history = [extInfo, ratings, 1]                    # [BS, 20033] per core
  x1 = lrelu(history @ [w1;b1].T); BN folded into w2 on host
  x2..x5 = lrelu(x @ wl.T + bl)
  gen = tanh(x5 @ w6.T + b6)                         # [BS, 65]
  s, cnt = per-row sum / count of nonzero ratings
  addv = s/cnt + gen[:, 64]
  out = gen[:, :64] @ movie_factors.T + addv[:, None] + movie_bias

Design (per core; layer 1 is at the fp8 DoubleRow compute wall ~135us, so
everything else hides under or packs tightly around it):
- Activations ride transposed ([feature, batch]): batch (512) is the matmul
  free dim, features the partition dim, so no on-device transposes are needed.
- History is staged to SBUF once in fp8 (ratings 0..5 are exact in e4m3) with
  a host-appended ones-row; layer 1 runs fp8 DoubleRow against 2^15-pre-scaled
  fp8 [w1;b1] slabs streamed from HBM, so the bias rides the contraction and
  the epilogue is a bias-free paired Lrelu.  K is host-padded to an even tile
  count so every step is a DoubleRow pair.
- BOTH cnt and s accumulate on the Vector engine under the layer-1 shadow as
  paired fused chains (cnt: min+add, s: plain add) into fp16 accumulators
  (integer partial sums <= 790 are exact in fp16; fp16 halves DVE traffic).
  This removes the former PE ones-matmul s-chain (~16us of Tensor time).
  Partition reduction is 4 tiny fp16 matmuls; the ones-row contribution is
  subtracted in the combine.
- Mid layers interleave their k-accumulation across 3 PSUM banks; some lrelu
  epilogues run on the DVE.
- addv bounces through DRAM into per-partition [128, NBT] and enters the
  reconstruction staging op as its per-partition bias.
- Reconstruction exploits PE row tiling (64x128 mode): the contraction is
  only the 64 factors (movie_bias is added on HOST during dequant), so the
  factor block is duplicated into SBUF partitions 64..127 (both in mft and in
  genext2) and each chunk-pair issues two CONCURRENT matmuls -- tile T0
  (SBUF rows 0-63) on the even chunk, tile T8 (rows 64-127) on the odd chunk,
  landing in adjacent PSUM banks.  This halves reconstruction Tensor time.
- Recon staging: fused scale+bias on alternating Vector/Scalar engines ->
  int8 DRAM in 2048-col blocks, with the output drains alternating between
  the Sync and GpSimd DMA queues so descriptor issue isn't single-queue
  limited.  Host dequantizes by the fixed scale 4/127 and adds movie_bias.
"""

import math
import sys

sys.path.insert(0, "/opt/trn_rl_repo")

import numpy as np
import ml_dtypes

BF16 = ml_dtypes.bfloat16
FP8 = ml_dtypes.float8_e4m3

NCORES = 8
BN_EPS = 0.05
SLOPE = 0.01

FULL_CFG = dict(
    BS=512,  # per-core batch
    UINFO=32,
    M=20000,
    F=64,
    DIMS=(1024, 512, 256, 512, 1024, 65),  # fan-outs of the 6 linear layers
    HTC=4,  # history K-tiles per DMA chunk (must be even for DoubleRow pairs)
    W1_SCALE=2.0**15,  # fp8 pre-scale: w1 ~ U(+-0.007) sits in e4m3 subnormals
    MLP_SCALES=(4096.0, 4096.0, 2048.0, 4096.0),  # 2^k per layer, |w|*s < 240
    OUT_DT="i8",  # "i8" (host dequant) or "bf16"
    OUT_SCALE=127.0 / 4.0,  # int8 quantization scale (|out| <= ~3.2)
    SC_DT="bf16",  # s/cnt DVE accumulator dtype ("bf16", "f16" or "f32")
    S_DVE_PAIRS=46,  # leading k-pair-steps whose s-accum rides the DVE;
    # the rest run as a PE ones-matmul chain after the mid layers (the DVE
    # can't hold both full chains under the layer-1 shadow, and GpSimd
    # chains poison SBUF bandwidth for everyone)
)


def _derived(cfg):
    d = dict(cfg)
    d["KH"] = cfg["UINFO"] + cfg["M"] + 1  # +1 ones-row carrying b1
    t1 = math.ceil(d["KH"] / 128)
    d["T1"] = t1 + (t1 % 2)  # pad to even so all steps are DoubleRow pairs
    d["NBT"] = cfg["BS"] // 128  # batch tiles per core
    d["CHUNKS"] = [(o, min(512, cfg["M"] - o)) for o in range(0, cfg["M"], 512)]
    return d


def build_nc(cfg):
    """Build + compile the (single-core SPMD) Bass program."""
    import concourse.bass as bass
    import concourse.tile as tile
    from concourse import bacc, mybir

    d = _derived(cfg)
    BS, UINFO, M, F = cfg["BS"], cfg["UINFO"], cfg["M"], cfg["F"]
    DIMS = cfg["DIMS"]
    T1, NBT, CHUNKS, HTC = d["T1"], d["NBT"], d["CHUNKS"], cfg["HTC"]
    FO1 = DIMS[0]
    FO1T = FO1 // 128
    w1_unscale = 1.0 / cfg["W1_SCALE"]
    f32 = mybir.dt.float32
    bf16 = mybir.dt.bfloat16
    f16 = mybir.dt.float16
    f8 = mybir.dt.float8e4
    i8 = mybir.dt.int8
    AF = mybir.ActivationFunctionType
    ALU = mybir.AluOpType

    OUT_I8 = cfg["OUT_DT"] == "i8"
    odt = i8 if OUT_I8 else bf16
    OSC = cfg["OUT_SCALE"] if OUT_I8 else 1.0
    scdt = {"bf16": bf16, "f16": f16, "f32": f32}[cfg["SC_DT"]]

    nc = bacc.Bacc("TRN2", target_bir_lowering=False, debug=False)

    # ---- DRAM I/O ----
    ht_d = nc.dram_tensor("ht", [128, T1, BS], f8, kind="ExternalInput")
    w1t_d = nc.dram_tensor("w1t", [128, T1, FO1], f8, kind="ExternalInput")
    w_d = {}
    for li in range(2, 7):
        fi, fo = DIMS[li - 2], DIMS[li - 1]
        wdt = f8 if li < 6 else bf16
        w_d[li] = nc.dram_tensor(f"w{li}t", [128, fi // 128, fo], wdt, kind="ExternalInput")
    bp_d = {}
    for li in range(2, 6):
        fot = math.ceil(DIMS[li - 1] / 128)
        bp_d[li] = nc.dram_tensor(f"b{li}p", [128, fot], f32, kind="ExternalInput")
    b6_d = nc.dram_tensor("b6p", [128, 1], f32, kind="ExternalInput")
    mft_d = nc.dram_tensor("mft", [128, M], bf16, kind="ExternalInput")
    out_d = nc.dram_tensor("out", [BS, M], odt, kind="ExternalOutput")
    av_d = nc.dram_tensor("av_scr", [BS], f32)  # addv row->partition bounce

    with tile.TileContext(nc) as tc, bass.ExitStack() as ctx:
        const = ctx.enter_context(tc.tile_pool(name="const", bufs=1))
        htp = ctx.enter_context(tc.tile_pool(name="htp", bufs=1))
        w1p = ctx.enter_context(tc.tile_pool(name="w1p", bufs=8))
        actp = ctx.enter_context(tc.tile_pool(name="actp", bufs=1))
        ost = ctx.enter_context(tc.tile_pool(name="ost", bufs=6))
        psp = ctx.enter_context(tc.tile_pool(name="psp", bufs=4, space="PSUM"))

        # ---- constants in SBUF (dispatched on the Scalar DMA queue) ----
        bp_sb = {}
        for li in range(2, 6):
            fot = math.ceil(DIMS[li - 1] / 128)
            bp_sb[li] = const.tile([128, fot], f32, name=f"b{li}p", tag=f"b{li}p")
            nc.scalar.dma_start(out=bp_sb[li][:], in_=bp_d[li][:])
        b6_sb = const.tile([128, 1], f32, name="b6p", tag="b6p")
        nc.scalar.dma_start(out=b6_sb[:], in_=b6_d[:])
        onesf = const.tile([128, 1], scdt, name="onesf", tag="onesf")
        nc.vector.memset(onesf[:], 1.0)
        # fp8 ones for the PE s-tail chain ([128,2,16]: DoubleRow weight
        # pair-step must be a multiple of 16 per the ISA)
        ones16 = const.tile([128, 2, 16], f8, name="ones16", tag="ones16")
        nc.vector.memset(ones16[:], 1.0)
        # per-partition mask for history tile 0 (extInfo rows excluded)
        rmask = const.tile([128, 1], f32, name="rmask", tag="rmask")
        nc.vector.memset(rmask[:], 1.0)
        nc.vector.memset(rmask[0:UINFO, :], 0.0)

        # ---- layer 1: one pass over history segments ----
        segs = []
        t0 = 0
        for tn in [2, 2]:
            segs.append((t0, tn))
            t0 += tn
        while t0 < T1:
            tn = min(HTC, T1 - t0)
            segs.append((t0, tn))
            t0 += tn
        NSEG = len(segs)

        x1t = actp.tile([128, FO1T, BS], f8, name="x1t", tag="x1t")
        c_acc = const.tile([128, 2, BS], scdt, name="c_acc", tag="c_acc")
        s_acc = const.tile([128, 2, BS], scdt, name="s_acc", tag="s_acc")
        # 4 paired PSUM tiles (2 banks each) -> bias-free paired epilogues
        ps1 = [psp.tile([128, 2, BS], f32, name="ps1", tag="ps") for _ in range(FO1T // 2)]

        nstep = T1 // 2
        SD = cfg["S_DVE_PAIRS"]
        s_tail = []  # (htt, lo) pair-steps whose s runs on the PE ones-chain
        step_i = 0
        sc_first = True
        for si_, (ts_, tn) in enumerate(segs):
            htt = htp.tile([128, tn, BS], f8, name="ht", tag="ht", bufs=NSEG)
            nc.sync.dma_start(out=htt[:], in_=ht_d[:, ts_ : ts_ + tn, :])

            lo = 0
            while lo < tn:
                t = ts_ + lo
                w1s = w1p.tile([128, 2, FO1], f8, name="w1s", tag="w1s")
                if step_i == 0:
                    h = FO1 // 2
                    nc.sync.dma_start(out=w1s[:, 0:2, 0:h], in_=w1t_d[:, t : t + 2, 0:h])
                    nc.sync.dma_start(out=w1s[:, 0:2, h:FO1], in_=w1t_d[:, t : t + 2, h:FO1])
                else:
                    nc.sync.dma_start(out=w1s[:, 0:2, :], in_=w1t_d[:, t : t + 2, :])
                for fo in range(FO1T):
                    fsl = slice(fo * 128, (fo + 1) * 128)
                    pdst = ps1[fo // 2][:, fo % 2, :]
                    nc.tensor.matmul(
                        pdst,
                        lhsT=w1s[:, 0:2, fsl],
                        rhs=htt[:, lo : lo + 2, :],
                        start=(step_i == 0),
                        stop=(step_i == nstep - 1),
                        perf_mode=mybir.MatmulPerfMode.DoubleRow,
                    )
                step_i += 1
                lo += 2

            # cnt chain (fused min+add) and the leading part of the s chain
            # (plain add) on the DVE under the layer-1 shadow, into 16-bit
            # accumulators (integer partials <= ~790, exact enough).  The
            # DVE can't hold BOTH full chains before the mid layers end, so
            # s pair-steps >= SD are deferred to a short PE ones-chain.
            # Segment 0 holds extInfo rows; init via masked ops.
            if sc_first:
                assert tn == 2
                nc.vector.tensor_scalar(
                    c_acc[:, 0, :], htt[:, 0, :], 1.0, rmask[:], op0=ALU.min, op1=ALU.mult
                )
                nc.vector.tensor_scalar(
                    c_acc[:, 1, :], htt[:, 1, :], 1.0, None, op0=ALU.min
                )
                nc.vector.tensor_scalar(
                    s_acc[:, 0, :], htt[:, 0, :], 1.0, rmask[:], op0=ALU.mult, op1=ALU.mult
                )
                nc.vector.tensor_copy(s_acc[:, 1, :], htt[:, 1, :])
                sc_first = False
            else:
                o = 0
                while o < tn:
                    nc.vector.scalar_tensor_tensor(
                        c_acc[:], htt[:, o : o + 2, :], 1.0, c_acc[:],
                        op0=ALU.min, op1=ALU.add,
                    )
                    if (ts_ + o) // 2 < SD:
                        nc.vector.tensor_add(s_acc[:], htt[:, o : o + 2, :], s_acc[:])
                    else:
                        s_tail.append((htt, o))
                    o += 2

        # layer-1 epilogue (bias-free: bias rode the matmul via the ones-row)
        for j in range(FO1T // 2):
            nc.scalar.activation(
                x1t[:, 2 * j : 2 * j + 2, :], ps1[j][:], AF.Lrelu,
                scale=w1_unscale, alpha=SLOPE,
            )

        # ---- remaining weights + movie factors: emitted late on the Sync
        # queue so the layer-1 ht/w1 stream gets the DMA bandwidth first.
        w_sb = {}
        for li in range(2, 7):
            fi, fo = DIMS[li - 2], DIMS[li - 1]
            wdt = f8 if li < 6 else bf16
            w_sb[li] = const.tile([128, fi // 128, fo], wdt, name=f"w{li}t", tag=f"w{li}t")
            nc.sync.dma_start(out=w_sb[li][:], in_=w_d[li][:])
        mft = const.tile([128, M], bf16, name="mft", tag="mft")
        nc.sync.dma_start(out=mft[:], in_=mft_d[:])

        # ---- layers 2..5 (lrelu) ----
        xin = x1t
        for li in range(2, 6):
            fi, fo = DIMS[li - 2], DIMS[li - 1]
            fit, fot = fi // 128, fo // 128
            xdt = f8 if li < 5 else bf16
            unsc = 1.0 / cfg["MLP_SCALES"][li - 2]
            xout = actp.tile(
                [128, fot, BS], xdt, name=f"x{li}t",
                tag=("x1t" if li == 5 else "x2t" if li == 4 else f"x{li}t"),
            )
            # interleave the k-accumulation across up to 4 fo-tile banks so
            # consecutive matmuls don't serialize on one bank's drain
            for g0 in range(0, fot, 4):
                gn = min(4, fot - g0)
                pss = [psp.tile([128, BS], f32, name="ps", tag="ps") for _ in range(gn)]
                ki = 0
                while ki < fit:
                    n2 = 2 if ki + 2 <= fit else 1
                    for j in range(gn):
                        ft = g0 + j
                        if n2 == 2:
                            nc.tensor.matmul(
                                pss[j][:],
                                lhsT=w_sb[li][:, ki : ki + 2, ft * 128 : (ft + 1) * 128],
                                rhs=xin[:, ki : ki + 2, :],
                                start=(ki == 0),
                                stop=(ki + 2 == fit),
                                perf_mode=mybir.MatmulPerfMode.DoubleRow,
                            )
                        else:
                            nc.tensor.matmul(
                                pss[j][:],
                                lhsT=w_sb[li][:, ki, ft * 128 : (ft + 1) * 128],
                                rhs=xin[:, ki, :],
                                start=(ki == 0),
                                stop=True,
                            )
                    ki += n2
                # epilogues all on the Scalar engine: the DVE queue carries
                # the s/cnt chains during layer 1 and would stall these
                # (strict FIFO), which in turn stalls the next layer's PE.
                for j in range(gn):
                    ft = g0 + j
                    nc.scalar.activation(
                        xout[:, ft, :], pss[j][:], AF.Lrelu,
                        bias=bp_sb[li][:, ft : ft + 1], scale=unsc, alpha=SLOPE,
                    )
            xin = xout

        # ---- cnt partition reduce + 1/cnt, emitted right after the mids:
        # c_acc is complete when the DVE chains drain (~layer-1 end), so the
        # slow DVE reciprocal runs far off the critical path, in the DVE's
        # idle window during the mids/s-tail.
        c_red = psp.tile([1, 2, BS], f32, name="c_red", tag="ps")
        nc.tensor.matmul(c_red[:, 0, :], lhsT=onesf[:], rhs=c_acc[:, 0, :], start=True, stop=True)
        nc.tensor.matmul(c_red[:, 1, :], lhsT=onesf[:], rhs=c_acc[:, 1, :], start=True, stop=True)
        # the host-appended ones-row counted +1 per batch (cnt chain half 0,
        # s PE-tail): subtract 1 from each.
        c0_sb = const.tile([1, BS], f32, name="c0_sb", tag="c0_sb")
        nc.vector.tensor_scalar_sub(c0_sb[:], c_red[0:1, 0, :], 1.0)
        c_sb = const.tile([1, BS], f32, name="c_sb", tag="c_sb")
        nc.vector.tensor_add(c_sb[:], c0_sb[:], c_red[0:1, 1, :])
        rc_sb = const.tile([1, BS], f32, name="rc_sb", tag="rc_sb")
        nc.vector.reciprocal(rc_sb[:], c_sb[:])

        # ---- PE s-tail: DoubleRow ones-chain over the deferred pair-steps
        # (history is long resident), double-buffered across one PSUM
        # slot's two banks.
        NT = len(s_tail)
        assert NT >= 2
        scx = psp.tile([16, 2, BS], f32, name="scx", tag="ps")
        for si, (htt, lo) in enumerate(s_tail):
            nc.tensor.matmul(
                scx[0:16, si % 2, :], lhsT=ones16[:], rhs=htt[:, lo : lo + 2, :],
                start=(si < 2), stop=(si >= NT - 2),
                perf_mode=mybir.MatmulPerfMode.DoubleRow,
            )
        s_red = psp.tile([1, 2, BS], f32, name="s_red", tag="ps")
        nc.tensor.matmul(s_red[:, 0, :], lhsT=onesf[:], rhs=s_acc[:, 0, :], start=True, stop=True)
        nc.tensor.matmul(s_red[:, 1, :], lhsT=onesf[:], rhs=s_acc[:, 1, :], start=True, stop=True)

        # ---- layer 6 (tanh) -> genf [65, BS] f32 ----
        fi, fo = DIMS[4], DIMS[5]
        fit = fi // 128
        assert fo == F + 1
        ps6 = psp.tile([fo, BS], f32, name="ps6", tag="ps")
        for ki in range(fit):
            nc.tensor.matmul(
                ps6[:],
                lhsT=w_sb[6][:, ki, 0:fo],
                rhs=xin[:, ki, :],
                start=(ki == 0),
                stop=(ki == fit - 1),
            )
        genf = actp.tile([fo, BS], f32, name="genf", tag="genf")
        nc.scalar.activation(genf[:], ps6[:], AF.Tanh, bias=b6_sb[0:fo, 0:1], scale=1.0)

        # ---- genext2: factor rows in bf16, duplicated into partitions
        # 64..127 (via SBUF->SBUF DMA) so reconstruction can row-tile the
        # PE.  Emitted BEFORE the meanV combine chain: the DVE queue is
        # strict FIFO and the cast gates the first reconstruction matmul.
        genext2 = actp.tile([128, BS], bf16, name="genext2", tag="genext")
        nc.vector.tensor_copy(genext2[0:F, :], genf[0:F, :])
        nc.sync.dma_start(out=genext2[F : 2 * F, :], in_=genext2[0:F, :])
        gl_sb = const.tile([1, BS], f32, name="gl_sb", tag="gl_sb")
        nc.sync.dma_start(out=gl_sb[:], in_=genf[F : F + 1, :])

        # s pre-scaled by OSC while combining the DVE halves + PE-tail banks
        s_sb = const.tile([1, BS], f32, name="s_sb", tag="s_sb")
        nc.vector.tensor_scalar(
            s_sb[:], s_red[0:1, 0, :], -1.0, float(OSC), op0=ALU.add, op1=ALU.mult
        )
        nc.vector.scalar_tensor_tensor(
            s_sb[:], s_red[0:1, 1, :], float(OSC), s_sb[:], op0=ALU.mult, op1=ALU.add
        )
        nc.vector.scalar_tensor_tensor(
            s_sb[:], scx[0:1, 0, :], float(OSC), s_sb[:], op0=ALU.mult, op1=ALU.add
        )
        nc.vector.scalar_tensor_tensor(
            s_sb[:], scx[0:1, 1, :], float(OSC), s_sb[:], op0=ALU.mult, op1=ALU.add
        )
        mv_sb = const.tile([1, BS], f32, name="mv_sb", tag="mv_sb")
        nc.vector.tensor_mul(mv_sb[:], rc_sb[:], s_sb[:])

        # addv = meanV*OSC + gen_last*OSC, bounced through DRAM into
        # per-partition layout [128, NBT] for the staging ops.
        av_sb = const.tile([1, BS], f32, name="av_sb", tag="av_sb")
        nc.vector.scalar_tensor_tensor(
            av_sb[:], gl_sb[:], float(OSC), mv_sb[:], op0=ALU.mult, op1=ALU.add
        )
        nc.sync.dma_start(out=av_d[:], in_=av_sb[0:1, :])
        addv_t = const.tile([128, NBT], f32, name="addv_t", tag="addv_t")
        nc.sync.dma_start(out=addv_t[:], in_=av_d.ap().rearrange("(t p) -> p t", p=128))

        # ---- reconstruction: out[bt*128+p, m] over movie chunk-pairs.
        # PE in 64x128 row-tiled mode: tile T0 (SBUF partitions 0-63) runs
        # the even chunk, tile T8 (64-127, the duplicated factor rows) the
        # odd chunk CONCURRENTLY, into adjacent PSUM banks.
        PAIRS = [CHUNKS[i : i + 2] for i in range(0, len(CHUNKS), 2)]
        for bt in range(NBT):
            lhsT_lo = genext2[0:F, bt * 128 : (bt + 1) * 128]
            lhsT_hi = genext2[F : 2 * F, bt * 128 : (bt + 1) * 128]
            st = None
            for pi, pair in enumerate(PAIRS):
                # staging alternates Scalar/Vector, time-balanced ~5:4
                # (GpSimd cannot read PSUM, so no third stager exists)
                eng = 0 if (pi % 9) in (0, 2, 4, 6, 8) else 1
                pr = psp.tile([128, 2, 512], f32, name="pr", tag="ps")
                for j, (co, cw) in enumerate(pair):
                    nc.tensor.matmul(
                        pr[:, j, 0:cw],
                        lhsT=(lhsT_lo if j == 0 else lhsT_hi),
                        rhs=(mft[0:F, co : co + cw] if j == 0 else mft[F : 2 * F, co : co + cw]),
                        start=True, stop=True,
                    )
                pw = sum(cw for _, cw in pair)
                if pi % 2 == 0:
                    st = ost.tile([128, 2048], odt, name="st", tag="st")
                    so, po = 0, pair[0][0]
                # stage the full [2,512] pair; only the valid prefix is DMA'd
                nst = 1024 if pw == 1024 else 512 + pair[1][1]
                pr2d = pr[:].opt()  # [128, 2, 512] -> contiguous [128, 1024]
                if eng == 0:
                    nc.scalar.activation(
                        st[:, so : so + 1024], pr2d, AF.Identity,
                        bias=addv_t[:, bt : bt + 1], scale=OSC,
                    )
                else:
                    nc.vector.tensor_scalar(
                        st[:, so : so + 1024], pr2d, OSC, addv_t[:, bt : bt + 1],
                        op0=ALU.mult, op1=ALU.add,
                    )
                so += nst
                if pi % 2 == 1 or pi == len(PAIRS) - 1:
                    # output drains alternate between the Sync and GpSimd
                    # DMA queues (both engines idle during reconstruction;
                    # descriptor issue costs ~0.6us each)
                    q = nc.sync if (pi // 2) % 2 == 0 else nc.gpsimd
                    q.dma_start(
                        out=out_d[bt * 128 : (bt + 1) * 128, po : po + so],
                        in_=st[:, 0:so],
                    )

    nc.compile()
    return nc


def prep_in_maps(cfg, inputs):
    """Shard + lay out the full inputs into per-core DRAM input maps."""
    d = _derived(cfg)
    BS, UINFO, M, F, DIMS, T1 = cfg["BS"], cfg["UINFO"], cfg["M"], cfg["F"], cfg["DIMS"], d["T1"]
    extInfo = np.asarray(inputs["extInfo"], np.float32)
    ratings = np.asarray(inputs["ratings"], np.float32)

    # BN (eval) fold into layer 2: y = g'(lrelu1) + b' with g' = bn_g/sqrt(1+eps)
    g = np.asarray(inputs["bn_g"], np.float32) / np.float32(np.sqrt(1.0 + BN_EPS))
    bnb = np.asarray(inputs["bn_b"], np.float32)
    w2 = np.asarray(inputs["w2"], np.float32)
    w2f = w2 * g[None, :]
    b2f = np.asarray(inputs["b2"], np.float32) + w2 @ bnb

    shared = {}
    # w1t: [KH,FO1] -> padded [T1*128, FO1] -> [128, T1, FO1]; the row at
    # index UINFO+M carries b1 (matching the ones-row in the history).
    w1 = np.asarray(inputs["w1"], np.float32)
    b1 = np.asarray(inputs["b1"], np.float32)
    FO1 = DIMS[0]
    w1tp = np.zeros((T1 * 128, FO1), FP8)
    w1tp[0 : w1.shape[1]] = (w1.T * np.float32(cfg["W1_SCALE"])).astype(FP8)
    w1tp[UINFO + M] = (b1 * np.float32(cfg["W1_SCALE"])).astype(FP8)
    shared["w1t"] = np.ascontiguousarray(w1tp.reshape(T1, 128, FO1).transpose(1, 0, 2))

    def pack_w(wT, fo, dt=BF16, scale=1.0):
        fi = wT.shape[0]
        w = (wT.astype(np.float32) * np.float32(scale)).astype(dt)
        return np.ascontiguousarray(w.reshape(fi // 128, 128, fo).transpose(1, 0, 2))

    scs = cfg["MLP_SCALES"]
    shared["w2t"] = pack_w(w2f.T, DIMS[1], FP8, scs[0])
    for li, wname in ((3, "w3"), (4, "w4"), (5, "w5"), (6, "w6")):
        w = np.asarray(inputs[wname], np.float32)
        fo = DIMS[li - 1]
        if li < 6:
            shared[f"w{li}t"] = pack_w(w.T, fo, FP8, scs[li - 2])
        else:
            shared[f"w{li}t"] = pack_w(w.T, fo)

    def pack_b(b, fo):
        fot = math.ceil(fo / 128)
        bp = np.zeros(fot * 128, np.float32)
        bp[:fo] = b
        return np.ascontiguousarray(bp.reshape(fot, 128).T)

    bsrc = {2: b2f}
    for li in (3, 4, 5):
        bsrc[li] = np.asarray(inputs[f"b{li}"], np.float32)
    for li in range(2, 6):
        shared[f"b{li}p"] = pack_b(bsrc[li], DIMS[li - 1])
    shared["b6p"] = pack_b(np.asarray(inputs["b6"], np.float32), DIMS[5])

    # bf16 mft: factor rows duplicated into partitions 64..127 for the
    # row-tiled reconstruction (movie_bias is added on host at dequant).
    mft = np.zeros((128, M), BF16)
    mft[0:F] = np.asarray(inputs["movie_factors"], np.float32).T.astype(BF16)
    mft[F : 2 * F] = mft[0:F]
    shared["mft"] = mft

    in_maps = []
    for c in range(NCORES):
        sl = slice(c * BS, (c + 1) * BS)
        htc = np.zeros((T1 * 128, BS), FP8)
        htc[0:UINFO] = extInfo[sl].T.astype(FP8)
        htc[UINFO : UINFO + M] = ratings[sl].T.astype(FP8)
        htc[UINFO + M] = np.float32(1.0)  # ones-row: picks up b1 from w1t
        m = dict(shared)
        m["ht"] = np.ascontiguousarray(htc.reshape(T1, 128, BS).transpose(1, 0, 2))
        in_maps.append(m)
    return in_maps


_NC_CACHE = {}


def run_on_hw(cfg, inputs, trace=False):
    from concourse.bass_utils import run_bass_kernel_spmd

    key = tuple(sorted((k, v) for k, v in cfg.items() if k != "DIMS")) + (cfg["DIMS"],)
    if key not in _NC_CACHE:
        _NC_CACHE[key] = build_nc(cfg)
    nc = _NC_CACHE[key]
    in_maps = prep_in_maps(cfg, inputs)
    br = run_bass_kernel_spmd(nc, in_maps, list(range(NCORES)), trace=trace)
    BS, M = cfg["BS"], cfg["M"]
    out = np.empty((NCORES * BS, M), np.float32)
    dq = np.float32(1.0 / cfg["OUT_SCALE"]) if cfg["OUT_DT"] == "i8" else np.float32(1.0)
    mb = np.asarray(inputs["movie_bias"], np.float32)[None, :]
    for c in range(NCORES):
        out[c * BS : (c + 1) * BS] = (
            np.asarray(br.results[c]["out"], dtype=np.float32) * dq + mb
        )
    return out, br


def kernel(**inputs) -> np.ndarray:
    try:
        out, _ = run_on_hw(FULL_CFG, inputs, trace=False)
    except Exception:
        # one retry for transient device/runtime hiccups
        out, _ = run_on_hw(FULL_CFG, inputs, trace=False)
    return out


# revision 22
# speedup vs baseline: 1.1210x; 1.0057x over previous
"""Trainium2 Bass kernel for nn_DLFG_79817672229311 (segment_reduce).

Computes, data-parallel over the batch axis on 8 NeuronCores:
  history = [extInfo, ratings, 1]                    # [BS, 20033] per core
  x1 = lrelu(history @ [w1;b1].T); BN folded into w2 on host
  x2..x5 = lrelu(x @ wl.T + bl)
  gen = tanh(x5 @ w6.T + b6)                         # [BS, 65]
  s, cnt = per-row sum / count of nonzero ratings
  addv = s/cnt + gen[:, 64]
  out = gen[:, :64] @ movie_factors.T + addv[:, None] + movie_bias

Design (per core; layer 1 is at the fp8 DoubleRow compute wall ~135us, so
everything else hides under or packs tightly around it):
- Activations ride transposed ([feature, batch]): batch (512) is the matmul
  free dim, features the partition dim, so no on-device transposes are needed.
- History is staged to SBUF once in fp8 (ratings 0..5 are exact in e4m3) with
  a host-appended ones-row; layer 1 runs fp8 DoubleRow against 2^15-pre-scaled
  fp8 [w1;b1] slabs streamed from HBM, so the bias rides the contraction and
  the epilogue is a bias-free paired Lrelu.  K is host-padded to an even tile
  count so every step is a DoubleRow pair.
- BOTH cnt and s accumulate on the Vector engine under the layer-1 shadow as
  paired fused chains (cnt: min+add, s: plain add) into fp16 accumulators
  (integer partial sums <= 790 are exact in fp16; fp16 halves DVE traffic).
  This removes the former PE ones-matmul s-chain (~16us of Tensor time).
  Partition reduction is 4 tiny fp16 matmuls; the ones-row contribution is
  subtracted in the combine.
- Mid layers interleave their k-accumulation across 3 PSUM banks; some lrelu
  epilogues run on the DVE.
- addv bounces through DRAM into per-partition [128, NBT] and enters the
  reconstruction staging op as its per-partition bias.
- Reconstruction exploits PE row tiling (64x128 mode): the contraction is
  only the 64 factors (movie_bias is added on HOST during dequant), so the
  factor block is duplicated into SBUF partitions 64..127 (both in mft and in
  genext2) and each chunk-pair issues two CONCURRENT matmuls -- tile T0
  (SBUF rows 0-63) on the even chunk, tile T8 (rows 64-127) on the odd chunk,
  landing in adjacent PSUM banks.  This halves reconstruction Tensor time.
- Recon staging: fused scale+bias on alternating Vector/Scalar engines ->
  int8 DRAM in 2048-col blocks, with the output drains alternating between
  the Sync and GpSimd DMA queues so descriptor issue isn't single-queue
  limited.  Host dequantizes by the fixed scale 4/127 and adds movie_bias.
"""

import math
import sys

sys.path.insert(0, "/opt/trn_rl_repo")

import numpy as np
import ml_dtypes

BF16 = ml_dtypes.bfloat16
FP8 = ml_dtypes.float8_e4m3

NCORES = 8
BN_EPS = 0.05
SLOPE = 0.01

FULL_CFG = dict(
    BS=512,  # per-core batch
    UINFO=32,
    M=20000,
    F=64,
    DIMS=(1024, 512, 256, 512, 1024, 65),  # fan-outs of the 6 linear layers
    HTC=4,  # history K-tiles per DMA chunk (must be even for DoubleRow pairs)
    W1_SCALE=2.0**15,  # fp8 pre-scale: w1 ~ U(+-0.007) sits in e4m3 subnormals
    MLP_SCALES=(4096.0, 4096.0, 2048.0, 4096.0),  # 2^k per layer, |w|*s < 240
    OUT_DT="i8",  # "i8" (host dequant) or "bf16"
    OUT_SCALE=127.0 / 4.0,  # int8 quantization scale (|out| <= ~3.2)
    SC_DT="bf16",  # s/cnt DVE accumulator dtype ("bf16", "f16" or "f32")
    S_DVE_PAIRS=54,  # leading k-pair-steps whose s-accum rides the DVE;
    # the rest run as a PE ones-matmul chain after the mid layers (the DVE
    # can't hold both full chains under the layer-1 shadow, and GpSimd
    # chains poison SBUF bandwidth for everyone)
)


def _derived(cfg):
    d = dict(cfg)
    d["KH"] = cfg["UINFO"] + cfg["M"] + 1  # +1 ones-row carrying b1
    t1 = math.ceil(d["KH"] / 128)
    d["T1"] = t1 + (t1 % 2)  # pad to even so all steps are DoubleRow pairs
    d["NBT"] = cfg["BS"] // 128  # batch tiles per core
    d["CHUNKS"] = [(o, min(512, cfg["M"] - o)) for o in range(0, cfg["M"], 512)]
    return d


def build_nc(cfg):
    """Build + compile the (single-core SPMD) Bass program."""
    import concourse.bass as bass
    import concourse.tile as tile
    from concourse import bacc, mybir

    d = _derived(cfg)
    BS, UINFO, M, F = cfg["BS"], cfg["UINFO"], cfg["M"], cfg["F"]
    DIMS = cfg["DIMS"]
    T1, NBT, CHUNKS, HTC = d["T1"], d["NBT"], d["CHUNKS"], cfg["HTC"]
    FO1 = DIMS[0]
    FO1T = FO1 // 128
    w1_unscale = 1.0 / cfg["W1_SCALE"]
    f32 = mybir.dt.float32
    bf16 = mybir.dt.bfloat16
    f16 = mybir.dt.float16
    f8 = mybir.dt.float8e4
    i8 = mybir.dt.int8
    AF = mybir.ActivationFunctionType
    ALU = mybir.AluOpType

    OUT_I8 = cfg["OUT_DT"] == "i8"
    odt = i8 if OUT_I8 else bf16
    OSC = cfg["OUT_SCALE"] if OUT_I8 else 1.0
    scdt = {"bf16": bf16, "f16": f16, "f32": f32}[cfg["SC_DT"]]

    nc = bacc.Bacc("TRN2", target_bir_lowering=False, debug=False)

    # ---- DRAM I/O ----
    ht_d = nc.dram_tensor("ht", [128, T1, BS], f8, kind="ExternalInput")
    w1t_d = nc.dram_tensor("w1t", [128, T1, FO1], f8, kind="ExternalInput")
    w_d = {}
    for li in range(2, 7):
        fi, fo = DIMS[li - 2], DIMS[li - 1]
        wdt = f8 if li < 6 else bf16
        w_d[li] = nc.dram_tensor(f"w{li}t", [128, fi // 128, fo], wdt, kind="ExternalInput")
    bp_d = {}
    for li in range(2, 6):
        fot = math.ceil(DIMS[li - 1] / 128)
        bp_d[li] = nc.dram_tensor(f"b{li}p", [128, fot], f32, kind="ExternalInput")
    b6_d = nc.dram_tensor("b6p", [128, 1], f32, kind="ExternalInput")
    mft_d = nc.dram_tensor("mft", [128, M], bf16, kind="ExternalInput")
    out_d = nc.dram_tensor("out", [BS, M], odt, kind="ExternalOutput")
    av_d = nc.dram_tensor("av_scr", [BS], f32)  # addv row->partition bounce

    with tile.TileContext(nc) as tc, bass.ExitStack() as ctx:
        const = ctx.enter_context(tc.tile_pool(name="const", bufs=1))
        htp = ctx.enter_context(tc.tile_pool(name="htp", bufs=1))
        w1p = ctx.enter_context(tc.tile_pool(name="w1p", bufs=8))
        actp = ctx.enter_context(tc.tile_pool(name="actp", bufs=1))
        ost = ctx.enter_context(tc.tile_pool(name="ost", bufs=6))
        psp = ctx.enter_context(tc.tile_pool(name="psp", bufs=4, space="PSUM"))

        # ---- constants in SBUF (dispatched on the Scalar DMA queue) ----
        bp_sb = {}
        for li in range(2, 6):
            fot = math.ceil(DIMS[li - 1] / 128)
            bp_sb[li] = const.tile([128, fot], f32, name=f"b{li}p", tag=f"b{li}p")
            nc.scalar.dma_start(out=bp_sb[li][:], in_=bp_d[li][:])
        b6_sb = const.tile([128, 1], f32, name="b6p", tag="b6p")
        nc.scalar.dma_start(out=b6_sb[:], in_=b6_d[:])
        onesf = const.tile([128, 1], scdt, name="onesf", tag="onesf")
        nc.vector.memset(onesf[:], 1.0)
        # fp8 ones for the PE s-tail chain ([128,2,16]: DoubleRow weight
        # pair-step must be a multiple of 16 per the ISA)
        ones16 = const.tile([128, 2, 16], f8, name="ones16", tag="ones16")
        nc.vector.memset(ones16[:], 1.0)
        # per-partition mask for history tile 0 (extInfo rows excluded)
        rmask = const.tile([128, 1], f32, name="rmask", tag="rmask")
        nc.vector.memset(rmask[:], 1.0)
        nc.vector.memset(rmask[0:UINFO, :], 0.0)

        # ---- layer 1: one pass over history segments ----
        segs = []
        t0 = 0
        for tn in [2, 2]:
            segs.append((t0, tn))
            t0 += tn
        while t0 < T1:
            tn = min(HTC, T1 - t0)
            segs.append((t0, tn))
            t0 += tn
        NSEG = len(segs)

        x1t = actp.tile([128, FO1T, BS], f8, name="x1t", tag="x1t")
        c_acc = const.tile([128, 2, BS], scdt, name="c_acc", tag="c_acc")
        s_acc = const.tile([128, 2, BS], scdt, name="s_acc", tag="s_acc")
        # 4 paired PSUM tiles (2 banks each) -> bias-free paired epilogues
        ps1 = [psp.tile([128, 2, BS], f32, name="ps1", tag="ps") for _ in range(FO1T // 2)]

        nstep = T1 // 2
        SD = cfg["S_DVE_PAIRS"]
        s_tail = []  # (htt, lo) pair-steps whose s runs on the PE ones-chain
        step_i = 0
        sc_first = True
        for si_, (ts_, tn) in enumerate(segs):
            htt = htp.tile([128, tn, BS], f8, name="ht", tag="ht", bufs=NSEG)
            nc.sync.dma_start(out=htt[:], in_=ht_d[:, ts_ : ts_ + tn, :])

            lo = 0
            while lo < tn:
                t = ts_ + lo
                w1s = w1p.tile([128, 2, FO1], f8, name="w1s", tag="w1s")
                if step_i == 0:
                    h = FO1 // 2
                    nc.sync.dma_start(out=w1s[:, 0:2, 0:h], in_=w1t_d[:, t : t + 2, 0:h])
                    nc.sync.dma_start(out=w1s[:, 0:2, h:FO1], in_=w1t_d[:, t : t + 2, h:FO1])
                else:
                    nc.sync.dma_start(out=w1s[:, 0:2, :], in_=w1t_d[:, t : t + 2, :])
                for fo in range(FO1T):
                    fsl = slice(fo * 128, (fo + 1) * 128)
                    pdst = ps1[fo // 2][:, fo % 2, :]
                    nc.tensor.matmul(
                        pdst,
                        lhsT=w1s[:, 0:2, fsl],
                        rhs=htt[:, lo : lo + 2, :],
                        start=(step_i == 0),
                        stop=(step_i == nstep - 1),
                        perf_mode=mybir.MatmulPerfMode.DoubleRow,
                    )
                step_i += 1
                lo += 2

            # cnt chain (fused min+add) and the leading part of the s chain
            # (plain add) on the DVE under the layer-1 shadow, into 16-bit
            # accumulators (integer partials <= ~790, exact enough).  The
            # DVE can't hold BOTH full chains before the mid layers end, so
            # s pair-steps >= SD are deferred to a short PE ones-chain.
            # Segment 0 holds extInfo rows; init via masked ops.
            if sc_first:
                assert tn == 2
                nc.vector.tensor_scalar(
                    c_acc[:, 0, :], htt[:, 0, :], 1.0, rmask[:], op0=ALU.min, op1=ALU.mult
                )
                nc.vector.tensor_scalar(
                    c_acc[:, 1, :], htt[:, 1, :], 1.0, None, op0=ALU.min
                )
                nc.vector.tensor_scalar(
                    s_acc[:, 0, :], htt[:, 0, :], 1.0, rmask[:], op0=ALU.mult, op1=ALU.mult
                )
                nc.vector.tensor_copy(s_acc[:, 1, :], htt[:, 1, :])
                sc_first = False
            else:
                o = 0
                while o < tn:
                    nc.vector.scalar_tensor_tensor(
                        c_acc[:], htt[:, o : o + 2, :], 1.0, c_acc[:],
                        op0=ALU.min, op1=ALU.add,
                    )
                    if (ts_ + o) // 2 < SD:
                        nc.vector.tensor_add(s_acc[:], htt[:, o : o + 2, :], s_acc[:])
                    else:
                        s_tail.append((htt, o))
                    o += 2

        # layer-1 epilogue (bias-free: bias rode the matmul via the ones-row)
        for j in range(FO1T // 2):
            nc.scalar.activation(
                x1t[:, 2 * j : 2 * j + 2, :], ps1[j][:], AF.Lrelu,
                scale=w1_unscale, alpha=SLOPE,
            )

        # ---- remaining weights + movie factors: emitted late on the Sync
        # queue so the layer-1 ht/w1 stream gets the DMA bandwidth first.
        w_sb = {}
        for li in range(2, 7):
            fi, fo = DIMS[li - 2], DIMS[li - 1]
            wdt = f8 if li < 6 else bf16
            w_sb[li] = const.tile([128, fi // 128, fo], wdt, name=f"w{li}t", tag=f"w{li}t")
            nc.sync.dma_start(out=w_sb[li][:], in_=w_d[li][:])
        mft = const.tile([128, M], bf16, name="mft", tag="mft")
        nc.sync.dma_start(out=mft[:], in_=mft_d[:])

        # ---- layers 2..5 (lrelu) ----
        xin = x1t
        for li in range(2, 6):
            fi, fo = DIMS[li - 2], DIMS[li - 1]
            fit, fot = fi // 128, fo // 128
            xdt = f8 if li < 5 else bf16
            unsc = 1.0 / cfg["MLP_SCALES"][li - 2]
            xout = actp.tile(
                [128, fot, BS], xdt, name=f"x{li}t",
                tag=("x1t" if li == 5 else "x2t" if li == 4 else f"x{li}t"),
            )
            # interleave the k-accumulation across up to 4 fo-tile banks so
            # consecutive matmuls don't serialize on one bank's drain
            for g0 in range(0, fot, 4):
                gn = min(4, fot - g0)
                pss = [psp.tile([128, BS], f32, name="ps", tag="ps") for _ in range(gn)]
                ki = 0
                while ki < fit:
                    n2 = 2 if ki + 2 <= fit else 1
                    for j in range(gn):
                        ft = g0 + j
                        if n2 == 2:
                            nc.tensor.matmul(
                                pss[j][:],
                                lhsT=w_sb[li][:, ki : ki + 2, ft * 128 : (ft + 1) * 128],
                                rhs=xin[:, ki : ki + 2, :],
                                start=(ki == 0),
                                stop=(ki + 2 == fit),
                                perf_mode=mybir.MatmulPerfMode.DoubleRow,
                            )
                        else:
                            nc.tensor.matmul(
                                pss[j][:],
                                lhsT=w_sb[li][:, ki, ft * 128 : (ft + 1) * 128],
                                rhs=xin[:, ki, :],
                                start=(ki == 0),
                                stop=True,
                            )
                    ki += n2
                # epilogues all on the Scalar engine: the DVE queue carries
                # the s/cnt chains during layer 1 and would stall these
                # (strict FIFO), which in turn stalls the next layer's PE.
                for j in range(gn):
                    ft = g0 + j
                    nc.scalar.activation(
                        xout[:, ft, :], pss[j][:], AF.Lrelu,
                        bias=bp_sb[li][:, ft : ft + 1], scale=unsc, alpha=SLOPE,
                    )
            xin = xout

        # ---- gen_last early: a 1-column slice of layer 6 + tanh, emitted
        # right before the s-tail so addv doesn't wait for the full layer-6
        # matmul + tanh.  8 tiny matmuls (~60cyc each) + one [1,BS] ACT.
        fi6, fo6 = DIMS[4], DIMS[5]
        fit6 = fi6 // 128
        assert fo6 == F + 1
        ps6b = psp.tile([1, BS], f32, name="ps6b", tag="ps")
        for ki in range(fit6):
            nc.tensor.matmul(
                ps6b[:],
                lhsT=w_sb[6][:, ki, F : F + 1],
                rhs=xin[:, ki, :],
                start=(ki == 0),
                stop=(ki == fit6 - 1),
            )
        genl = const.tile([1, BS], f32, name="genl", tag="genl")
        nc.scalar.activation(genl[:], ps6b[:], AF.Tanh, bias=b6_sb[F : F + 1, 0:1], scale=1.0)

        # ---- cnt partition reduce + 1/cnt, emitted right after the mids:
        # c_acc is complete when the DVE chains drain (~layer-1 end), so the
        # slow DVE reciprocal runs far off the critical path, in the DVE's
        # idle window during the mids/s-tail.
        c_red = psp.tile([1, 2, BS], f32, name="c_red", tag="ps")
        nc.tensor.matmul(c_red[:, 0, :], lhsT=onesf[:], rhs=c_acc[:, 0, :], start=True, stop=True)
        nc.tensor.matmul(c_red[:, 1, :], lhsT=onesf[:], rhs=c_acc[:, 1, :], start=True, stop=True)
        # the host-appended ones-row counted +1 per batch (cnt chain half 0,
        # s PE-tail): subtract 1 from each.
        c0_sb = const.tile([1, BS], f32, name="c0_sb", tag="c0_sb")
        nc.vector.tensor_scalar_sub(c0_sb[:], c_red[0:1, 0, :], 1.0)
        c_sb = const.tile([1, BS], f32, name="c_sb", tag="c_sb")
        nc.vector.tensor_add(c_sb[:], c0_sb[:], c_red[0:1, 1, :])
        rc_sb = const.tile([1, BS], f32, name="rc_sb", tag="rc_sb")
        nc.vector.reciprocal(rc_sb[:], c_sb[:])

        # ---- PE s-tail: DoubleRow ones-chain over the deferred pair-steps
        # (history is long resident), double-buffered across one PSUM
        # slot's two banks.
        NT = len(s_tail)
        assert NT >= 2
        scx = psp.tile([16, 2, BS], f32, name="scx", tag="ps")
        for si, (htt, lo) in enumerate(s_tail):
            nc.tensor.matmul(
                scx[0:16, si % 2, :], lhsT=ones16[:], rhs=htt[:, lo : lo + 2, :],
                start=(si < 2), stop=(si >= NT - 2),
                perf_mode=mybir.MatmulPerfMode.DoubleRow,
            )
        s_red = psp.tile([1, 2, BS], f32, name="s_red", tag="ps")
        nc.tensor.matmul(s_red[:, 0, :], lhsT=onesf[:], rhs=s_acc[:, 0, :], start=True, stop=True)
        nc.tensor.matmul(s_red[:, 1, :], lhsT=onesf[:], rhs=s_acc[:, 1, :], start=True, stop=True)

        # ---- layer 6 (tanh) -> genf [65, BS] f32 ----
        fi, fo = DIMS[4], DIMS[5]
        fit = fi // 128
        assert fo == F + 1
        ps6 = psp.tile([fo, BS], f32, name="ps6", tag="ps")
        for ki in range(fit):
            nc.tensor.matmul(
                ps6[:],
                lhsT=w_sb[6][:, ki, 0:fo],
                rhs=xin[:, ki, :],
                start=(ki == 0),
                stop=(ki == fit - 1),
            )
        genf = actp.tile([fo, BS], f32, name="genf", tag="genf")
        nc.scalar.activation(genf[:], ps6[:], AF.Tanh, bias=b6_sb[0:fo, 0:1], scale=1.0)

        # ---- genext2: factor rows in bf16, duplicated into partitions
        # 64..127 (via SBUF->SBUF DMA) so reconstruction can row-tile the
        # PE.  Emitted BEFORE the meanV combine chain: the DVE queue is
        # strict FIFO and the cast gates the first reconstruction matmul.
        genext2 = actp.tile([128, BS], bf16, name="genext2", tag="genext")
        nc.vector.tensor_copy(genext2[0:F, :], genf[0:F, :])
        nc.sync.dma_start(out=genext2[F : 2 * F, :], in_=genext2[0:F, :])

        # s pre-scaled by OSC while combining the DVE halves + PE-tail banks
        s_sb = const.tile([1, BS], f32, name="s_sb", tag="s_sb")
        nc.vector.tensor_scalar(
            s_sb[:], s_red[0:1, 0, :], -1.0, float(OSC), op0=ALU.add, op1=ALU.mult
        )
        nc.vector.scalar_tensor_tensor(
            s_sb[:], s_red[0:1, 1, :], float(OSC), s_sb[:], op0=ALU.mult, op1=ALU.add
        )
        nc.vector.scalar_tensor_tensor(
            s_sb[:], scx[0:1, 0, :], float(OSC), s_sb[:], op0=ALU.mult, op1=ALU.add
        )
        nc.vector.scalar_tensor_tensor(
            s_sb[:], scx[0:1, 1, :], float(OSC), s_sb[:], op0=ALU.mult, op1=ALU.add
        )
        mv_sb = const.tile([1, BS], f32, name="mv_sb", tag="mv_sb")
        nc.vector.tensor_mul(mv_sb[:], rc_sb[:], s_sb[:])

        # addv = meanV*OSC + gen_last*OSC, bounced through DRAM into
        # per-partition layout [128, NBT] for the staging ops (gen_last came
        # from the early layer-6 slice, so this only waits on the s combine).
        av_sb = const.tile([1, BS], f32, name="av_sb", tag="av_sb")
        nc.vector.scalar_tensor_tensor(
            av_sb[:], genl[:], float(OSC), mv_sb[:], op0=ALU.mult, op1=ALU.add
        )
        nc.sync.dma_start(out=av_d[:], in_=av_sb[0:1, :])
        addv_t = const.tile([128, NBT], f32, name="addv_t", tag="addv_t")
        nc.sync.dma_start(out=addv_t[:], in_=av_d.ap().rearrange("(t p) -> p t", p=128))

        # ---- reconstruction: out[bt*128+p, m] over movie chunk-pairs.
        # PE in 64x128 row-tiled mode: tile T0 (SBUF partitions 0-63) runs
        # the even chunk, tile T8 (64-127, the duplicated factor rows) the
        # odd chunk CONCURRENTLY, into adjacent PSUM banks.
        PAIRS = [CHUNKS[i : i + 2] for i in range(0, len(CHUNKS), 2)]
        for bt in range(NBT):
            lhsT_lo = genext2[0:F, bt * 128 : (bt + 1) * 128]
            lhsT_hi = genext2[F : 2 * F, bt * 128 : (bt + 1) * 128]
            st = None
            for pi, pair in enumerate(PAIRS):
                # staging alternates Scalar/Vector, time-balanced ~5:4
                # (GpSimd cannot read PSUM, so no third stager exists)
                eng = 0 if (pi % 9) in (0, 2, 4, 6, 8) else 1
                pr = psp.tile([128, 2, 512], f32, name="pr", tag="ps")
                for j, (co, cw) in enumerate(pair):
                    nc.tensor.matmul(
                        pr[:, j, 0:cw],
                        lhsT=(lhsT_lo if j == 0 else lhsT_hi),
                        rhs=(mft[0:F, co : co + cw] if j == 0 else mft[F : 2 * F, co : co + cw]),
                        start=True, stop=True,
                    )
                pw = sum(cw for _, cw in pair)
                if pi % 2 == 0:
                    st = ost.tile([128, 2048], odt, name="st", tag="st")
                    so, po = 0, pair[0][0]
                # stage the full [2,512] pair; only the valid prefix is DMA'd
                nst = 1024 if pw == 1024 else 512 + pair[1][1]
                pr2d = pr[:].opt()  # [128, 2, 512] -> contiguous [128, 1024]
                if eng == 0:
                    nc.scalar.activation(
                        st[:, so : so + 1024], pr2d, AF.Identity,
                        bias=addv_t[:, bt : bt + 1], scale=OSC,
                    )
                else:
                    nc.vector.tensor_scalar(
                        st[:, so : so + 1024], pr2d, OSC, addv_t[:, bt : bt + 1],
                        op0=ALU.mult, op1=ALU.add,
                    )
                so += nst
                if pi % 2 == 1 or pi == len(PAIRS) - 1:
                    # output drains alternate between the Sync and GpSimd
                    # DMA queues (both engines idle during reconstruction;
                    # descriptor issue costs ~0.6us each)
                    q = nc.sync if (pi // 2) % 2 == 0 else nc.gpsimd
                    q.dma_start(
                        out=out_d[bt * 128 : (bt + 1) * 128, po : po + so],
                        in_=st[:, 0:so],
                    )

    nc.compile()
    return nc


def prep_in_maps(cfg, inputs):
    """Shard + lay out the full inputs into per-core DRAM input maps."""
    d = _derived(cfg)
    BS, UINFO, M, F, DIMS, T1 = cfg["BS"], cfg["UINFO"], cfg["M"], cfg["F"], cfg["DIMS"], d["T1"]
    extInfo = np.asarray(inputs["extInfo"], np.float32)
    ratings = np.asarray(inputs["ratings"], np.float32)

    # BN (eval) fold into layer 2: y = g'(lrelu1) + b' with g' = bn_g/sqrt(1+eps)
    g = np.asarray(inputs["bn_g"], np.float32) / np.float32(np.sqrt(1.0 + BN_EPS))
    bnb = np.asarray(inputs["bn_b"], np.float32)
    w2 = np.asarray(inputs["w2"], np.float32)
    w2f = w2 * g[None, :]
    b2f = np.asarray(inputs["b2"], np.float32) + w2 @ bnb

    shared = {}
    # w1t: [KH,FO1] -> padded [T1*128, FO1] -> [128, T1, FO1]; the row at
    # index UINFO+M carries b1 (matching the ones-row in the history).
    w1 = np.asarray(inputs["w1"], np.float32)
    b1 = np.asarray(inputs["b1"], np.float32)
    FO1 = DIMS[0]
    w1tp = np.zeros((T1 * 128, FO1), FP8)
    w1tp[0 : w1.shape[1]] = (w1.T * np.float32(cfg["W1_SCALE"])).astype(FP8)
    w1tp[UINFO + M] = (b1 * np.float32(cfg["W1_SCALE"])).astype(FP8)
    shared["w1t"] = np.ascontiguousarray(w1tp.reshape(T1, 128, FO1).transpose(1, 0, 2))

    def pack_w(wT, fo, dt=BF16, scale=1.0):
        fi = wT.shape[0]
        w = (wT.astype(np.float32) * np.float32(scale)).astype(dt)
        return np.ascontiguousarray(w.reshape(fi // 128, 128, fo).transpose(1, 0, 2))

    scs = cfg["MLP_SCALES"]
    shared["w2t"] = pack_w(w2f.T, DIMS[1], FP8, scs[0])
    for li, wname in ((3, "w3"), (4, "w4"), (5, "w5"), (6, "w6")):
        w = np.asarray(inputs[wname], np.float32)
        fo = DIMS[li - 1]
        if li < 6:
            shared[f"w{li}t"] = pack_w(w.T, fo, FP8, scs[li - 2])
        else:
            shared[f"w{li}t"] = pack_w(w.T, fo)

    def pack_b(b, fo):
        fot = math.ceil(fo / 128)
        bp = np.zeros(fot * 128, np.float32)
        bp[:fo] = b
        return np.ascontiguousarray(bp.reshape(fot, 128).T)

    bsrc = {2: b2f}
    for li in (3, 4, 5):
        bsrc[li] = np.asarray(inputs[f"b{li}"], np.float32)
    for li in range(2, 6):
        shared[f"b{li}p"] = pack_b(bsrc[li], DIMS[li - 1])
    shared["b6p"] = pack_b(np.asarray(inputs["b6"], np.float32), DIMS[5])

    # bf16 mft: factor rows duplicated into partitions 64..127 for the
    # row-tiled reconstruction (movie_bias is added on host at dequant).
    mft = np.zeros((128, M), BF16)
    mft[0:F] = np.asarray(inputs["movie_factors"], np.float32).T.astype(BF16)
    mft[F : 2 * F] = mft[0:F]
    shared["mft"] = mft

    in_maps = []
    for c in range(NCORES):
        sl = slice(c * BS, (c + 1) * BS)
        htc = np.zeros((T1 * 128, BS), FP8)
        htc[0:UINFO] = extInfo[sl].T.astype(FP8)
        htc[UINFO : UINFO + M] = ratings[sl].T.astype(FP8)
        htc[UINFO + M] = np.float32(1.0)  # ones-row: picks up b1 from w1t
        m = dict(shared)
        m["ht"] = np.ascontiguousarray(htc.reshape(T1, 128, BS).transpose(1, 0, 2))
        in_maps.append(m)
    return in_maps


_NC_CACHE = {}


def run_on_hw(cfg, inputs, trace=False):
    from concourse.bass_utils import run_bass_kernel_spmd

    key = tuple(sorted((k, v) for k, v in cfg.items() if k != "DIMS")) + (cfg["DIMS"],)
    if key not in _NC_CACHE:
        _NC_CACHE[key] = build_nc(cfg)
    nc = _NC_CACHE[key]
    in_maps = prep_in_maps(cfg, inputs)
    br = run_bass_kernel_spmd(nc, in_maps, list(range(NCORES)), trace=trace)
    BS, M = cfg["BS"], cfg["M"]
    out = np.empty((NCORES * BS, M), np.float32)
    dq = np.float32(1.0 / cfg["OUT_SCALE"]) if cfg["OUT_DT"] == "i8" else np.float32(1.0)
    mb = np.asarray(inputs["movie_bias"], np.float32)[None, :]
    for c in range(NCORES):
        out[c * BS : (c + 1) * BS] = (
            np.asarray(br.results[c]["out"], dtype=np.float32) * dq + mb
        )
    return out, br


def kernel(**inputs) -> np.ndarray:
    try:
        out, _ = run_on_hw(FULL_CFG, inputs, trace=False)
    except Exception:
        # one retry for transient device/runtime hiccups
        out, _ = run_on_hw(FULL_CFG, inputs, trace=False)
    return out


# revision 34
# speedup vs baseline: 1.1225x; 1.0013x over previous
"""Trainium2 Bass kernel for nn_DLFG_79817672229311 (segment_reduce).

Computes, data-parallel over the batch axis on 8 NeuronCores:
  history = [extInfo, ratings, 1]                    # [BS, 20033] per core
  x1 = lrelu(history @ [w1;b1].T); BN folded into w2 on host
  x2..x5 = lrelu(x @ wl.T + bl)
  gen = tanh(x5 @ w6.T + b6)                         # [BS, 65]
  s, cnt = per-row sum / count of nonzero ratings
  addv = s/cnt + gen[:, 64]
  out = gen[:, :64] @ movie_factors.T + addv[:, None] + movie_bias

Design (per core; layer 1 is at the fp8 DoubleRow compute wall ~135us, so
everything else hides under or packs tightly around it):
- Activations ride transposed ([feature, batch]): batch (512) is the matmul
  free dim, features the partition dim, so no on-device transposes are needed.
- History is staged to SBUF once in fp8 (ratings 0..5 are exact in e4m3) with
  a host-appended ones-row; layer 1 runs fp8 DoubleRow against 2^15-pre-scaled
  fp8 [w1;b1] slabs streamed from HBM, so the bias rides the contraction and
  the epilogue is a bias-free paired Lrelu.  K is host-padded to an even tile
  count so every step is a DoubleRow pair.
- BOTH cnt and s accumulate on the Vector engine under the layer-1 shadow as
  paired fused chains (cnt: min+add, s: plain add) into fp16 accumulators
  (integer partial sums <= 790 are exact in fp16; fp16 halves DVE traffic).
  This removes the former PE ones-matmul s-chain (~16us of Tensor time).
  Partition reduction is 4 tiny fp16 matmuls; the ones-row contribution is
  subtracted in the combine.
- Mid layers interleave their k-accumulation across 3 PSUM banks; some lrelu
  epilogues run on the DVE.
- addv bounces through DRAM into per-partition [128, NBT] and enters the
  reconstruction staging op as its per-partition bias.
- Reconstruction exploits PE row tiling (64x128 mode): the contraction is
  only the 64 factors (movie_bias is added on HOST during dequant), so the
  factor block is duplicated into SBUF partitions 64..127 (both in mft and in
  genext2) and each chunk-pair issues two CONCURRENT matmuls -- tile T0
  (SBUF rows 0-63) on the even chunk, tile T8 (rows 64-127) on the odd chunk,
  landing in adjacent PSUM banks.  This halves reconstruction Tensor time.
- Recon staging: fused scale+bias on alternating Vector/Scalar engines ->
  int8 DRAM in 2048-col blocks, with the output drains alternating between
  the Sync and GpSimd DMA queues so descriptor issue isn't single-queue
  limited.  Host dequantizes by the fixed scale 4/127 and adds movie_bias.
"""

import math
import sys

sys.path.insert(0, "/opt/trn_rl_repo")

import numpy as np
import ml_dtypes

BF16 = ml_dtypes.bfloat16
FP8 = ml_dtypes.float8_e4m3

NCORES = 8
BN_EPS = 0.05
SLOPE = 0.01

FULL_CFG = dict(
    BS=512,  # per-core batch
    UINFO=32,
    M=20000,
    F=64,
    DIMS=(1024, 512, 256, 512, 1024, 65),  # fan-outs of the 6 linear layers
    HTC=4,  # history K-tiles per DMA chunk (must be even for DoubleRow pairs)
    W1_SCALE=2.0**15,  # fp8 pre-scale: w1 ~ U(+-0.007) sits in e4m3 subnormals
    MLP_SCALES=(4096.0, 4096.0, 2048.0, 4096.0),  # 2^k per layer, |w|*s < 240
    OUT_DT="i8",  # "i8" (host dequant) or "bf16"
    OUT_SCALE=127.0 / 4.0,  # int8 quantization scale (|out| <= ~3.2)
    SC_DT="bf16",  # s/cnt DVE accumulator dtype ("bf16", "f16" or "f32")
    S_DVE_PAIRS=54,  # leading k-pair-steps whose s-accum rides the DVE;
    # the rest run as a PE ones-matmul chain after the mid layers (the DVE
    # can't hold both full chains under the layer-1 shadow, and GpSimd
    # chains poison SBUF bandwidth for everyone)
)


def _derived(cfg):
    d = dict(cfg)
    d["KH"] = cfg["UINFO"] + cfg["M"] + 1  # +1 ones-row carrying b1
    t1 = math.ceil(d["KH"] / 128)
    d["T1"] = t1 + (t1 % 2)  # pad to even so all steps are DoubleRow pairs
    d["NBT"] = cfg["BS"] // 128  # batch tiles per core
    d["CHUNKS"] = [(o, min(512, cfg["M"] - o)) for o in range(0, cfg["M"], 512)]
    return d


def build_nc(cfg):
    """Build + compile the (single-core SPMD) Bass program."""
    import concourse.bass as bass
    import concourse.tile as tile
    from concourse import bacc, mybir

    d = _derived(cfg)
    BS, UINFO, M, F = cfg["BS"], cfg["UINFO"], cfg["M"], cfg["F"]
    DIMS = cfg["DIMS"]
    T1, NBT, CHUNKS, HTC = d["T1"], d["NBT"], d["CHUNKS"], cfg["HTC"]
    FO1 = DIMS[0]
    FO1T = FO1 // 128
    w1_unscale = 1.0 / cfg["W1_SCALE"]
    f32 = mybir.dt.float32
    bf16 = mybir.dt.bfloat16
    f16 = mybir.dt.float16
    f8 = mybir.dt.float8e4
    i8 = mybir.dt.int8
    AF = mybir.ActivationFunctionType
    ALU = mybir.AluOpType

    OUT_I8 = cfg["OUT_DT"] == "i8"
    odt = i8 if OUT_I8 else bf16
    OSC = cfg["OUT_SCALE"] if OUT_I8 else 1.0
    scdt = {"bf16": bf16, "f16": f16, "f32": f32}[cfg["SC_DT"]]

    nc = bacc.Bacc("TRN2", target_bir_lowering=False, debug=False)

    # ---- DRAM I/O ----
    ht_d = nc.dram_tensor("ht", [128, T1, BS], f8, kind="ExternalInput")
    w1t_d = nc.dram_tensor("w1t", [128, T1, FO1], f8, kind="ExternalInput")
    w_d = {}
    for li in range(2, 7):
        fi, fo = DIMS[li - 2], DIMS[li - 1]
        wdt = f8 if li < 6 else bf16
        w_d[li] = nc.dram_tensor(f"w{li}t", [128, fi // 128, fo], wdt, kind="ExternalInput")
    bp_d = {}
    for li in range(2, 6):
        fot = math.ceil(DIMS[li - 1] / 128)
        bp_d[li] = nc.dram_tensor(f"b{li}p", [128, fot], f32, kind="ExternalInput")
    b6_d = nc.dram_tensor("b6p", [128, 1], f32, kind="ExternalInput")
    mft_d = nc.dram_tensor("mft", [128, M], bf16, kind="ExternalInput")
    out_d = nc.dram_tensor("out", [BS, M], odt, kind="ExternalOutput")
    av_d = nc.dram_tensor("av_scr", [BS], f32)  # addv row->partition bounce

    with tile.TileContext(nc) as tc, bass.ExitStack() as ctx:
        const = ctx.enter_context(tc.tile_pool(name="const", bufs=1))
        htp = ctx.enter_context(tc.tile_pool(name="htp", bufs=1))
        w1p = ctx.enter_context(tc.tile_pool(name="w1p", bufs=12))
        actp = ctx.enter_context(tc.tile_pool(name="actp", bufs=1))
        ost = ctx.enter_context(tc.tile_pool(name="ost", bufs=6))
        psp = ctx.enter_context(tc.tile_pool(name="psp", bufs=4, space="PSUM"))

        segs = []
        t0 = 0
        for tn in [2, 2]:
            segs.append((t0, tn))
            t0 += tn
        while t0 < T1:
            tn = min(HTC, T1 - t0)
            segs.append((t0, tn))
            t0 += tn
        NSEG = len(segs)

        # ---- history segments 0/1 prefetched on the Scalar DMA queue so
        # they transfer in parallel with the first w1 slabs on Sync ----
        ht_pre = {}
        for si_ in (0, 1):
            htt = htp.tile([128, 2, BS], f8, name="ht", tag="ht", bufs=NSEG)
            nc.scalar.dma_start(out=htt[:], in_=ht_d[:, 2 * si_ : 2 * si_ + 2, :])
            ht_pre[si_] = htt

        # ---- constants in SBUF (dispatched on the Scalar DMA queue) ----
        bp_sb = {}
        for li in range(2, 6):
            fot = math.ceil(DIMS[li - 1] / 128)
            bp_sb[li] = const.tile([128, fot], f32, name=f"b{li}p", tag=f"b{li}p")
            nc.scalar.dma_start(out=bp_sb[li][:], in_=bp_d[li][:])
        b6_sb = const.tile([128, 1], f32, name="b6p", tag="b6p")
        nc.scalar.dma_start(out=b6_sb[:], in_=b6_d[:])
        onesf = const.tile([128, 1], scdt, name="onesf", tag="onesf")
        nc.vector.memset(onesf[:], 1.0)
        # fp8 ones for the PE s-tail chain ([128,2,16]: DoubleRow weight
        # pair-step must be a multiple of 16 per the ISA)
        ones16 = const.tile([128, 2, 16], f8, name="ones16", tag="ones16")
        nc.vector.memset(ones16[:], 1.0)
        # per-partition mask for history tile 0 (extInfo rows excluded)
        rmask = const.tile([128, 1], f32, name="rmask", tag="rmask")
        nc.vector.memset(rmask[:], 1.0)
        nc.vector.memset(rmask[0:UINFO, :], 0.0)

        # ---- layer 1: one pass over history segments ----
        x1t = actp.tile([128, FO1T, BS], f8, name="x1t", tag="x1t")
        c_acc = const.tile([128, 2, BS], scdt, name="c_acc", tag="c_acc")
        s_acc = const.tile([128, 2, BS], scdt, name="s_acc", tag="s_acc")
        # 4 paired PSUM tiles (2 banks each) -> bias-free paired epilogues
        ps1 = [psp.tile([128, 2, BS], f32, name="ps1", tag="ps") for _ in range(FO1T // 2)]

        nstep = T1 // 2
        SD = cfg["S_DVE_PAIRS"]
        s_tail = []  # (htt, lo) pair-steps whose s runs on the PE ones-chain
        step_i = 0
        sc_first = True
        for si_, (ts_, tn) in enumerate(segs):
            if si_ in ht_pre:
                htt = ht_pre[si_]
            else:
                htt = htp.tile([128, tn, BS], f8, name="ht", tag="ht", bufs=NSEG)
                nc.sync.dma_start(out=htt[:], in_=ht_d[:, ts_ : ts_ + tn, :])

            lo = 0
            while lo < tn:
                t = ts_ + lo
                w1s = w1p.tile([128, 2, FO1], f8, name="w1s", tag="w1s")
                if step_i == 0:
                    h = FO1 // 2
                    nc.sync.dma_start(out=w1s[:, 0:2, 0:h], in_=w1t_d[:, t : t + 2, 0:h])
                    nc.sync.dma_start(out=w1s[:, 0:2, h:FO1], in_=w1t_d[:, t : t + 2, h:FO1])
                else:
                    nc.sync.dma_start(out=w1s[:, 0:2, :], in_=w1t_d[:, t : t + 2, :])
                for fo in range(FO1T):
                    fsl = slice(fo * 128, (fo + 1) * 128)
                    pdst = ps1[fo // 2][:, fo % 2, :]
                    nc.tensor.matmul(
                        pdst,
                        lhsT=w1s[:, 0:2, fsl],
                        rhs=htt[:, lo : lo + 2, :],
                        start=(step_i == 0),
                        stop=(step_i == nstep - 1),
                        perf_mode=mybir.MatmulPerfMode.DoubleRow,
                    )
                step_i += 1
                lo += 2

            # cnt chain (fused min+add) and the leading part of the s chain
            # (plain add) on the DVE under the layer-1 shadow, into 16-bit
            # accumulators (integer partials <= ~790, exact enough).  The
            # DVE can't hold BOTH full chains before the mid layers end, so
            # s pair-steps >= SD are deferred to a short PE ones-chain.
            # Segment 0 holds extInfo rows; init via masked ops.
            if sc_first:
                assert tn == 2
                nc.vector.tensor_scalar(
                    c_acc[:, 0, :], htt[:, 0, :], 1.0, rmask[:], op0=ALU.min, op1=ALU.mult
                )
                nc.vector.tensor_scalar(
                    c_acc[:, 1, :], htt[:, 1, :], 1.0, None, op0=ALU.min
                )
                nc.vector.tensor_scalar(
                    s_acc[:, 0, :], htt[:, 0, :], 1.0, rmask[:], op0=ALU.mult, op1=ALU.mult
                )
                nc.vector.tensor_copy(s_acc[:, 1, :], htt[:, 1, :])
                sc_first = False
            else:
                o = 0
                while o < tn:
                    nc.vector.scalar_tensor_tensor(
                        c_acc[:], htt[:, o : o + 2, :], 1.0, c_acc[:],
                        op0=ALU.min, op1=ALU.add,
                    )
                    if (ts_ + o) // 2 < SD:
                        nc.vector.tensor_add(s_acc[:], htt[:, o : o + 2, :], s_acc[:])
                    else:
                        s_tail.append((htt, o))
                    o += 2

        # layer-1 epilogue (bias-free: bias rode the matmul via the ones-row)
        for j in range(FO1T // 2):
            nc.scalar.activation(
                x1t[:, 2 * j : 2 * j + 2, :], ps1[j][:], AF.Lrelu,
                scale=w1_unscale, alpha=SLOPE,
            )

        # ---- remaining weights + movie factors: emitted late on the Sync
        # queue so the layer-1 ht/w1 stream gets the DMA bandwidth first.
        w_sb = {}
        for li in range(2, 7):
            fi, fo = DIMS[li - 2], DIMS[li - 1]
            wdt = f8 if li < 6 else bf16
            w_sb[li] = const.tile([128, fi // 128, fo], wdt, name=f"w{li}t", tag=f"w{li}t")
            nc.sync.dma_start(out=w_sb[li][:], in_=w_d[li][:])
        mft = const.tile([128, M], bf16, name="mft", tag="mft")
        nc.sync.dma_start(out=mft[:], in_=mft_d[:])

        # ---- layers 2..5 (lrelu) ----
        xin = x1t
        for li in range(2, 6):
            fi, fo = DIMS[li - 2], DIMS[li - 1]
            fit, fot = fi // 128, fo // 128
            xdt = f8 if li < 5 else bf16
            unsc = 1.0 / cfg["MLP_SCALES"][li - 2]
            xout = actp.tile(
                [128, fot, BS], xdt, name=f"x{li}t",
                tag=("x1t" if li == 5 else "x2t" if li == 4 else f"x{li}t"),
            )
            # interleave the k-accumulation across up to 4 fo-tile banks so
            # consecutive matmuls don't serialize on one bank's drain
            for g0 in range(0, fot, 4):
                gn = min(4, fot - g0)
                pss = [psp.tile([128, BS], f32, name="ps", tag="ps") for _ in range(gn)]
                ki = 0
                while ki < fit:
                    n2 = 2 if ki + 2 <= fit else 1
                    for j in range(gn):
                        ft = g0 + j
                        if n2 == 2:
                            nc.tensor.matmul(
                                pss[j][:],
                                lhsT=w_sb[li][:, ki : ki + 2, ft * 128 : (ft + 1) * 128],
                                rhs=xin[:, ki : ki + 2, :],
                                start=(ki == 0),
                                stop=(ki + 2 == fit),
                                perf_mode=mybir.MatmulPerfMode.DoubleRow,
                            )
                        else:
                            nc.tensor.matmul(
                                pss[j][:],
                                lhsT=w_sb[li][:, ki, ft * 128 : (ft + 1) * 128],
                                rhs=xin[:, ki, :],
                                start=(ki == 0),
                                stop=True,
                            )
                    ki += n2
                # epilogues all on the Scalar engine: the DVE queue carries
                # the s/cnt chains during layer 1 and would stall these
                # (strict FIFO), which in turn stalls the next layer's PE.
                for j in range(gn):
                    ft = g0 + j
                    nc.scalar.activation(
                        xout[:, ft, :], pss[j][:], AF.Lrelu,
                        bias=bp_sb[li][:, ft : ft + 1], scale=unsc, alpha=SLOPE,
                    )
            xin = xout

        # ---- gen_last early: a 1-column slice of layer 6 + tanh, emitted
        # right before the s-tail so addv doesn't wait for the full layer-6
        # matmul + tanh.  8 tiny matmuls (~60cyc each) + one [1,BS] ACT.
        fi6, fo6 = DIMS[4], DIMS[5]
        fit6 = fi6 // 128
        assert fo6 == F + 1
        ps6b = psp.tile([1, BS], f32, name="ps6b", tag="ps")
        for ki in range(fit6):
            nc.tensor.matmul(
                ps6b[:],
                lhsT=w_sb[6][:, ki, F : F + 1],
                rhs=xin[:, ki, :],
                start=(ki == 0),
                stop=(ki == fit6 - 1),
            )
        genl = const.tile([1, BS], f32, name="genl", tag="genl")
        nc.scalar.activation(genl[:], ps6b[:], AF.Tanh, bias=b6_sb[F : F + 1, 0:1], scale=1.0)

        # ---- cnt partition reduce + 1/cnt, emitted right after the mids:
        # c_acc is complete when the DVE chains drain (~layer-1 end), so the
        # slow DVE reciprocal runs far off the critical path, in the DVE's
        # idle window during the mids/s-tail.
        c_red = psp.tile([1, 2, BS], f32, name="c_red", tag="ps")
        nc.tensor.matmul(c_red[:, 0, :], lhsT=onesf[:], rhs=c_acc[:, 0, :], start=True, stop=True)
        nc.tensor.matmul(c_red[:, 1, :], lhsT=onesf[:], rhs=c_acc[:, 1, :], start=True, stop=True)
        # the host-appended ones-row counted +1 per batch (cnt chain half 0,
        # s PE-tail): subtract 1 from each.
        c0_sb = const.tile([1, BS], f32, name="c0_sb", tag="c0_sb")
        nc.vector.tensor_scalar_sub(c0_sb[:], c_red[0:1, 0, :], 1.0)
        c_sb = const.tile([1, BS], f32, name="c_sb", tag="c_sb")
        nc.vector.tensor_add(c_sb[:], c0_sb[:], c_red[0:1, 1, :])
        rc_sb = const.tile([1, BS], f32, name="rc_sb", tag="rc_sb")
        nc.vector.reciprocal(rc_sb[:], c_sb[:])

        # ---- PE s-tail: DoubleRow ones-chain over the deferred pair-steps
        # (history is long resident), double-buffered across one PSUM
        # slot's two banks, then the two DVE-half partition reduces.
        NT = len(s_tail)
        assert NT >= 2
        scx = psp.tile([16, 2, BS], f32, name="scx", tag="ps")
        for si, (htt, lo) in enumerate(s_tail):
            nc.tensor.matmul(
                scx[0:16, si % 2, :], lhsT=ones16[:], rhs=htt[:, lo : lo + 2, :],
                start=(si < 2), stop=(si >= NT - 2),
                perf_mode=mybir.MatmulPerfMode.DoubleRow,
            )
        s_red = psp.tile([1, 2, BS], f32, name="s_red", tag="ps")
        nc.tensor.matmul(s_red[:, 0, :], lhsT=onesf[:], rhs=s_acc[:, 0, :], start=True, stop=True)
        nc.tensor.matmul(s_red[:, 1, :], lhsT=onesf[:], rhs=s_acc[:, 1, :], start=True, stop=True)

        # ---- layer 6 (tanh) -> genf [65, BS] f32 ----
        fi, fo = DIMS[4], DIMS[5]
        fit = fi // 128
        assert fo == F + 1
        ps6 = psp.tile([fo, BS], f32, name="ps6", tag="ps")
        for ki in range(fit):
            nc.tensor.matmul(
                ps6[:],
                lhsT=w_sb[6][:, ki, 0:fo],
                rhs=xin[:, ki, :],
                start=(ki == 0),
                stop=(ki == fit - 1),
            )
        genf = actp.tile([fo, BS], f32, name="genf", tag="genf")
        nc.scalar.activation(genf[:], ps6[:], AF.Tanh, bias=b6_sb[0:fo, 0:1], scale=1.0)

        # ---- genext2: factor rows in bf16, duplicated into partitions
        # 64..127 (via SBUF->SBUF DMA) so reconstruction can row-tile the
        # PE.  Emitted BEFORE the meanV combine chain: the DVE queue is
        # strict FIFO and the cast gates the first reconstruction matmul.
        genext2 = actp.tile([128, BS], bf16, name="genext2", tag="genext")
        nc.vector.tensor_copy(genext2[0:F, :], genf[0:F, :])
        nc.sync.dma_start(out=genext2[F : 2 * F, :], in_=genext2[0:F, :])

        # s pre-scaled by OSC (ones-row subtracted) -> meanV -> addv
        s_sb = const.tile([1, BS], f32, name="s_sb", tag="s_sb")
        nc.vector.tensor_scalar(
            s_sb[:], s_red[0:1, 0, :], -1.0, float(OSC), op0=ALU.add, op1=ALU.mult
        )
        nc.vector.scalar_tensor_tensor(
            s_sb[:], s_red[0:1, 1, :], float(OSC), s_sb[:], op0=ALU.mult, op1=ALU.add
        )
        nc.vector.scalar_tensor_tensor(
            s_sb[:], scx[0:1, 0, :], float(OSC), s_sb[:], op0=ALU.mult, op1=ALU.add
        )
        nc.vector.scalar_tensor_tensor(
            s_sb[:], scx[0:1, 1, :], float(OSC), s_sb[:], op0=ALU.mult, op1=ALU.add
        )
        mv_sb = const.tile([1, BS], f32, name="mv_sb", tag="mv_sb")
        nc.vector.tensor_mul(mv_sb[:], rc_sb[:], s_sb[:])
        av_sb = const.tile([1, BS], f32, name="av_sb", tag="av_sb")
        nc.vector.scalar_tensor_tensor(
            av_sb[:], genl[:], float(OSC), mv_sb[:], op0=ALU.mult, op1=ALU.add
        )
        # bounce through DRAM into per-partition layout [128, NBT] for the
        # staging ops (row b -> partition b%128, column b//128)
        nc.sync.dma_start(out=av_d[:], in_=av_sb[0:1, :])
        addv_t = const.tile([128, NBT], f32, name="addv_t", tag="addv_t")
        nc.sync.dma_start(out=addv_t[:], in_=av_d.ap().rearrange("(t p) -> p t", p=128))

        # ---- reconstruction: out[bt*128+p, m] over movie chunk-pairs.
        # PE in 64x128 row-tiled mode: tile T0 (SBUF partitions 0-63) runs
        # the even chunk, tile T8 (64-127, the duplicated factor rows) the
        # odd chunk CONCURRENTLY, into adjacent PSUM banks.
        PAIRS = [CHUNKS[i : i + 2] for i in range(0, len(CHUNKS), 2)]
        for bt in range(NBT):
            lhsT_lo = genext2[0:F, bt * 128 : (bt + 1) * 128]
            lhsT_hi = genext2[F : 2 * F, bt * 128 : (bt + 1) * 128]
            st = None
            for pi, pair in enumerate(PAIRS):
                # staging alternates Scalar/Vector, time-balanced ~5:4
                # (GpSimd cannot read PSUM, so no third stager exists)
                eng = 0 if (pi % 9) in (0, 2, 4, 6, 8) else 1
                pr = psp.tile([128, 2, 512], f32, name="pr", tag="ps")
                for j, (co, cw) in enumerate(pair):
                    nc.tensor.matmul(
                        pr[:, j, 0:cw],
                        lhsT=(lhsT_lo if j == 0 else lhsT_hi),
                        rhs=(mft[0:F, co : co + cw] if j == 0 else mft[F : 2 * F, co : co + cw]),
                        start=True, stop=True,
                    )
                pw = sum(cw for _, cw in pair)
                if pi % 2 == 0:
                    st = ost.tile([128, 2048], odt, name="st", tag="st")
                    so, po = 0, pair[0][0]
                # stage only the valid width (last pair is 544, not 1024)
                nst = 1024 if pw == 1024 else 512 + pair[1][1]
                pr2d = pr[:].opt()[:, 0:nst]  # contiguous [128, nst]
                if eng == 0:
                    nc.scalar.activation(
                        st[:, so : so + nst], pr2d, AF.Identity,
                        bias=addv_t[:, bt : bt + 1], scale=OSC,
                    )
                else:
                    nc.vector.tensor_scalar(
                        st[:, so : so + nst], pr2d, OSC, addv_t[:, bt : bt + 1],
                        op0=ALU.mult, op1=ALU.add,
                    )
                so += nst
                if pi % 2 == 1 or pi == len(PAIRS) - 1:
                    # output drains alternate between the Sync and GpSimd
                    # DMA queues (both engines idle during reconstruction;
                    # descriptor issue costs ~0.6us each)
                    q = nc.sync if (pi // 2) % 2 == 0 else nc.gpsimd
                    q.dma_start(
                        out=out_d[bt * 128 : (bt + 1) * 128, po : po + so],
                        in_=st[:, 0:so],
                    )

    nc.compile()
    return nc


def prep_in_maps(cfg, inputs):
    """Shard + lay out the full inputs into per-core DRAM input maps."""
    d = _derived(cfg)
    BS, UINFO, M, F, DIMS, T1 = cfg["BS"], cfg["UINFO"], cfg["M"], cfg["F"], cfg["DIMS"], d["T1"]
    extInfo = np.asarray(inputs["extInfo"], np.float32)
    ratings = np.asarray(inputs["ratings"], np.float32)

    # BN (eval) fold into layer 2: y = g'(lrelu1) + b' with g' = bn_g/sqrt(1+eps)
    g = np.asarray(inputs["bn_g"], np.float32) / np.float32(np.sqrt(1.0 + BN_EPS))
    bnb = np.asarray(inputs["bn_b"], np.float32)
    w2 = np.asarray(inputs["w2"], np.float32)
    w2f = w2 * g[None, :]
    b2f = np.asarray(inputs["b2"], np.float32) + w2 @ bnb

    shared = {}
    # w1t: [KH,FO1] -> padded [T1*128, FO1] -> [128, T1, FO1]; the row at
    # index UINFO+M carries b1 (matching the ones-row in the history).
    w1 = np.asarray(inputs["w1"], np.float32)
    b1 = np.asarray(inputs["b1"], np.float32)
    FO1 = DIMS[0]
    w1tp = np.zeros((T1 * 128, FO1), FP8)
    w1tp[0 : w1.shape[1]] = (w1.T * np.float32(cfg["W1_SCALE"])).astype(FP8)
    w1tp[UINFO + M] = (b1 * np.float32(cfg["W1_SCALE"])).astype(FP8)
    shared["w1t"] = np.ascontiguousarray(w1tp.reshape(T1, 128, FO1).transpose(1, 0, 2))

    def pack_w(wT, fo, dt=BF16, scale=1.0):
        fi = wT.shape[0]
        w = (wT.astype(np.float32) * np.float32(scale)).astype(dt)
        return np.ascontiguousarray(w.reshape(fi // 128, 128, fo).transpose(1, 0, 2))

    scs = cfg["MLP_SCALES"]
    shared["w2t"] = pack_w(w2f.T, DIMS[1], FP8, scs[0])
    for li, wname in ((3, "w3"), (4, "w4"), (5, "w5"), (6, "w6")):
        w = np.asarray(inputs[wname], np.float32)
        fo = DIMS[li - 1]
        if li < 6:
            shared[f"w{li}t"] = pack_w(w.T, fo, FP8, scs[li - 2])
        else:
            shared[f"w{li}t"] = pack_w(w.T, fo)

    def pack_b(b, fo):
        fot = math.ceil(fo / 128)
        bp = np.zeros(fot * 128, np.float32)
        bp[:fo] = b
        return np.ascontiguousarray(bp.reshape(fot, 128).T)

    bsrc = {2: b2f}
    for li in (3, 4, 5):
        bsrc[li] = np.asarray(inputs[f"b{li}"], np.float32)
    for li in range(2, 6):
        shared[f"b{li}p"] = pack_b(bsrc[li], DIMS[li - 1])
    shared["b6p"] = pack_b(np.asarray(inputs["b6"], np.float32), DIMS[5])

    # bf16 mft: factor rows duplicated into partitions 64..127 for the
    # row-tiled reconstruction (movie_bias is added on host at dequant).
    mft = np.zeros((128, M), BF16)
    mft[0:F] = np.asarray(inputs["movie_factors"], np.float32).T.astype(BF16)
    mft[F : 2 * F] = mft[0:F]
    shared["mft"] = mft

    in_maps = []
    for c in range(NCORES):
        sl = slice(c * BS, (c + 1) * BS)
        htc = np.zeros((T1 * 128, BS), FP8)
        htc[0:UINFO] = extInfo[sl].T.astype(FP8)
        htc[UINFO : UINFO + M] = ratings[sl].T.astype(FP8)
        htc[UINFO + M] = np.float32(1.0)  # ones-row: picks up b1 from w1t
        m = dict(shared)
        m["ht"] = np.ascontiguousarray(htc.reshape(T1, 128, BS).transpose(1, 0, 2))
        in_maps.append(m)
    return in_maps


_NC_CACHE = {}


def run_on_hw(cfg, inputs, trace=False):
    from concourse.bass_utils import run_bass_kernel_spmd

    key = tuple(sorted((k, v) for k, v in cfg.items() if k != "DIMS")) + (cfg["DIMS"],)
    if key not in _NC_CACHE:
        _NC_CACHE[key] = build_nc(cfg)
    nc = _NC_CACHE[key]
    in_maps = prep_in_maps(cfg, inputs)
    br = run_bass_kernel_spmd(nc, in_maps, list(range(NCORES)), trace=trace)
    BS, M = cfg["BS"], cfg["M"]
    out = np.empty((NCORES * BS, M), np.float32)
    dq = np.float32(1.0 / cfg["OUT_SCALE"]) if cfg["OUT_DT"] == "i8" else np.float32(1.0)
    mb = np.asarray(inputs["movie_bias"], np.float32)[None, :]
    for c in range(NCORES):
        out[c * BS : (c + 1) * BS] = (
            np.asarray(br.results[c]["out"], dtype=np.float32) * dq + mb
        )
    return out, br


def kernel(**inputs) -> np.ndarray:
    try:
        out, _ = run_on_hw(FULL_CFG, inputs, trace=False)
    except Exception:
        # one retry for transient device/runtime hiccups
        out, _ = run_on_hw(FULL_CFG, inputs, trace=False)
    return out


# revision 36
# speedup vs baseline: 1.1306x; 1.0073x over previous
"""Trainium2 Bass kernel for nn_DLFG_79817672229311 (segment_reduce).

Computes, data-parallel over the batch axis on 8 NeuronCores:
  history = [extInfo, ratings, 1]                    # [BS, 20033] per core
  x1 = lrelu(history @ [w1;b1].T); BN folded into w2 on host
  x2..x5 = lrelu(x @ wl.T + bl)
  gen = tanh(x5 @ w6.T + b6)                         # [BS, 65]
  s, cnt = per-row sum / count of nonzero ratings
  addv = s/cnt + gen[:, 64]
  out = gen[:, :64] @ movie_factors.T + addv[:, None] + movie_bias

Design (per core; layer 1 is at the fp8 DoubleRow compute wall ~135us, so
everything else hides under or packs tightly around it):
- Activations ride transposed ([feature, batch]): batch (512) is the matmul
  free dim, features the partition dim, so no on-device transposes are needed.
- History is staged to SBUF once in fp8 (ratings 0..5 are exact in e4m3) with
  a host-appended ones-row; layer 1 runs fp8 DoubleRow against 2^15-pre-scaled
  fp8 [w1;b1] slabs streamed from HBM, so the bias rides the contraction and
  the epilogue is a bias-free paired Lrelu.  K is host-padded to an even tile
  count so every step is a DoubleRow pair.
- BOTH cnt and s accumulate on the Vector engine under the layer-1 shadow as
  paired fused chains (cnt: min+add, s: plain add) into fp16 accumulators
  (integer partial sums <= 790 are exact in fp16; fp16 halves DVE traffic).
  This removes the former PE ones-matmul s-chain (~16us of Tensor time).
  Partition reduction is 4 tiny fp16 matmuls; the ones-row contribution is
  subtracted in the combine.
- Mid layers interleave their k-accumulation across 3 PSUM banks; some lrelu
  epilogues run on the DVE.
- addv bounces through DRAM into per-partition [128, NBT] and enters the
  reconstruction staging op as its per-partition bias.
- Reconstruction exploits PE row tiling (64x128 mode): the contraction is
  only the 64 factors (movie_bias is added on HOST during dequant), so the
  factor block is duplicated into SBUF partitions 64..127 (both in mft and in
  genext2) and each chunk-pair issues two CONCURRENT matmuls -- tile T0
  (SBUF rows 0-63) on the even chunk, tile T8 (rows 64-127) on the odd chunk,
  landing in adjacent PSUM banks.  This halves reconstruction Tensor time.
- Recon staging: fused scale+bias on alternating Vector/Scalar engines ->
  int8 DRAM in 2048-col blocks, with the output drains alternating between
  the Sync and GpSimd DMA queues so descriptor issue isn't single-queue
  limited.  Host dequantizes by the fixed scale 4/127 and adds movie_bias.
"""

import math
import sys

sys.path.insert(0, "/opt/trn_rl_repo")

import numpy as np
import ml_dtypes

BF16 = ml_dtypes.bfloat16
FP8 = ml_dtypes.float8_e4m3

NCORES = 8
BN_EPS = 0.05
SLOPE = 0.01

FULL_CFG = dict(
    BS=512,  # per-core batch
    UINFO=32,
    M=20000,
    F=64,
    DIMS=(1024, 512, 256, 512, 1024, 65),  # fan-outs of the 6 linear layers
    HTC=4,  # history K-tiles per DMA chunk (must be even for DoubleRow pairs)
    W1_SCALE=2.0**15,  # fp8 pre-scale: w1 ~ U(+-0.007) sits in e4m3 subnormals
    MLP_SCALES=(4096.0, 4096.0, 2048.0, 4096.0),  # 2^k per layer, |w|*s < 240
    OUT_DT="i8",  # "i8" (host dequant) or "bf16"
    OUT_SCALE=127.0 / 4.0,  # int8 quantization scale (|out| <= ~3.2)
    SC_DT="bf16",  # s/cnt DVE accumulator dtype ("bf16", "f16" or "f32")
    S_DVE_PAIRS=54,  # leading k-pair-steps whose s-accum rides the DVE;
    # the rest run as a PE ones-matmul chain after the mid layers (the DVE
    # can't hold both full chains under the layer-1 shadow, and GpSimd
    # chains poison SBUF bandwidth for everyone)
)


def _derived(cfg):
    d = dict(cfg)
    d["KH"] = cfg["UINFO"] + cfg["M"] + 1  # +1 ones-row carrying b1
    t1 = math.ceil(d["KH"] / 128)
    d["T1"] = t1 + (t1 % 2)  # pad to even so all steps are DoubleRow pairs
    d["NBT"] = cfg["BS"] // 128  # batch tiles per core
    d["CHUNKS"] = [(o, min(512, cfg["M"] - o)) for o in range(0, cfg["M"], 512)]
    return d


def build_nc(cfg):
    """Build + compile the (single-core SPMD) Bass program."""
    import concourse.bass as bass
    import concourse.tile as tile
    from concourse import bacc, mybir

    d = _derived(cfg)
    BS, UINFO, M, F = cfg["BS"], cfg["UINFO"], cfg["M"], cfg["F"]
    DIMS = cfg["DIMS"]
    T1, NBT, CHUNKS, HTC = d["T1"], d["NBT"], d["CHUNKS"], cfg["HTC"]
    FO1 = DIMS[0]
    FO1T = FO1 // 128
    w1_unscale = 1.0 / cfg["W1_SCALE"]
    f32 = mybir.dt.float32
    bf16 = mybir.dt.bfloat16
    f16 = mybir.dt.float16
    f8 = mybir.dt.float8e4
    i8 = mybir.dt.int8
    AF = mybir.ActivationFunctionType
    ALU = mybir.AluOpType

    OUT_I8 = cfg["OUT_DT"] == "i8"
    odt = i8 if OUT_I8 else bf16
    OSC = cfg["OUT_SCALE"] if OUT_I8 else 1.0
    scdt = {"bf16": bf16, "f16": f16, "f32": f32}[cfg["SC_DT"]]

    nc = bacc.Bacc("TRN2", target_bir_lowering=False, debug=False)

    # ---- DRAM I/O ----
    ht_d = nc.dram_tensor("ht", [128, T1, BS], f8, kind="ExternalInput")
    w1t_d = nc.dram_tensor("w1t", [128, T1, FO1], f8, kind="ExternalInput")
    w_d = {}
    for li in range(2, 7):
        fi, fo = DIMS[li - 2], DIMS[li - 1]
        wdt = f8 if li < 6 else bf16
        w_d[li] = nc.dram_tensor(f"w{li}t", [128, fi // 128, fo], wdt, kind="ExternalInput")
    bp_d = {}
    for li in range(2, 6):
        fot = math.ceil(DIMS[li - 1] / 128)
        bp_d[li] = nc.dram_tensor(f"b{li}p", [128, fot], f32, kind="ExternalInput")
    b6_d = nc.dram_tensor("b6p", [128, 1], f32, kind="ExternalInput")
    mft_d = nc.dram_tensor("mft", [128, M], bf16, kind="ExternalInput")
    out_d = nc.dram_tensor("out", [BS, M], odt, kind="ExternalOutput")
    av_d = nc.dram_tensor("av_scr", [BS], f32)  # addv row->partition bounce

    with tile.TileContext(nc) as tc, bass.ExitStack() as ctx:
        const = ctx.enter_context(tc.tile_pool(name="const", bufs=1))
        htp = ctx.enter_context(tc.tile_pool(name="htp", bufs=1))
        w1p = ctx.enter_context(tc.tile_pool(name="w1p", bufs=12))
        actp = ctx.enter_context(tc.tile_pool(name="actp", bufs=1))
        ost = ctx.enter_context(tc.tile_pool(name="ost", bufs=6))
        psp = ctx.enter_context(tc.tile_pool(name="psp", bufs=4, space="PSUM"))

        segs = []
        t0 = 0
        for tn in [2, 2]:
            segs.append((t0, tn))
            t0 += tn
        while t0 < T1:
            tn = min(HTC, T1 - t0)
            segs.append((t0, tn))
            t0 += tn
        NSEG = len(segs)

        # ---- history segments 0/1 prefetched on the Scalar DMA queue so
        # they transfer in parallel with the first w1 slabs on Sync ----
        ht_pre = {}
        for si_ in (0, 1):
            htt = htp.tile([128, 2, BS], f8, name="ht", tag="ht", bufs=NSEG)
            nc.scalar.dma_start(out=htt[:], in_=ht_d[:, 2 * si_ : 2 * si_ + 2, :])
            ht_pre[si_] = htt

        # ---- constants in SBUF (dispatched on the Scalar DMA queue) ----
        bp_sb = {}
        for li in range(2, 6):
            fot = math.ceil(DIMS[li - 1] / 128)
            bp_sb[li] = const.tile([128, fot], f32, name=f"b{li}p", tag=f"b{li}p")
            nc.scalar.dma_start(out=bp_sb[li][:], in_=bp_d[li][:])
        b6_sb = const.tile([128, 1], f32, name="b6p", tag="b6p")
        nc.scalar.dma_start(out=b6_sb[:], in_=b6_d[:])
        onesf = const.tile([128, 1], scdt, name="onesf", tag="onesf")
        nc.vector.memset(onesf[:], 1.0)
        # fp8 ones for the PE s-tail chain ([128,2,16]: DoubleRow weight
        # pair-step must be a multiple of 16 per the ISA)
        ones16 = const.tile([128, 2, 16], f8, name="ones16", tag="ones16")
        nc.vector.memset(ones16[:], 1.0)
        # per-partition mask for history tile 0 (extInfo rows excluded)
        rmask = const.tile([128, 1], f32, name="rmask", tag="rmask")
        nc.vector.memset(rmask[:], 1.0)
        nc.vector.memset(rmask[0:UINFO, :], 0.0)

        # ---- layer 1: one pass over history segments ----
        x1t = actp.tile([128, FO1T, BS], f8, name="x1t", tag="x1t")
        c_acc = const.tile([128, 2, BS], scdt, name="c_acc", tag="c_acc")
        s_acc = const.tile([128, 2, BS], scdt, name="s_acc", tag="s_acc")
        # 4 paired PSUM tiles (2 banks each) -> bias-free paired epilogues
        ps1 = [psp.tile([128, 2, BS], f32, name="ps1", tag="ps") for _ in range(FO1T // 2)]

        nstep = T1 // 2
        SD = cfg["S_DVE_PAIRS"]
        s_tail = []  # (htt, lo) pair-steps whose s runs on the PE ones-chain
        step_i = 0
        sc_first = True
        for si_, (ts_, tn) in enumerate(segs):
            if si_ in ht_pre:
                htt = ht_pre[si_]
            else:
                htt = htp.tile([128, tn, BS], f8, name="ht", tag="ht", bufs=NSEG)
                nc.sync.dma_start(out=htt[:], in_=ht_d[:, ts_ : ts_ + tn, :])

            lo = 0
            while lo < tn:
                t = ts_ + lo
                w1s = w1p.tile([128, 2, FO1], f8, name="w1s", tag="w1s")
                if step_i == 0:
                    h = FO1 // 2
                    nc.sync.dma_start(out=w1s[:, 0:2, 0:h], in_=w1t_d[:, t : t + 2, 0:h])
                    nc.sync.dma_start(out=w1s[:, 0:2, h:FO1], in_=w1t_d[:, t : t + 2, h:FO1])
                else:
                    nc.sync.dma_start(out=w1s[:, 0:2, :], in_=w1t_d[:, t : t + 2, :])
                for fo in range(FO1T):
                    fsl = slice(fo * 128, (fo + 1) * 128)
                    pdst = ps1[fo // 2][:, fo % 2, :]
                    nc.tensor.matmul(
                        pdst,
                        lhsT=w1s[:, 0:2, fsl],
                        rhs=htt[:, lo : lo + 2, :],
                        start=(step_i == 0),
                        stop=(step_i == nstep - 1),
                        perf_mode=mybir.MatmulPerfMode.DoubleRow,
                    )
                step_i += 1
                lo += 2

            # cnt chain (fused min+add) and the leading part of the s chain
            # (plain add) on the DVE under the layer-1 shadow, into 16-bit
            # accumulators (integer partials <= ~790, exact enough).  The
            # DVE can't hold BOTH full chains before the mid layers end, so
            # s pair-steps >= SD are deferred to a short PE ones-chain.
            # Segment 0 holds extInfo rows; init via masked ops.
            if sc_first:
                assert tn == 2
                nc.vector.tensor_scalar(
                    c_acc[:, 0, :], htt[:, 0, :], 1.0, rmask[:], op0=ALU.min, op1=ALU.mult
                )
                nc.vector.tensor_scalar(
                    c_acc[:, 1, :], htt[:, 1, :], 1.0, None, op0=ALU.min
                )
                nc.vector.tensor_scalar(
                    s_acc[:, 0, :], htt[:, 0, :], 1.0, rmask[:], op0=ALU.mult, op1=ALU.mult
                )
                nc.vector.tensor_copy(s_acc[:, 1, :], htt[:, 1, :])
                sc_first = False
            else:
                o = 0
                while o < tn:
                    nc.vector.scalar_tensor_tensor(
                        c_acc[:], htt[:, o : o + 2, :], 1.0, c_acc[:],
                        op0=ALU.min, op1=ALU.add,
                    )
                    if (ts_ + o) // 2 < SD:
                        nc.vector.tensor_add(s_acc[:], htt[:, o : o + 2, :], s_acc[:])
                    else:
                        s_tail.append((htt, o))
                    o += 2

        # layer-1 epilogue (bias-free: bias rode the matmul via the ones-row)
        for j in range(FO1T // 2):
            nc.scalar.activation(
                x1t[:, 2 * j : 2 * j + 2, :], ps1[j][:], AF.Lrelu,
                scale=w1_unscale, alpha=SLOPE,
            )

        # ---- remaining weights + movie factors: emitted late on the Sync
        # queue so the layer-1 ht/w1 stream gets the DMA bandwidth first.
        w_sb = {}
        for li in range(2, 7):
            fi, fo = DIMS[li - 2], DIMS[li - 1]
            wdt = f8 if li < 6 else bf16
            w_sb[li] = const.tile([128, fi // 128, fo], wdt, name=f"w{li}t", tag=f"w{li}t")
            nc.sync.dma_start(out=w_sb[li][:], in_=w_d[li][:])
        mft = const.tile([128, M], bf16, name="mft", tag="mft")
        nc.sync.dma_start(out=mft[:], in_=mft_d[:])

        # ---- layers 2..5 (lrelu) ----
        xin = x1t
        for li in range(2, 6):
            fi, fo = DIMS[li - 2], DIMS[li - 1]
            fit, fot = fi // 128, fo // 128
            xdt = f8 if li < 5 else bf16
            unsc = 1.0 / cfg["MLP_SCALES"][li - 2]
            xout = actp.tile(
                [128, fot, BS], xdt, name=f"x{li}t",
                tag=("x1t" if li == 5 else "x2t" if li == 4 else f"x{li}t"),
            )
            # interleave the k-accumulation across up to 4 fo-tile banks so
            # consecutive matmuls don't serialize on one bank's drain
            for g0 in range(0, fot, 4):
                gn = min(4, fot - g0)
                pss = [psp.tile([128, BS], f32, name="ps", tag="ps") for _ in range(gn)]
                ki = 0
                while ki < fit:
                    n2 = 2 if ki + 2 <= fit else 1
                    for j in range(gn):
                        ft = g0 + j
                        if n2 == 2:
                            nc.tensor.matmul(
                                pss[j][:],
                                lhsT=w_sb[li][:, ki : ki + 2, ft * 128 : (ft + 1) * 128],
                                rhs=xin[:, ki : ki + 2, :],
                                start=(ki == 0),
                                stop=(ki + 2 == fit),
                                perf_mode=mybir.MatmulPerfMode.DoubleRow,
                            )
                        else:
                            nc.tensor.matmul(
                                pss[j][:],
                                lhsT=w_sb[li][:, ki, ft * 128 : (ft + 1) * 128],
                                rhs=xin[:, ki, :],
                                start=(ki == 0),
                                stop=True,
                            )
                    ki += n2
                # epilogues all on the Scalar engine: the DVE queue carries
                # the s/cnt chains during layer 1 and would stall these
                # (strict FIFO), which in turn stalls the next layer's PE.
                for j in range(gn):
                    ft = g0 + j
                    nc.scalar.activation(
                        xout[:, ft, :], pss[j][:], AF.Lrelu,
                        bias=bp_sb[li][:, ft : ft + 1], scale=unsc, alpha=SLOPE,
                    )
            xin = xout

        # ---- gen_last early: a 1-column slice of layer 6 + tanh, emitted
        # right before the s-tail so addv doesn't wait for the full layer-6
        # matmul + tanh.  8 tiny matmuls (~60cyc each) + one [1,BS] ACT.
        fi6, fo6 = DIMS[4], DIMS[5]
        fit6 = fi6 // 128
        assert fo6 == F + 1
        ps6b = psp.tile([1, BS], f32, name="ps6b", tag="ps")
        for ki in range(fit6):
            nc.tensor.matmul(
                ps6b[:],
                lhsT=w_sb[6][:, ki, F : F + 1],
                rhs=xin[:, ki, :],
                start=(ki == 0),
                stop=(ki == fit6 - 1),
            )
        genl = const.tile([1, BS], f32, name="genl", tag="genl")
        nc.scalar.activation(genl[:], ps6b[:], AF.Tanh, bias=b6_sb[F : F + 1, 0:1], scale=1.0)

        # ---- cnt partition reduce + 1/cnt, emitted right after the mids:
        # c_acc is complete when the DVE chains drain (~layer-1 end), so the
        # slow DVE reciprocal runs far off the critical path, in the DVE's
        # idle window during the mids/s-tail.
        c_red = psp.tile([1, 2, BS], f32, name="c_red", tag="ps")
        nc.tensor.matmul(c_red[:, 0, :], lhsT=onesf[:], rhs=c_acc[:, 0, :], start=True, stop=True)
        nc.tensor.matmul(c_red[:, 1, :], lhsT=onesf[:], rhs=c_acc[:, 1, :], start=True, stop=True)
        # the host-appended ones-row counted +1 per batch (cnt chain half 0,
        # s PE-tail): subtract 1 from each.
        c0_sb = const.tile([1, BS], f32, name="c0_sb", tag="c0_sb")
        nc.vector.tensor_scalar_sub(c0_sb[:], c_red[0:1, 0, :], 1.0)
        c_sb = const.tile([1, BS], f32, name="c_sb", tag="c_sb")
        nc.vector.tensor_add(c_sb[:], c0_sb[:], c_red[0:1, 1, :])
        rc_sb = const.tile([1, BS], f32, name="rc_sb", tag="rc_sb")
        nc.vector.reciprocal(rc_sb[:], c_sb[:])

        # ---- PE s-tail: DoubleRow ones-chain over the deferred pair-steps
        # (history is long resident), accumulating into ONE PSUM bank; the
        # DVE-half partition reduces accumulate into row 0 of the same bank,
        # so the total s needs just one DVE op afterwards.
        NT = len(s_tail)
        assert NT >= 2
        scx = psp.tile([16, BS], f32, name="scx", tag="ps")
        for si, (htt, lo) in enumerate(s_tail):
            nc.tensor.matmul(
                scx[:], lhsT=ones16[:], rhs=htt[:, lo : lo + 2, :],
                start=(si == 0), stop=False,
                perf_mode=mybir.MatmulPerfMode.DoubleRow,
                skip_group_check=True,
            )
        nc.tensor.matmul(
            scx[0:1, :], lhsT=onesf[:], rhs=s_acc[:, 0, :], start=False, stop=False,
            skip_group_check=True,
        )
        nc.tensor.matmul(
            scx[0:1, :], lhsT=onesf[:], rhs=s_acc[:, 1, :], start=False, stop=True,
            skip_group_check=True,
        )

        # ---- layer 6 (tanh) -> genf [65, BS] f32 ----
        fi, fo = DIMS[4], DIMS[5]
        fit = fi // 128
        assert fo == F + 1
        ps6 = psp.tile([fo, BS], f32, name="ps6", tag="ps")
        for ki in range(fit):
            nc.tensor.matmul(
                ps6[:],
                lhsT=w_sb[6][:, ki, 0:fo],
                rhs=xin[:, ki, :],
                start=(ki == 0),
                stop=(ki == fit - 1),
            )
        genf = actp.tile([fo, BS], f32, name="genf", tag="genf")
        nc.scalar.activation(genf[:], ps6[:], AF.Tanh, bias=b6_sb[0:fo, 0:1], scale=1.0)

        # ---- genext2: factor rows in bf16, duplicated into partitions
        # 64..127 (via SBUF->SBUF DMA) so reconstruction can row-tile the
        # PE.  Emitted BEFORE the meanV combine chain: the DVE queue is
        # strict FIFO and the cast gates the first reconstruction matmul.
        genext2 = actp.tile([128, BS], bf16, name="genext2", tag="genext")
        nc.vector.tensor_copy(genext2[0:F, :], genf[0:F, :])
        nc.sync.dma_start(out=genext2[F : 2 * F, :], in_=genext2[0:F, :])

        # s pre-scaled by OSC (ones-row subtracted) -> meanV -> addv
        s_sb = const.tile([1, BS], f32, name="s_sb", tag="s_sb")
        nc.vector.tensor_scalar(
            s_sb[:], scx[0:1, :], -1.0, float(OSC), op0=ALU.add, op1=ALU.mult
        )
        mv_sb = const.tile([1, BS], f32, name="mv_sb", tag="mv_sb")
        nc.vector.tensor_mul(mv_sb[:], rc_sb[:], s_sb[:])
        av_sb = const.tile([1, BS], f32, name="av_sb", tag="av_sb")
        nc.vector.scalar_tensor_tensor(
            av_sb[:], genl[:], float(OSC), mv_sb[:], op0=ALU.mult, op1=ALU.add
        )
        # bounce through DRAM into per-partition layout [128, NBT] for the
        # staging ops (row b -> partition b%128, column b//128)
        nc.sync.dma_start(out=av_d[:], in_=av_sb[0:1, :])
        addv_t = const.tile([128, NBT], f32, name="addv_t", tag="addv_t")
        nc.sync.dma_start(out=addv_t[:], in_=av_d.ap().rearrange("(t p) -> p t", p=128))

        # ---- reconstruction: out[bt*128+p, m] over movie chunk-pairs.
        # PE in 64x128 row-tiled mode: tile T0 (SBUF partitions 0-63) runs
        # the even chunk, tile T8 (64-127, the duplicated factor rows) the
        # odd chunk CONCURRENTLY, into adjacent PSUM banks.
        PAIRS = [CHUNKS[i : i + 2] for i in range(0, len(CHUNKS), 2)]
        for bt in range(NBT):
            lhsT_lo = genext2[0:F, bt * 128 : (bt + 1) * 128]
            lhsT_hi = genext2[F : 2 * F, bt * 128 : (bt + 1) * 128]
            st = None
            for pi, pair in enumerate(PAIRS):
                # staging alternates Scalar/Vector, time-balanced ~5:4
                # (GpSimd cannot read PSUM, so no third stager exists)
                eng = 0 if (pi % 9) in (0, 2, 4, 6, 8) else 1
                pr = psp.tile([128, 2, 512], f32, name="pr", tag="ps")
                for j, (co, cw) in enumerate(pair):
                    nc.tensor.matmul(
                        pr[:, j, 0:cw],
                        lhsT=(lhsT_lo if j == 0 else lhsT_hi),
                        rhs=(mft[0:F, co : co + cw] if j == 0 else mft[F : 2 * F, co : co + cw]),
                        start=True, stop=True,
                    )
                pw = sum(cw for _, cw in pair)
                if pi % 2 == 0:
                    st = ost.tile([128, 2048], odt, name="st", tag="st")
                    so, po = 0, pair[0][0]
                # stage only the valid width (last pair is 544, not 1024)
                nst = 1024 if pw == 1024 else 512 + pair[1][1]
                pr2d = pr[:].opt()[:, 0:nst]  # contiguous [128, nst]
                if eng == 0:
                    nc.scalar.activation(
                        st[:, so : so + nst], pr2d, AF.Identity,
                        bias=addv_t[:, bt : bt + 1], scale=OSC,
                    )
                else:
                    nc.vector.tensor_scalar(
                        st[:, so : so + nst], pr2d, OSC, addv_t[:, bt : bt + 1],
                        op0=ALU.mult, op1=ALU.add,
                    )
                so += nst
                if pi % 2 == 1 or pi == len(PAIRS) - 1:
                    # output drains alternate between the Sync and GpSimd
                    # DMA queues (both engines idle during reconstruction;
                    # descriptor issue costs ~0.6us each)
                    q = nc.sync if (pi // 2) % 2 == 0 else nc.gpsimd
                    q.dma_start(
                        out=out_d[bt * 128 : (bt + 1) * 128, po : po + so],
                        in_=st[:, 0:so],
                    )

    nc.compile()
    return nc


def prep_in_maps(cfg, inputs):
    """Shard + lay out the full inputs into per-core DRAM input maps."""
    d = _derived(cfg)
    BS, UINFO, M, F, DIMS, T1 = cfg["BS"], cfg["UINFO"], cfg["M"], cfg["F"], cfg["DIMS"], d["T1"]
    extInfo = np.asarray(inputs["extInfo"], np.float32)
    ratings = np.asarray(inputs["ratings"], np.float32)

    # BN (eval) fold into layer 2: y = g'(lrelu1) + b' with g' = bn_g/sqrt(1+eps)
    g = np.asarray(inputs["bn_g"], np.float32) / np.float32(np.sqrt(1.0 + BN_EPS))
    bnb = np.asarray(inputs["bn_b"], np.float32)
    w2 = np.asarray(inputs["w2"], np.float32)
    w2f = w2 * g[None, :]
    b2f = np.asarray(inputs["b2"], np.float32) + w2 @ bnb

    shared = {}
    # w1t: [KH,FO1] -> padded [T1*128, FO1] -> [128, T1, FO1]; the row at
    # index UINFO+M carries b1 (matching the ones-row in the history).
    w1 = np.asarray(inputs["w1"], np.float32)
    b1 = np.asarray(inputs["b1"], np.float32)
    FO1 = DIMS[0]
    w1tp = np.zeros((T1 * 128, FO1), FP8)
    w1tp[0 : w1.shape[1]] = (w1.T * np.float32(cfg["W1_SCALE"])).astype(FP8)
    w1tp[UINFO + M] = (b1 * np.float32(cfg["W1_SCALE"])).astype(FP8)
    shared["w1t"] = np.ascontiguousarray(w1tp.reshape(T1, 128, FO1).transpose(1, 0, 2))

    def pack_w(wT, fo, dt=BF16, scale=1.0):
        fi = wT.shape[0]
        w = (wT.astype(np.float32) * np.float32(scale)).astype(dt)
        return np.ascontiguousarray(w.reshape(fi // 128, 128, fo).transpose(1, 0, 2))

    scs = cfg["MLP_SCALES"]
    shared["w2t"] = pack_w(w2f.T, DIMS[1], FP8, scs[0])
    for li, wname in ((3, "w3"), (4, "w4"), (5, "w5"), (6, "w6")):
        w = np.asarray(inputs[wname], np.float32)
        fo = DIMS[li - 1]
        if li < 6:
            shared[f"w{li}t"] = pack_w(w.T, fo, FP8, scs[li - 2])
        else:
            shared[f"w{li}t"] = pack_w(w.T, fo)

    def pack_b(b, fo):
        fot = math.ceil(fo / 128)
        bp = np.zeros(fot * 128, np.float32)
        bp[:fo] = b
        return np.ascontiguousarray(bp.reshape(fot, 128).T)

    bsrc = {2: b2f}
    for li in (3, 4, 5):
        bsrc[li] = np.asarray(inputs[f"b{li}"], np.float32)
    for li in range(2, 6):
        shared[f"b{li}p"] = pack_b(bsrc[li], DIMS[li - 1])
    shared["b6p"] = pack_b(np.asarray(inputs["b6"], np.float32), DIMS[5])

    # bf16 mft: factor rows duplicated into partitions 64..127 for the
    # row-tiled reconstruction (movie_bias is added on host at dequant).
    mft = np.zeros((128, M), BF16)
    mft[0:F] = np.asarray(inputs["movie_factors"], np.float32).T.astype(BF16)
    mft[F : 2 * F] = mft[0:F]
    shared["mft"] = mft

    in_maps = []
    for c in range(NCORES):
        sl = slice(c * BS, (c + 1) * BS)
        htc = np.zeros((T1 * 128, BS), FP8)
        htc[0:UINFO] = extInfo[sl].T.astype(FP8)
        htc[UINFO : UINFO + M] = ratings[sl].T.astype(FP8)
        htc[UINFO + M] = np.float32(1.0)  # ones-row: picks up b1 from w1t
        m = dict(shared)
        m["ht"] = np.ascontiguousarray(htc.reshape(T1, 128, BS).transpose(1, 0, 2))
        in_maps.append(m)
    return in_maps


_NC_CACHE = {}


def run_on_hw(cfg, inputs, trace=False):
    from concourse.bass_utils import run_bass_kernel_spmd

    key = tuple(sorted((k, v) for k, v in cfg.items() if k != "DIMS")) + (cfg["DIMS"],)
    if key not in _NC_CACHE:
        _NC_CACHE[key] = build_nc(cfg)
    nc = _NC_CACHE[key]
    in_maps = prep_in_maps(cfg, inputs)
    br = run_bass_kernel_spmd(nc, in_maps, list(range(NCORES)), trace=trace)
    BS, M = cfg["BS"], cfg["M"]
    out = np.empty((NCORES * BS, M), np.float32)
    dq = np.float32(1.0 / cfg["OUT_SCALE"]) if cfg["OUT_DT"] == "i8" else np.float32(1.0)
    mb = np.asarray(inputs["movie_bias"], np.float32)[None, :]
    for c in range(NCORES):
        out[c * BS : (c + 1) * BS] = (
            np.asarray(br.results[c]["out"], dtype=np.float32) * dq + mb
        )
    return out, br


def kernel(**inputs) -> np.ndarray:
    try:
        out, _ = run_on_hw(FULL_CFG, inputs, trace=False)
    except Exception:
        # one retry for transient device/runtime hiccups
        out, _ = run_on_hw(FULL_CFG, inputs, trace=False)
    return out
